# revision 21
# baseline (speedup 1.0000x reference)
"""Trainium2 Bass kernel for nn_BaseGenerator (4-layer dense transformer).

Strategy: pure data-parallel over batch (B=8 -> 8 NeuronCores, no
collectives).  Each core runs the full transformer on one batch element.

v2 scheme (cost model: fp8-e4m3 DoubleRow = 0.5 cycle/col, K=256/instr):
  - ALL GEMMs (QKV, Wo, FFN1, FFN2, head) fp8 DR with 3-term error
    compensation (w8*x8 + dw8*x8 + w8*dx8); weights pre-scaled by 2^8.
  - q/k/at stored fp8; scores fp8 non-DR (1 cyc/col); AV fp8 DR with
    v-residual compensation; softmax via exp + ones-row denominator.
  - attention causally trimmed: mask idz-matmuls / scores / exp / AV only
    cover q >= k-block (masked region written -240 by the idz matmul).
  - LN statistics via fp8-DR ones-matmuls on quantized r8/sq8 pair tiles.
  - token-half software pipelining (A=[0,256) B=[256,512)): GEMM halves are
    emitted with a lag so each half's LN vector chain overlaps the other
    half's matmuls; PE stays busy across LN boundaries.
  - embedding gather and the final-head affine fold (mean*rstd*rowsum(Wp)
    + genW@lnf_b + gen_b) are host-side; device ships f16 rstd-scaled
    logits + an f32 mean*rstd row.
"""

import os
import sys

for _p in ("/opt/trn_rl_repo",):
    if _p not in sys.path:
        sys.path.insert(0, _p)

import ml_dtypes
import numpy as np

import concourse.bass as bass
import concourse.mybir as mybir
import concourse.tile as tile
from concourse import bacc
from concourse.bass_utils import run_bass_kernel_spmd

F16 = np.float16
F8 = ml_dtypes.float8_e4m3

L, E, H, F = 4, 1024, 16, 4096
B, S = 8, 512
SH = S // 2  # half width 256
VV, VR = 40, 30
DIST_V = 200
PAD_ID = 0
DH = E // H  # 64
NE = E // 128  # 8 feature chunks
NEP = NE // 2  # 4 fp8 pair tiles
NF = F // 128  # 32
NO = 10  # logit row tiles (1280 padded)
NEG = -240.0

WSC = 256.0
DEQ = 1.0 / WSC

f32 = mybir.dt.float32
f16 = mybir.dt.float16
f8 = mybir.dt.float8e4
AF = mybir.ActivationFunctionType
OP = mybir.AluOpType
DR = mybir.MatmulPerfMode.DoubleRow

_CACHE = {}

HALVES = (slice(0, SH), slice(SH, S))


# ----------------------------------------------------------------------------
# host-side input prep
# ----------------------------------------------------------------------------

def _f8(x):
    return np.ascontiguousarray(np.asarray(x, np.float32).astype(F8))


def _f16(x):
    return np.ascontiguousarray(np.asarray(x, np.float32).astype(F16))


def _block_dr2(W, gsize):
    """W [O, I] -> fp8 (w8, dw8) blocks [G, 128, I//256, 2(i), 2(t), gsize]
    with [g, p, cp, i, t, o] <- W[g*gsize + o, cp*256 + i*128 + p]*WSC,
    t=0: e4m3 quant, t=1: e4m3 residual."""
    O, I = W.shape
    Ws = np.asarray(W, np.float32) * WSC
    w8 = Ws.astype(F8).astype(np.float32)
    dw8 = (Ws - w8).astype(F8).astype(np.float32)
    out = np.empty((O // gsize, 128, I // 256, 2, 2, gsize), F8)
    for t, wv in enumerate((w8, dw8)):
        Wb = wv.reshape(O // gsize, gsize, I // 256, 2, 128)  # g o cp i p
        out[:, :, :, :, t, :] = Wb.transpose(0, 4, 2, 3, 1).astype(F8)
    return np.ascontiguousarray(out)


def _pp(v):  # [..., N*128] -> [..., 128, N]
    *lead, N = v.shape
    return np.ascontiguousarray(
        v.reshape(*lead, N // 128, 128).swapaxes(-1, -2).astype(np.float32)
    )


def _prep_shared(inp):
    out = {}

    Wqkv = np.asarray(inp["Wqkv"], np.float32).copy()  # [L, 3E, E]
    bqkv = np.asarray(inp["bqkv"], np.float32).copy()  # [L, 3E]
    att_sc = 1.0 / np.sqrt(DH)
    bqkv[:, :E] *= att_sc

    out["wqkv"] = np.stack([_block_dr2(Wqkv[l], 512) for l in range(L)])
    Wo = np.asarray(inp["Wo"], np.float32)
    out["wo8"] = np.stack([_block_dr2(Wo[l], 512) for l in range(L)])
    W1 = np.asarray(inp["W1"], np.float32)
    out["w1"] = np.stack([_block_dr2(W1[l], 256) for l in range(L)])
    W2 = np.asarray(inp["W2"], np.float32)
    out["w2"] = np.stack([_block_dr2(W2[l], 128) for l in range(L)])

    genW = np.asarray(inp["gen_W"], np.float32)  # [1200, E]
    lnf_s_v = np.asarray(inp["lnf_s"], np.float32)
    lnf_b_v = np.asarray(inp["lnf_b"], np.float32)
    Wp = genW * lnf_s_v[None, :]
    Wp_pad = np.zeros((1280, E), np.float32)
    Wp_pad[:1200] = Wp
    out["genw8"] = _block_dr2(Wp_pad, 256)  # [5, 128, 4, 2, 2, 256]
    _CACHE["nws"] = -Wp.sum(1)
    _CACHE["gb"] = genW @ lnf_b_v + np.asarray(inp["gen_b"], np.float32)

    out["bqkv_pp"] = _pp(bqkv[:, : 2 * E])  # [L, 128, 16]
    out["bv_row"] = _f16(bqkv[:, 2 * E:].reshape(L, 1, E))  # [L, 1, E]
    out["bo_pp"] = _pp(np.asarray(inp["bo"], np.float32))
    out["b1_pp"] = _pp(np.asarray(inp["b1"], np.float32))
    out["b2_pp"] = _pp(np.asarray(inp["b2"], np.float32))

    ln_s = np.stack([np.asarray(inp["ln1_s"], np.float32),
                     np.asarray(inp["ln2_s"], np.float32)], 1)
    ln_b = np.stack([np.asarray(inp["ln1_b"], np.float32),
                     np.asarray(inp["ln2_b"], np.float32)], 1)
    out["ln_s_pp"] = _pp(ln_s)  # [L, 2, 128, 8]
    out["ln_b_pp"] = _pp(ln_b)

    idz = np.zeros((128, 2, 2, 128), np.float32)
    for v in range(2):
        idz[:, v, v, :] = np.eye(128)
    out["idz"] = _f8(idz)
    out["ones8"] = _f8(np.ones((128, 2, 128), np.float32))
    return out


def _prep_percore(inp):
    val = np.asarray(inp["val_sequences"]).astype(np.int64)
    ring = np.asarray(inp["ring_sequences"]).astype(np.int64)
    dist = np.asarray(inp["distance_squares"]).astype(np.int64)
    de = np.asarray(inp["dist_emb"], np.float32)  # [200, H]

    # embedding (f16 tables, f32 math, f16 result) == device one-hot matmul
    ve = np.asarray(inp["val_emb"], np.float32).astype(F16).astype(np.float32)
    re = np.asarray(inp["ring_emb"], np.float32).astype(F16).astype(np.float32)
    h = (ve[val] + re[ring]) * np.sqrt(E)  # [B, S, E] f32
    hB = h.reshape(B, S, NE, 128).transpose(0, 2, 3, 1)  # [B, NE, 128, S]
    hinit = np.ascontiguousarray(hB.astype(F16))

    m = de[dist].astype(F8).astype(np.float32)  # [B, q, k, H]
    m = m.transpose(0, 3, 2, 1)          # [B, H, k, q]
    kk = np.arange(S)
    causal = kk[:, None] <= kk[None, :]  # keep k <= q
    m = np.where(causal[None, None], m, NEG)
    padk = val == PAD_ID
    m = np.where(padk[:, None, :, None], NEG, m)
    # -> [B, H, 2(qh), 128(p), 4(kc), SH(qi)]: k = kc*128+p, q = qh*256+qi
    m = m.reshape(B, H, 4, 128, 2, SH).transpose(0, 1, 4, 3, 2, 5)
    m = np.ascontiguousarray(m.astype(F8))

    return [{"mask": m[b], "hinit": hinit[b]} for b in range(B)]


# ----------------------------------------------------------------------------
# device program
# ----------------------------------------------------------------------------

def _declare(nc):
    d = {}

    def di(name, shape, dt):
        d[name] = nc.dram_tensor(name, list(shape), dt, kind="ExternalInput").ap()

    di("wqkv", (L, 6, 128, 4, 2, 2, 512), f8)
    di("wo8", (L, 2, 128, 4, 2, 2, 512), f8)
    di("w1", (L, 16, 128, 4, 2, 2, 256), f8)
    di("w2", (L, 8, 128, 16, 2, 2, 128), f8)
    di("genw8", (5, 128, 4, 2, 2, 256), f8)
    di("bqkv_pp", (L, 128, 16), f32)
    di("bv_row", (L, 1, E), f16)
    di("bo_pp", (L, 128, 8), f32)
    di("b1_pp", (L, 128, 32), f32)
    di("b2_pp", (L, 128, 8), f32)
    di("ln_s_pp", (L, 2, 128, 8), f32)
    di("ln_b_pp", (L, 2, 128, 8), f32)
    di("idz", (128, 2, 2, 128), f8)
    di("ones8", (128, 2, 128), f8)
    di("mask", (H, 2, 128, 4, SH), f8)
    di("hinit", (NE, 128, S), f16)
    d["logits"] = nc.dram_tensor("logits", [NO, 128, S], f16,
                                 kind="ExternalOutput").ap()
    if os.environ.get("BG_DEBUG"):
        def do(name, shape, dt=f16):
            d[name] = nc.dram_tensor(name, list(shape), dt,
                                     kind="ExternalOutput").ap()
        do("dbg_h0", (128, S))
        do("dbg_hp0", (128, 2, S), f8)
        do("dbg_q0", (128, S), f8)
        do("dbg_k0", (128, S), f8)
        do("dbg_v0", (128, 2, H, DH + 1), f8)
        do("dbg_atA", (128, 1, 2, SH), f8)
        do("dbg_atB", (128, 2, 2, SH), f8)
        do("dbg_cx0", (128, 2, S), f8)
        do("dbg_r1", (128, S))
        do("dbg_h1", (128, S))
        do("dbg_ff0", (128, 2, S), f8)
        do("dbg_r2", (128, S))
    d["mrstd"] = nc.dram_tensor("mrstd", [1, S], f32,
                                kind="ExternalOutput").ap()
    return d


class Ctx:
    pass


def _emit(nc, tc, d, ctx):
    hw = nc.sync
    g = Ctx()
    g.nc = nc
    g.d = d

    def pool(name, bufs, space="SBUF"):
        return ctx.enter_context(
            tc.tile_pool(name=name, bufs=bufs, space=space))

    g.wpool = pool("wpool", 3)      # 8KB: wqkv + wo blocks
    g.w1pool = pool("w1pool", 5)    # 4KB: ffn1 blocks (lag-3 span)
    g.w2pool = pool("w2pool", 2)    # 8KB: ffn2 blocks
    g.gwpool = pool("gwpool", 1)    # 4KB: head blocks
    g.maskpool = pool("maskpool", 2)
    g.hpool = pool("hpool", 16)   # h/h1/h2 residual tiles
    g.rpool = pool("rpool", 8)     # r1/r2 LN-input tiles
    g.hppool = pool("hppool", 8)
    g.dhppool = pool("dhppool", 8)
    g.r8pool = pool("r8pool", 4)
    g.qkpool = pool("qkpool", 16)   # f8 [128,S]
    g.vpool = pool("vpool", 2)
    g.atpool = pool("atpool", 6)
    g.ctxpool = pool("ctxpool", 2)
    g.cxppool = pool("cxppool", 4)
    g.ffpool = pool("ffpool", 16)
    g.tmppool = pool("tmppool", 4)
    g.smallf = pool("smallf", 8)    # [1, SH]
    g.srec = pool("srec", 2)        # [1, SH] f32 softmax denom
    g.recpool = pool("recpool", 2)  # [DH, SH] f32
    g.bcpool = pool("bcpool", 3)    # [128, S] f16 broadcasts
    g.outpool = pool("outpool", 2)
    g.pppool = pool("pppool", 4)
    g.bvpool = pool("bvpool", 1)

    g.ps_gemm = pool("ps_gemm", 4, "PSUM")
    g.ps_att = pool("ps_att", 2, "PSUM")

    cpool = pool("cpool", 1)
    g.idz = cpool.tile([128, 2, 2, 128], f8)
    hw.dma_start(out=g.idz, in_=d["idz"])
    g.ones8 = cpool.tile([128, 2, 128], f8)
    hw.dma_start(out=g.ones8, in_=d["ones8"])
    g.eps_t = cpool.tile([128, 1], f32)
    nc.vector.memset(g.eps_t, 1e-5)

    # --- embedding: DMA h_init, quantize pairs -------------------------------
    with nc.named_scope("embed"):
        h_t = []
        for c in range(NE):
            ht = g.hpool.tile([128, S], f16, tag="h", name=f"h0_{c}")
            hw.dma_start(out=ht, in_=d["hinit"][c])
            h_t.append(ht)
        hp_t = [g.hppool.tile([128, 2, S], f8, tag="hp", name=f"ehp{j}")
                for j in range(NEP)]
        dhp_t = [g.dhppool.tile([128, 2, S], f8, tag="dhp", name=f"edhp{j}")
                 for j in range(NEP)]
        for X in HALVES:
            for j in range(NEP):
                for i in range(2):
                    nc.gpsimd.tensor_copy(hp_t[j][:, i, X],
                                          h_t[2 * j + i][:, X])
                    nc.gpsimd.tensor_sub(dhp_t[j][:, i, X],
                                         h_t[2 * j + i][:, X],
                                         hp_t[j][:, i, X])

    if "dbg_h0" in d:
        hw.dma_start(out=d["dbg_h0"], in_=h_t[0])
        hw.dma_start(out=d["dbg_hp0"], in_=hp_t[0])
    for l in range(L):
        h_t, hp_t, dhp_t = _layer(g, l, h_t, hp_t, dhp_t)

    with nc.named_scope("final"):
        _final(g, h_t, hp_t, dhp_t)


def _gemm3(g, ps, wt, mi, xp, dxp, X, ncp=4, gs=128):
    """3-term fp8 DR chain into ps[:, X]; wt [128, ncp, 2, 2, G] blocks,
    mi slices gs output columns."""
    mm = g.nc.tensor.matmul
    sl = slice(mi * gs, (mi + 1) * gs)
    for cp in range(ncp):
        w8 = wt[:, cp, :, 0, sl]
        dw8 = wt[:, cp, :, 1, sl]
        mm(ps[:, X], w8, xp[cp][:, :, X], start=(cp == 0), stop=False,
           perf_mode=DR)
        mm(ps[:, X], dw8, xp[cp][:, :, X], start=False, stop=False,
           perf_mode=DR)
        mm(ps[:, X], w8, dxp[cp][:, :, X], start=False, stop=(cp == ncp - 1),
           perf_mode=DR)


def _layer(g, l, h_t, hp_t, dhp_t):
    nc = g.nc
    d = g.d
    mm = nc.tensor.matmul
    hw = nc.sync

    # per-layer small params
    bqkv_pp = g.pppool.tile([128, 16], f32, tag="pp16", bufs=2)
    hw.dma_start(out=bqkv_pp, in_=d["bqkv_pp"][l])
    bo_pp = g.pppool.tile([128, 8], f32, tag="pp8", bufs=12)
    hw.dma_start(out=bo_pp, in_=d["bo_pp"][l])
    b1_pp = g.pppool.tile([128, 32], f32, tag="pp32", bufs=2)
    hw.dma_start(out=b1_pp, in_=d["b1_pp"][l])
    b2_pp = g.pppool.tile([128, 8], f32, tag="pp8", bufs=12)
    hw.dma_start(out=b2_pp, in_=d["b2_pp"][l])
    ln_s = [g.pppool.tile([128, 8], f32, tag="pp8", bufs=12,
                          name=f"lns{l}_{i}") for i in range(2)]
    ln_b = [g.pppool.tile([128, 8], f32, tag="pp8", bufs=12,
                          name=f"lnb{l}_{i}") for i in range(2)]
    for i in range(2):
        hw.dma_start(out=ln_s[i], in_=d["ln_s_pp"][l, i])
        hw.dma_start(out=ln_b[i], in_=d["ln_b_pp"][l, i])
    bvb = g.bvpool.tile([128, E], f16, tag="bvb", name=f"bvb{l}")
    hw.dma_start(out=bvb, in_=d["bv_row"][l].to_broadcast((128, E)))

    wq_sb = {}

    def load_wqkv(gi):
        if gi not in wq_sb:
            wt = g.wpool.tile([128, 4, 2, 2, 512], f8, tag="w",
                              name=f"wqkv{l}_{gi}")
            hw.dma_start(out=wt, in_=d["wqkv"][l, gi])
            wq_sb[gi] = wt
        return wq_sb[gi]

    qk_t = [g.qkpool.tile([128, S], f8, tag="qk", name=f"qk{l}_{t}")
            for t in range(16)]  # q 0..7, k 8..15
    v_t = []
    dv_t = []
    for kcp in range(2):
        vt = g.vpool.tile([128, 2, H, DH + 1], f8, tag="v", name=f"v{l}_{kcp}")
        nc.vector.memset(vt[:, :, :, DH:DH + 1], 1.0)
        v_t.append(vt)
        dvt = g.vpool.tile([128, 2, H, DH + 1], f8, tag="dv",
                           name=f"dv{l}_{kcp}")
        nc.vector.memset(dvt[:, :, :, DH:DH + 1], 0.0)
        dv_t.append(dvt)

    def qkv_chain(gi, mi, X):
        wt = load_wqkv(gi)
        mt = gi * 4 + mi
        ps = g.ps_gemm.tile([128, S], f32, tag="gemm", name=f"qkps{l}")
        _gemm3(g, ps, wt, mi, hp_t, dhp_t, X)
        if gi < 2:  # Q
            nc.scalar.activation(qk_t[mt][:, X], ps[:, X], AF.Identity,
                                 bias=bqkv_pp[:, mt:mt + 1],
                                 scale=float(DEQ / np.sqrt(DH)))
        else:  # K on DVE
            nc.vector.tensor_scalar(qk_t[mt][:, X], ps[:, X], float(DEQ),
                                    bqkv_pp[:, mt:mt + 1], OP.mult, OP.add)

    def v_chain(gi, n):
        wt = load_wqkv(4 + gi)
        ps = g.ps_gemm.tile([128, S], f32, tag="gemm", name=f"vps{l}")
        for cp in range(4):
            xs = hp_t[cp][:, :, n * 128:(n + 1) * 128]
            dxs = dhp_t[cp][:, :, n * 128:(n + 1) * 128]
            w8 = wt[:, cp, :, 0, :]
            dw8 = wt[:, cp, :, 1, :]
            mm(ps, xs, w8, start=(cp == 0), stop=False, perf_mode=DR)
            mm(ps, xs, dw8, start=False, stop=False, perf_mode=DR)
            mm(ps, dxs, w8, start=False, stop=(cp == 3), perf_mode=DR)
        tmp = g.tmppool.tile([128, S], f16, tag="vtmp", bufs=2,
                             name=f"vtmp{l}_{gi}_{n}")
        nc.vector.scalar_tensor_tensor(
            tmp, ps, float(DEQ), bvb[:, gi * 512:(gi + 1) * 512],
            OP.mult, OP.add)
        v8s = v_t[n // 2][:, n % 2, gi * 8:(gi + 1) * 8, 0:DH]
        tmpr = tmp.rearrange("p (a b) -> p a b", a=8)
        nc.gpsimd.tensor_copy(v8s, tmpr)
        nc.gpsimd.tensor_sub(dv_t[n // 2][:, n % 2, gi * 8:(gi + 1) * 8, 0:DH],
                             tmpr, v8s)

    # --- attention -----------------------------------------------------------
    ctx16 = {}
    cx_t = [g.cxppool.tile([128, 2, S], f8, tag="cx", name=f"cx{l}_{j}")
            for j in range(NEP)]
    dcx_t = [g.cxppool.tile([128, 2, S], f8, tag="dcx", name=f"dcx{l}_{j}")
             for j in range(NEP)]

    def emit_attn_half(hx, filler):
        X = HALVES[hx]
        nkcp = 1 + hx
        at_q = {}

        def emit_scores(h):
            qt = qk_t[h // 2]
            kt = qk_t[8 + h // 2]
            r0 = (h % 2) * DH
            mt_ = g.maskpool.tile([128, 2 * nkcp, SH], f8,
                                  tag=("mA", "mB")[hx], bufs=2,
                                  name=f"mk{l}_{hx}_{h}")
            hw.dma_start(out=mt_, in_=d["mask"][h, hx, :, 0:2 * nkcp, :])
            spsf = g.ps_att.tile([128, 2, 2, SH], f32, tag="att",
                                 name=f"s{l}_{hx}_{h}")
            sps = spsf[:, 0:nkcp]
            for kcp in range(nkcp):
                mrh = mt_[:, 2 * kcp:2 * kcp + 2, :]
                for kci in range(2):
                    kc = kcp * 2 + kci
                    qlo = max(kc * 128, X.start)
                    mm(sps[:, kcp, kci, :], g.idz[:, kci], mrh,
                       start=True, stop=False, perf_mode=DR)
                    mm(sps[:, kcp, kci, qlo - X.start:],
                       kt[r0:r0 + DH, kc * 128:(kc + 1) * 128],
                       qt[r0:r0 + DH, qlo:X.stop], start=False, stop=True)
            at = g.atpool.tile([128, nkcp, 2, SH], f8,
                               tag=("atA", "atB")[hx], bufs=3,
                               name=f"a{l}_{hx}_{h}")
            nc.scalar.activation(at, sps, AF.Exp)
            if l == 0 and h == 0 and f"dbg_at{'AB'[hx]}" in d:
                hw.dma_start(out=d[f"dbg_at{'AB'[hx]}"], in_=at)
            at_q[h] = at

        def emit_av(h):
            at = at_q.pop(h)
            cps = g.ps_gemm.tile([128, S], f32, tag="gemm", name=f"c{l}_{hx}")
            for kcp in range(nkcp):
                mm(cps[0:DH + 1, X], v_t[kcp][:, :, h, :], at[:, kcp],
                   start=(kcp == 0), stop=False, perf_mode=DR)
                mm(cps[0:DH + 1, X], dv_t[kcp][:, :, h, :], at[:, kcp],
                   start=False, stop=(kcp == nkcp - 1), perf_mode=DR)
            srow = g.srec.tile([1, SH], f32, tag="sw", bufs=2,
                               name=f"sw{l}_{hx}_{h}")
            nc.vector.tensor_copy(srow, cps[DH:DH + 1, X])
            rec = g.srec.tile([1, SH], f32, tag="sr", name=f"re{l}_{hx}_{h}")
            nc.vector.reciprocal_approx_fast(out=rec, in_=srow)
            recR = g.recpool.tile([DH, SH], f32, tag="recf",
                                  name=f"rr{l}_{hx}_{h}")
            nc.gpsimd.partition_broadcast(recR, rec, channels=DH)
            j, i = h // 4, (h // 2) % 2
            hh = h % 2
            if (j, i) not in ctx16:
                ctx16[(j, i)] = g.ctxpool.tile([128, S], f16, tag="ctx",
                                               name=f"cxt{l}_{hx}_{j}_{i}")
            nc.vector.tensor_mul(ctx16[(j, i)][hh * DH:(hh + 1) * DH, X],
                                 cps[0:DH, X], recR)
            if hh == 1:
                ct = ctx16.pop((j, i))
                nc.gpsimd.tensor_copy(cx_t[j][:, i, X], ct[:, X])
                nc.gpsimd.tensor_sub(dcx_t[j][:, i, X], ct[:, X],
                                     cx_t[j][:, i, X])

        fi = iter(filler)
        emit_scores(0)
        emit_scores(1)
        for h in range(2, H):
            emit_scores(h)
            emit_av(h - 2)
            for fn in (next(fi, None), next(fi, None)):
                if fn is not None:
                    fn()
        emit_av(H - 2)
        emit_av(H - 1)
        for fn in fi:
            if fn is not None:
                fn()

    # --- out-proj ------------------------------------------------------------
    wo_sb = {}

    def load_wo(gi):
        if gi not in wo_sb:
            wt = g.wpool.tile([128, 4, 2, 2, 512], f8, tag="w",
                              name=f"wo{l}_{gi}")
            hw.dma_start(out=wt, in_=d["wo8"][l, gi])
            wo_sb[gi] = wt
        return wo_sb[gi]

    r1_t = [g.rpool.tile([128, S], f16, tag="r", name=f"r1_{l}_{mt}")
            for mt in range(NE)]

    def wo_chain(gi, mi, X):
        wt = load_wo(gi)
        mt = gi * 4 + mi
        ps = g.ps_gemm.tile([128, S], f32, tag="gemm", name=f"wops{l}")
        _gemm3(g, ps, wt, mi, cx_t, dcx_t, X)
        wo_o = g.tmppool.tile([128, SH], f16, tag="woo", bufs=2,
                              name=f"woo{l}_{mt}")
        nc.scalar.activation(wo_o, ps[:, X], AF.Identity,
                             bias=bo_pp[:, mt:mt + 1], scale=DEQ)
        nc.vector.tensor_add(r1_t[mt][:, X], wo_o, h_t[mt][:, X])

    # --- emit: QKV (A leads, B lags 2 blocks); attention; Wo -----------------
    with nc.named_scope(f"L{l}_qkv"):
        # A-g0, A-g1, B-g0, A-g2, B-g1, A-g3, B-g2, B-g3, then V half A;
        # V half B (token blocks 2,3) is emitted as attn_A filler.
        seq = [("qk", 0, 0), ("qk", 1, 0), ("qk", 0, 1), ("qk", 2, 0),
               ("qk", 1, 1), ("qk", 3, 0), ("qk", 2, 1), ("qk", 3, 1),
               ("v", 0, 0), ("v", 0, 1), ("v", 1, 0), ("v", 1, 1)]
        for kind, gi, b in seq:
            if kind == "qk":
                for mi in range(4):
                    qkv_chain(gi, mi, HALVES[b])
            else:
                v_chain(gi, b)

    with nc.named_scope(f"L{l}_attn"):
        load_wo(0)
        load_wo(1)
        emit_attn_half(0, [lambda gi=gi, n=n: v_chain(gi, n)
                           for gi in range(2) for n in (2, 3)])
        emit_attn_half(1, [None] * 6 +
                       [lambda gi=gi, mi=mi: wo_chain(gi, mi, HALVES[0])
                        for gi in range(2) for mi in range(4)])

    if l == 0 and "dbg_q0" in d:
        hw.dma_start(out=d["dbg_q0"], in_=qk_t[0])
        hw.dma_start(out=d["dbg_k0"], in_=qk_t[8])
        hw.dma_start(out=d["dbg_v0"], in_=v_t[0])
        hw.dma_start(out=d["dbg_cx0"], in_=cx_t[0])
        hw.dma_start(out=d["dbg_r1"], in_=r1_t[0])
    h1_t = [g.hpool.tile([128, S], f16, tag="h", name=f"h1_{l}_{c}")
            for c in range(NE)]
    h1p_t = [g.hppool.tile([128, 2, S], f8, tag="hp", name=f"h1p{l}_{j}")
             for j in range(NEP)]
    dh1p_t = [g.dhppool.tile([128, 2, S], f8, tag="dhp", name=f"dh1p{l}_{j}")
              for j in range(NEP)]
    with nc.named_scope(f"L{l}_ln1a"):
        _ln_half(g, 0, r1_t, ln_s[0], ln_b[0], h1_t, h1p_t, dh1p_t,
                 tag=f"l1{l}")
    with nc.named_scope(f"L{l}_wob"):
        for gi in range(2):
            for mi in range(4):
                wo_chain(gi, mi, HALVES[1])

    # --- FFN -----------------------------------------------------------------
    with nc.named_scope(f"L{l}_ffn"):
        ffp_t = [g.ffpool.tile([128, 2, S], f8, tag="ff", name=f"ff{l}_{j}")
                 for j in range(NF // 2)]
        dffp_t = [g.ffpool.tile([128, 2, S], f8, tag="dff",
                                name=f"dff{l}_{j}") for j in range(NF // 2)]
        w1_sb = {}

        def load_w1(b):
            if b in w1_sb or b > 15:
                return
            wt = g.w1pool.tile([128, 4, 2, 2, 256], f8, tag="w1",
                               name=f"w1_{l}_{b}")
            hw.dma_start(out=wt, in_=d["w1"][l, b])
            w1_sb[b] = wt

        def ffn1_block(b, X):
            wt = w1_sb[b]
            for mi in range(2):
                mt = 2 * b + mi
                ps = g.ps_gemm.tile([128, S], f32, tag="gemm", name=f"f1ps{l}")
                _gemm3(g, ps, wt, mi, h1p_t, dh1p_t, X)
                ft = g.tmppool.tile([128, S], f16, tag="ffh", bufs=3,
                                    name=f"ffh{l}_{mt}")
                nc.scalar.activation(ft[:, X], ps[:, X], AF.Gelu,
                                     bias=b1_pp[:, mt:mt + 1], scale=DEQ)
                nc.gpsimd.tensor_copy(ffp_t[mt // 2][:, mt % 2, X], ft[:, X])
                nc.gpsimd.tensor_sub(dffp_t[mt // 2][:, mt % 2, X], ft[:, X],
                                     ffp_t[mt // 2][:, mt % 2, X])

        if l == 0 and "dbg_h1" in d:
            hw.dma_start(out=d["dbg_h1"], in_=h1_t[0])
        load_w1(0)
        load_w1(1)
        for b in range(16):
            load_w1(b + 1)
            ffn1_block(b, HALVES[0])
            if b == 2:
                with nc.named_scope(f"L{l}_ln1b"):
                    _ln_half(g, 1, r1_t, ln_s[0], ln_b[0], h1_t, h1p_t,
                             dh1p_t, tag=f"l1{l}")
            if b >= 3:
                ffn1_block(b - 3, HALVES[1])
        for b in range(13, 16):
            ffn1_block(b, HALVES[1])

        # FFN2: 8 blocks of 128 outputs x 16 cp; A/B pairwise per block
        r2_t = [g.rpool.tile([128, S], f16, tag="r", name=f"r2_{l}_{mt}")
                for mt in range(NE)]
        h2_t = [g.hpool.tile([128, S], f16, tag="h", name=f"h2_{l}_{c}")
                for c in range(NE)]
        h2p_t = [g.hppool.tile([128, 2, S], f8, tag="hp", name=f"h2p{l}_{j}")
                 for j in range(NEP)]
        dh2p_t = [g.dhppool.tile([128, 2, S], f8, tag="dhp",
                                 name=f"dh2p{l}_{j}") for j in range(NEP)]
        w2_sb = {}

        def load_w2(b):
            if b in w2_sb or b > 7:
                return
            wt = g.w2pool.tile([128, 16, 2, 2, 128], f8, tag="w2",
                               name=f"w2_{l}_{b}")
            hw.dma_start(out=wt, in_=d["w2"][l, b])
            w2_sb[b] = wt

        def ffn2_block(b, X):
            mt = b
            wt = w2_sb[b]
            ps = g.ps_gemm.tile([128, S], f32, tag="gemm", name=f"f2ps{l}")
            for cp in range(16):
                w8 = wt[:, cp, :, 0, :]
                dw8 = wt[:, cp, :, 1, :]
                mm(ps[:, X], w8, ffp_t[cp][:, :, X], start=(cp == 0),
                   stop=False, perf_mode=DR)
                mm(ps[:, X], dw8, ffp_t[cp][:, :, X], start=False,
                   stop=False, perf_mode=DR)
                mm(ps[:, X], w8, dffp_t[cp][:, :, X], start=False,
                   stop=(cp == 15), perf_mode=DR)
            f2o = g.tmppool.tile([128, S], f16, tag="ffh", bufs=3,
                                 name=f"f2o{l}_{mt}")
            nc.scalar.activation(f2o[:, X], ps[:, X], AF.Identity,
                                 bias=b2_pp[:, mt:mt + 1], scale=DEQ)
            nc.vector.tensor_add(r2_t[mt][:, X], f2o[:, X], h1_t[mt][:, X])

        load_w2(0)
        for b in range(8):
            load_w2(b + 1)
            ffn2_block(b, HALVES[0])
            if b == 7:
                with nc.named_scope(f"L{l}_ln2a"):
                    _ln_half(g, 0, r2_t, ln_s[1], ln_b[1], h2_t, h2p_t,
                             dh2p_t, tag=f"l2{l}")
            ffn2_block(b, HALVES[1])
        with nc.named_scope(f"L{l}_ln2b"):
            _ln_half(g, 1, r2_t, ln_s[1], ln_b[1], h2_t, h2p_t, dh2p_t,
                     tag=f"l2{l}")
        if l == 0 and "dbg_ff0" in d:
            hw.dma_start(out=d["dbg_ff0"], in_=ffp_t[0])
            hw.dma_start(out=d["dbg_r2"], in_=r2_t[0])

    return h2_t, h2p_t, dh2p_t


def _ln_half(g, hx, r_t, s_pp, b_pp, out_t, outp_t, outdp_t, tag=""):
    """LayerNorm of half hx: fp8-DR stats, vector chain, quantized outputs."""
    nc = g.nc
    mm = nc.tensor.matmul
    X = HALVES[hx]

    r8 = [g.r8pool.tile([128, 2, S], f8, tag="r8", name=f"r8{tag}_{hx}_{j}")
          for j in range(NEP)]
    sq8 = [g.r8pool.tile([128, 2, S], f8, tag="sq8", name=f"sq{tag}_{hx}_{j}")
           for j in range(NEP)]
    for j in range(NEP):
        for i in range(2):
            nc.gpsimd.tensor_copy(r8[j][:, i, X], r_t[2 * j + i][:, X])
        nc.gpsimd.tensor_mul(sq8[j][:, :, X], r8[j][:, :, X], r8[j][:, :, X])

    sums_r2 = g.ps_gemm.tile([128, S], f32, tag="gemm", name=f"lsr{tag}_{hx}")
    sums_q2 = g.ps_gemm.tile([128, S], f32, tag="gemm", name=f"lsq{tag}_{hx}")
    sums_r = sums_r2[0:1]
    sums_q = sums_q2[0:1]
    for j in range(NEP):
        mm(sums_r2[:, X], g.ones8, r8[j][:, :, X], start=(j == 0),
           stop=(j == NEP - 1), perf_mode=DR)
    for j in range(NEP):
        mm(sums_q2[:, X], g.ones8, sq8[j][:, :, X], start=(j == 0),
           stop=(j == NEP - 1), perf_mode=DR)

    mean_b = g.smallf.tile([1, SH], f16, tag="sb", bufs=4,
                           name=f"mn{tag}_{hx}")
    nc.vector.tensor_scalar(mean_b, sums_r[:, X], 1.0 / E, None, OP.mult)
    meanR = g.bcpool.tile([128, SH], f16, tag="rec", name=f"mR{tag}_{hx}")
    nc.gpsimd.partition_broadcast(meanR, mean_b, channels=128)

    s2 = g.smallf.tile([1, SH], f32, tag="sf", bufs=4, name=f"s2{tag}_{hx}")
    nc.scalar.activation(s2, sums_r[:, X], AF.Square)
    varE = g.smallf.tile([1, SH], f32, tag="sf", bufs=4,
                         name=f"vE{tag}_{hx}")
    nc.vector.scalar_tensor_tensor(varE, s2, -1.0 / E, sums_q[:, X],
                                   OP.mult, OP.add)
    std = g.smallf.tile([1, SH], f32, tag="sf", bufs=4, name=f"st{tag}_{hx}")
    nc.scalar.activation(std, varE, AF.Sqrt, bias=g.eps_t[:1, :],
                         scale=1.0 / E)
    rstd = g.smallf.tile([1, SH], f32, tag="sf", bufs=4,
                         name=f"rs{tag}_{hx}")
    nc.vector.reciprocal_approx_fast(out=rstd, in_=std)
    rstd_b = g.smallf.tile([1, SH], f16, tag="sb", bufs=4,
                           name=f"rb{tag}_{hx}")
    nc.vector.tensor_copy(rstd_b, rstd)
    rstdR = g.bcpool.tile([128, SH], f16, tag="rec", name=f"rR{tag}_{hx}")
    nc.gpsimd.partition_broadcast(rstdR, rstd_b, channels=128)

    for c in range(NE):
        t2 = g.tmppool.tile([128, SH], f16, tag="t2", bufs=2,
                            name=f"t2{tag}_{hx}_{c}")
        nc.vector.tensor_sub(t2, r_t[c][:, X], meanR)
        t1 = g.tmppool.tile([128, SH], f16, tag="t1", bufs=2,
                            name=f"t1{tag}_{hx}_{c}")
        nc.vector.tensor_mul(t1, t2, rstdR)
        nc.vector.tensor_scalar(out_t[c][:, X], t1, s_pp[:, c:c + 1],
                                b_pp[:, c:c + 1], OP.mult, OP.add)
        nc.gpsimd.tensor_scalar(outp_t[c // 2][:, c % 2, X], t1,
                                s_pp[:, c:c + 1], b_pp[:, c:c + 1],
                                OP.mult, OP.add)
        nc.gpsimd.tensor_sub(outdp_t[c // 2][:, c % 2, X], out_t[c][:, X],
                             outp_t[c // 2][:, c % 2, X])


def _final(g, h_t, hp_t, dhp_t):
    nc = g.nc
    d = g.d
    mm = nc.tensor.matmul
    hw = nc.sync

    mrstd = g.smallf.tile([1, S], f32, tag="mrs", bufs=1, name="fmr")
    rstdR = g.bcpool.tile([128, S], f16, tag="frR", bufs=1, name="frR")

    for hx, X in enumerate(HALVES):
        sq8 = [g.r8pool.tile([128, 2, S], f8, tag="sq8",
                             name=f"fsq{hx}_{j}") for j in range(NEP)]
        for j in range(NEP):
            nc.gpsimd.tensor_mul(sq8[j][:, :, X], hp_t[j][:, :, X],
                                 hp_t[j][:, :, X])
        sums_r2 = g.ps_gemm.tile([128, S], f32, tag="gemm", name=f"fsr{hx}")
        sums_q2 = g.ps_gemm.tile([128, S], f32, tag="gemm", name=f"fsq{hx}")
        sums_r = sums_r2[0:1]
        sums_q = sums_q2[0:1]
        for j in range(NEP):
            mm(sums_r2[:, X], g.ones8, hp_t[j][:, :, X], start=(j == 0),
               stop=(j == NEP - 1), perf_mode=DR)
        for j in range(NEP):
            mm(sums_q2[:, X], g.ones8, sq8[j][:, :, X], start=(j == 0),
               stop=(j == NEP - 1), perf_mode=DR)
        s2 = g.smallf.tile([1, SH], f32, tag="sf", bufs=4, name=f"fs2{hx}")
        nc.scalar.activation(s2, sums_r[:, X], AF.Square)
        varE = g.smallf.tile([1, SH], f32, tag="sf", bufs=4, name=f"fvE{hx}")
        nc.vector.scalar_tensor_tensor(varE, s2, -1.0 / E, sums_q[:, X],
                                       OP.mult, OP.add)
        std = g.smallf.tile([1, SH], f32, tag="sf", bufs=4, name=f"fst{hx}")
        nc.scalar.activation(std, varE, AF.Sqrt, bias=g.eps_t[:1, :],
                             scale=1.0 / E)
        rstd = g.smallf.tile([1, SH], f32, tag="sf", bufs=4, name=f"frs{hx}")
        nc.vector.reciprocal_approx_fast(out=rstd, in_=std)
        rstd_b = g.smallf.tile([1, SH], f16, tag="sb", bufs=4,
                               name=f"frb{hx}")
        nc.vector.tensor_copy(rstd_b, rstd)
        nc.gpsimd.partition_broadcast(rstdR[:, X], rstd_b, channels=128)
        nc.vector.scalar_tensor_tensor(mrstd[:, X], sums_r[:, X], 1.0 / E,
                                       rstd, OP.mult, OP.mult)
    hw.dma_start(out=d["mrstd"], in_=mrstd)

    for mt in range(NO):
        gi, mi = divmod(mt, 2)
        if mi == 0:
            wt = g.gwpool.tile([128, 4, 2, 2, 256], f8, tag="gw",
                               name=f"gw{gi}")
            hw.dma_start(out=wt, in_=d["genw8"][gi])
            _final.wt = wt
        ps = g.ps_gemm.tile([128, S], f32, tag="gemm", name=f"hd{mt}")
        _gemm3(g, ps, _final.wt, mi, hp_t, dhp_t, slice(0, S))
        gt = g.outpool.tile([128, S], f16, tag="f16out", name=f"gt{mt}")
        nc.vector.scalar_tensor_tensor(gt, ps, float(DEQ), rstdR,
                                       OP.mult, OP.mult)
        hw.dma_start(out=d["logits"][mt], in_=gt)


def _build():
    if "nc" in _CACHE:
        return _CACHE["nc"]
    from contextlib import ExitStack

    nc = bacc.Bacc("TRN2", debug=False)
    d = _declare(nc)
    with tile.TileContext(nc) as tc:
        with ExitStack() as ctx:
            _emit(nc, tc, d, ctx)
    nc.compile()
    _CACHE["nc"] = nc
    return nc


def kernel_internal(inputs, trace=False, trace_kwargs=None):
    shared = _prep_shared(inputs)
    cores = _prep_percore(inputs)
    nc = _build()
    in_maps = []
    for b in range(B):
        m = dict(shared)
        m.update(cores[b])
        in_maps.append(m)
    res = run_bass_kernel_spmd(
        nc, in_maps, core_ids=list(range(B)), trace=trace,
        **(trace_kwargs or {}),
    )
    nws = _CACHE["nws"]
    gb = _CACHE["gb"]
    outs = []
    for b in range(B):
        lo = np.asarray(res.results[b]["logits"], np.float32)  # [10,128,512]
        mr = np.asarray(res.results[b]["mrstd"], np.float32).reshape(S)
        lo = lo.reshape(NO * 128, S)[:VV * VR].T  # [512, 1200]
        lo = lo + mr[:, None] * nws[None, :] + gb[None, :]
        outs.append(lo)
    out = np.stack(outs).astype(np.float32)  # [B, S, 1200]
    return out, res


def kernel(**inputs):
    out, _ = kernel_internal(inputs)
    return out


# revision 30
# speedup vs baseline: 1.1006x; 1.1006x over previous
"""Trainium2 Bass kernel for nn_BaseGenerator (4-layer dense transformer).

Strategy: pure data-parallel over batch (B=8 -> 8 NeuronCores, no
collectives).  Each core runs the full transformer on one batch element.

v2 scheme (cost model: fp8-e4m3 DoubleRow = 0.5 cycle/col, K=256/instr):
  - ALL GEMMs (QKV, Wo, FFN1, FFN2, head) fp8 DR with 3-term error
    compensation (w8*x8 + dw8*x8 + w8*dx8); weights pre-scaled by 2^8.
  - q/k/at stored fp8; scores fp8 non-DR (1 cyc/col); AV fp8 DR with
    v-residual compensation; softmax via exp + ones-row denominator.
  - attention causally trimmed: mask idz-matmuls / scores / exp / AV only
    cover q >= k-block (masked region written -240 by the idz matmul).
  - LN statistics via fp8-DR ones-matmuls on quantized r8/sq8 pair tiles.
  - token-half software pipelining (A=[0,256) B=[256,512)): GEMM halves are
    emitted with a lag so each half's LN vector chain overlaps the other
    half's matmuls; PE stays busy across LN boundaries.
  - embedding gather and the final-head affine fold (mean*rstd*rowsum(Wp)
    + genW@lnf_b + gen_b) are host-side; device ships f16 rstd-scaled
    logits + an f32 mean*rstd row.
"""

import os
import sys

for _p in ("/opt/trn_rl_repo",):
    if _p not in sys.path:
        sys.path.insert(0, _p)

import ml_dtypes
import numpy as np

import concourse.bass as bass
import concourse.mybir as mybir
import concourse.tile as tile
from concourse import bacc
from concourse.bass_utils import run_bass_kernel_spmd

F16 = np.float16
F8 = ml_dtypes.float8_e4m3

L, E, H, F = 4, 1024, 16, 4096
B, S = 8, 512
SH = S // 2  # half width 256
VV, VR = 40, 30
DIST_V = 200
PAD_ID = 0
DH = E // H  # 64
NE = E // 128  # 8 feature chunks
NEP = NE // 2  # 4 fp8 pair tiles
NF = F // 128  # 32
NO = 10  # logit row tiles (1280 padded)
NEG = -240.0

WSC = 256.0
DEQ = 1.0 / WSC

f32 = mybir.dt.float32
f16 = mybir.dt.float16
f8 = mybir.dt.float8e4
AF = mybir.ActivationFunctionType
OP = mybir.AluOpType
DR = mybir.MatmulPerfMode.DoubleRow

_CACHE = {}

HALVES = (slice(0, SH), slice(SH, S))


# ----------------------------------------------------------------------------
# host-side input prep
# ----------------------------------------------------------------------------

def _f8(x):
    return np.ascontiguousarray(np.asarray(x, np.float32).astype(F8))


def _f16(x):
    return np.ascontiguousarray(np.asarray(x, np.float32).astype(F16))


def _block_dr2(W, gsize):
    """W [O, I] -> fp8 (w8, dw8) blocks [G, 128, I//256, 2(i), 2(t), gsize]
    with [g, p, cp, i, t, o] <- W[g*gsize + o, cp*256 + i*128 + p]*WSC,
    t=0: e4m3 quant, t=1: e4m3 residual."""
    O, I = W.shape
    Ws = np.asarray(W, np.float32) * WSC
    w8 = Ws.astype(F8).astype(np.float32)
    dw8 = (Ws - w8).astype(F8).astype(np.float32)
    out = np.empty((O // gsize, 128, I // 256, 2, 2, gsize), F8)
    for t, wv in enumerate((w8, dw8)):
        Wb = wv.reshape(O // gsize, gsize, I // 256, 2, 128)  # g o cp i p
        out[:, :, :, :, t, :] = Wb.transpose(0, 4, 2, 3, 1).astype(F8)
    return np.ascontiguousarray(out)


def _pp(v):  # [..., N*128] -> [..., 128, N]
    *lead, N = v.shape
    return np.ascontiguousarray(
        v.reshape(*lead, N // 128, 128).swapaxes(-1, -2).astype(np.float32)
    )


def _prep_shared(inp):
    out = {}

    Wqkv = np.asarray(inp["Wqkv"], np.float32).copy()  # [L, 3E, E]
    bqkv = np.asarray(inp["bqkv"], np.float32).copy()  # [L, 3E]
    att_sc = 1.0 / np.sqrt(DH)
    bqkv[:, :E] *= att_sc

    out["wqkv"] = np.stack([_block_dr2(Wqkv[l], 512) for l in range(L)])
    Wo = np.asarray(inp["Wo"], np.float32)
    out["wo8"] = np.stack([_block_dr2(Wo[l], 512) for l in range(L)])
    W1 = np.asarray(inp["W1"], np.float32)
    out["w1"] = np.stack([_block_dr2(W1[l], 256) for l in range(L)])
    W2 = np.asarray(inp["W2"], np.float32)
    out["w2"] = np.stack([_block_dr2(W2[l], 128) for l in range(L)])

    genW = np.asarray(inp["gen_W"], np.float32)  # [1200, E]
    lnf_s_v = np.asarray(inp["lnf_s"], np.float32)
    lnf_b_v = np.asarray(inp["lnf_b"], np.float32)
    Wp = genW * lnf_s_v[None, :]
    Wp_pad = np.zeros((1280, E), np.float32)
    Wp_pad[:1200] = Wp
    out["genw8"] = _block_dr2(Wp_pad, 256)  # [5, 128, 4, 2, 2, 256]
    _CACHE["nws"] = -Wp.sum(1)
    _CACHE["gb"] = genW @ lnf_b_v + np.asarray(inp["gen_b"], np.float32)

    out["bqkv_pp"] = _pp(bqkv[:, : 2 * E])  # [L, 128, 16]
    out["bv_row"] = _f16(bqkv[:, 2 * E:].reshape(L, 1, E))  # [L, 1, E]
    out["bo_pp"] = _pp(np.asarray(inp["bo"], np.float32))
    out["b1_pp"] = _pp(np.asarray(inp["b1"], np.float32))
    out["b2_pp"] = _pp(np.asarray(inp["b2"], np.float32))

    ln_s = np.stack([np.asarray(inp["ln1_s"], np.float32),
                     np.asarray(inp["ln2_s"], np.float32)], 1)
    ln_b = np.stack([np.asarray(inp["ln1_b"], np.float32),
                     np.asarray(inp["ln2_b"], np.float32)], 1)
    out["ln_s_pp"] = _pp(ln_s)  # [L, 2, 128, 8]
    out["ln_b_pp"] = _pp(ln_b)

    idz = np.zeros((128, 2, 2, 128), np.float32)
    for v in range(2):
        idz[:, v, v, :] = np.eye(128)
    out["idz"] = _f8(idz)
    out["ones8"] = _f8(np.ones((128, 2, 128), np.float32))
    return out


def _prep_percore(inp):
    val = np.asarray(inp["val_sequences"]).astype(np.int64)
    ring = np.asarray(inp["ring_sequences"]).astype(np.int64)
    dist = np.asarray(inp["distance_squares"]).astype(np.int64)
    de = np.asarray(inp["dist_emb"], np.float32)  # [200, H]

    # embedding (f16 tables, f32 math, f16 result) == device one-hot matmul
    ve = np.asarray(inp["val_emb"], np.float32).astype(F16).astype(np.float32)
    re = np.asarray(inp["ring_emb"], np.float32).astype(F16).astype(np.float32)
    h = (ve[val] + re[ring]) * np.sqrt(E)  # [B, S, E] f32
    # pair layout [NEP, 128, 2, S]: slot i = feature chunk 2j+i
    hB = h.reshape(B, S, NEP, 2, 128).transpose(0, 2, 4, 3, 1)
    hinit = np.ascontiguousarray(hB.astype(F16))

    m = de[dist].astype(F8).astype(np.float32)  # [B, q, k, H]
    m = m.transpose(0, 3, 2, 1)          # [B, H, k, q]
    kk = np.arange(S)
    causal = kk[:, None] <= kk[None, :]  # keep k <= q
    m = np.where(causal[None, None], m, NEG)
    padk = val == PAD_ID
    m = np.where(padk[:, None, :, None], NEG, m)
    # -> [B, H, 2(qh), 128(p), 4(kc), SH(qi)]: k = kc*128+p, q = qh*256+qi
    m = m.reshape(B, H, 4, 128, 2, SH).transpose(0, 1, 4, 3, 2, 5)
    m = np.ascontiguousarray(m.astype(F8))

    return [{"mask": m[b], "hinit": hinit[b]} for b in range(B)]


# ----------------------------------------------------------------------------
# device program
# ----------------------------------------------------------------------------

def _declare(nc):
    d = {}

    def di(name, shape, dt):
        d[name] = nc.dram_tensor(name, list(shape), dt, kind="ExternalInput").ap()

    di("wqkv", (L, 6, 128, 4, 2, 2, 512), f8)
    di("wo8", (L, 2, 128, 4, 2, 2, 512), f8)
    di("w1", (L, 16, 128, 4, 2, 2, 256), f8)
    di("w2", (L, 8, 128, 16, 2, 2, 128), f8)
    di("genw8", (5, 128, 4, 2, 2, 256), f8)
    di("bqkv_pp", (L, 128, 16), f32)
    di("bv_row", (L, 1, E), f16)
    di("bo_pp", (L, 128, 8), f32)
    di("b1_pp", (L, 128, 32), f32)
    di("b2_pp", (L, 128, 8), f32)
    di("ln_s_pp", (L, 2, 128, 8), f32)
    di("ln_b_pp", (L, 2, 128, 8), f32)
    di("idz", (128, 2, 2, 128), f8)
    di("ones8", (128, 2, 128), f8)
    di("mask", (H, 2, 128, 4, SH), f8)
    di("hinit", (NEP, 128, 2, S), f16)
    d["logits"] = nc.dram_tensor("logits", [NO, 128, S], f16,
                                 kind="ExternalOutput").ap()
    if os.environ.get("BG_DEBUG"):
        def do(name, shape, dt=f16):
            d[name] = nc.dram_tensor(name, list(shape), dt,
                                     kind="ExternalOutput").ap()
        do("dbg_h0", (128, S))
        do("dbg_hp0", (128, 2, S), f8)
        do("dbg_q0", (128, S), f8)
        do("dbg_k0", (128, S), f8)
        do("dbg_v0", (128, 2, H, DH + 1), f8)
        do("dbg_atA", (128, 1, 2, SH), f8)
        do("dbg_atB", (128, 2, 2, SH), f8)
        do("dbg_cx0", (128, 2, S), f8)
        do("dbg_r1", (128, S))
        do("dbg_h1", (128, S))
        do("dbg_ff0", (128, 2, S), f8)
        do("dbg_r2", (128, S))
    d["mrstd"] = nc.dram_tensor("mrstd", [1, S], f16,
                                kind="ExternalOutput").ap()
    return d


class Ctx:
    pass


def _emit(nc, tc, d, ctx):
    hw = nc.sync
    g = Ctx()
    g.nc = nc
    g.d = d

    def pool(name, bufs, space="SBUF"):
        return ctx.enter_context(
            tc.tile_pool(name=name, bufs=bufs, space=space))

    g.wpool = pool("wpool", 3)      # 8KB: wqkv + wo blocks
    g.w1pool = pool("w1pool", 5)    # 4KB: ffn1 blocks (lag-3 span)
    g.w2pool = pool("w2pool", 2)    # 8KB: ffn2 blocks
    g.maskpool = pool("maskpool", 2)
    g.hpool = pool("hpool", 8)    # h/h1/h2 residual pair tiles [128,2,S]
    g.rpool = pool("rpool", 4)     # r1/r2 LN-input pair tiles [128,2,S]
    g.hppool = pool("hppool", 8)
    g.dhppool = pool("dhppool", 8)
    g.r8pool = pool("r8pool", 4)
    g.qkpool = pool("qkpool", 16)   # f8 [128,S]
    g.vpool = pool("vpool", 2)
    g.atpool = pool("atpool", 6)
    g.ctxpool = pool("ctxpool", 2)
    g.cxppool = pool("cxppool", 4)
    g.ffpool = pool("ffpool", 16)
    g.tmppool = pool("tmppool", 4)
    g.smallf = pool("smallf", 8)    # [1, SH]
    g.srec = pool("srec", 2)        # [1, SH] f32 softmax denom
    g.recpool = pool("recpool", 2)  # [DH, SH] f32
    g.bcpool = pool("bcpool", 2)    # [128, S] f16 broadcasts
    g.outpool = pool("outpool", 2)
    g.pppool = pool("pppool", 4)
    g.bvpool = pool("bvpool", 1)

    g.ps_gemm = pool("ps_gemm", 4, "PSUM")
    g.ps_att = pool("ps_att", 2, "PSUM")

    cpool = pool("cpool", 1)
    g.idz = cpool.tile([128, 2, 2, 128], f8)
    hw.dma_start(out=g.idz, in_=d["idz"])
    g.ones8 = cpool.tile([128, 2, 128], f8)
    hw.dma_start(out=g.ones8, in_=d["ones8"])
    g.eps_t = cpool.tile([128, 1], f32)
    nc.vector.memset(g.eps_t, 1e-5)

    # --- embedding: DMA h_init, quantize pairs -------------------------------
    with nc.named_scope("embed"):
        h_t = []
        for j in range(NEP):
            ht = g.hpool.tile([128, 2, S], f16, tag="h", name=f"h0_{j}")
            hw.dma_start(out=ht, in_=d["hinit"][j])
            h_t.append(ht)
        hp_t = [g.hppool.tile([128, 2, S], f8, tag="hp", name=f"ehp{j}")
                for j in range(NEP)]
        dhp_t = [g.dhppool.tile([128, 2, S], f8, tag="dhp", name=f"edhp{j}")
                 for j in range(NEP)]
        for X in HALVES:
            for j in range(NEP):
                nc.gpsimd.tensor_copy(hp_t[j][:, :, X], h_t[j][:, :, X])
                nc.gpsimd.tensor_sub(dhp_t[j][:, :, X], h_t[j][:, :, X],
                                     hp_t[j][:, :, X])

    if "dbg_h0" in d:
        hw.dma_start(out=d["dbg_h0"], in_=h_t[0][:, 0, :])
        hw.dma_start(out=d["dbg_hp0"], in_=hp_t[0])
    for l in range(L):
        h_t, hp_t, dhp_t = _layer(g, l, h_t, hp_t, dhp_t)

    with nc.named_scope("final"):
        _final(g, h_t, hp_t, dhp_t)


def _gemm3(g, ps, wt, mi, xp, dxp, X, ncp=4, gs=128):
    """3-term fp8 DR chain into ps[:, X]; wt [128, ncp, 2, 2, G] blocks,
    mi slices gs output columns."""
    mm = g.nc.tensor.matmul
    sl = slice(mi * gs, (mi + 1) * gs)
    for cp in range(ncp):
        w8 = wt[:, cp, :, 0, sl]
        dw8 = wt[:, cp, :, 1, sl]
        mm(ps[:, X], w8, xp[cp][:, :, X], start=(cp == 0), stop=False,
           perf_mode=DR)
        mm(ps[:, X], dw8, xp[cp][:, :, X], start=False, stop=False,
           perf_mode=DR)
        mm(ps[:, X], w8, dxp[cp][:, :, X], start=False, stop=(cp == ncp - 1),
           perf_mode=DR)


def _layer(g, l, h_t, hp_t, dhp_t):
    nc = g.nc
    d = g.d
    mm = nc.tensor.matmul
    hw = nc.sync

    # per-layer small params
    bqkv_pp = g.pppool.tile([128, 16], f32, tag="pp16", bufs=2)
    hw.dma_start(out=bqkv_pp, in_=d["bqkv_pp"][l])
    bo_pp = g.pppool.tile([128, 8], f32, tag="pp8", bufs=8)
    hw.dma_start(out=bo_pp, in_=d["bo_pp"][l])
    b1_pp = g.pppool.tile([128, 32], f32, tag="pp32", bufs=2)
    hw.dma_start(out=b1_pp, in_=d["b1_pp"][l])
    b2_pp = g.pppool.tile([128, 8], f32, tag="pp8", bufs=8)
    hw.dma_start(out=b2_pp, in_=d["b2_pp"][l])
    ln_s = [g.pppool.tile([128, 8], f32, tag="pp8", bufs=8,
                          name=f"lns{l}_{i}") for i in range(2)]
    ln_b = [g.pppool.tile([128, 8], f32, tag="pp8", bufs=8,
                          name=f"lnb{l}_{i}") for i in range(2)]
    for i in range(2):
        hw.dma_start(out=ln_s[i], in_=d["ln_s_pp"][l, i])
        hw.dma_start(out=ln_b[i], in_=d["ln_b_pp"][l, i])
    bvb = g.bvpool.tile([128, E], f16, tag="bvb", name=f"bvb{l}")
    hw.dma_start(out=bvb, in_=d["bv_row"][l].to_broadcast((128, E)))

    wq_sb = {}

    def load_wqkv(gi):
        if gi not in wq_sb:
            wt = g.wpool.tile([128, 4, 2, 2, 512], f8, tag="w",
                              name=f"wqkv{l}_{gi}")
            hw.dma_start(out=wt, in_=d["wqkv"][l, gi])
            wq_sb[gi] = wt
        return wq_sb[gi]

    qk_t = [g.qkpool.tile([128, S], f8, tag="qk", name=f"qk{l}_{t}")
            for t in range(16)]  # q 0..7, k 8..15
    v_t = []
    dv_t = []
    for kcp in range(2):
        vt = g.vpool.tile([128, 2, H, DH + 1], f8, tag="v", name=f"v{l}_{kcp}")
        nc.vector.memset(vt[:, :, :, DH:DH + 1], 1.0)
        v_t.append(vt)
        dvt = g.vpool.tile([128, 2, H, DH + 1], f8, tag="dv",
                           name=f"dv{l}_{kcp}")
        nc.vector.memset(dvt[:, :, :, DH:DH + 1], 0.0)
        dv_t.append(dvt)

    def qkv_chain(gi, mi, X):
        wt = load_wqkv(gi)
        mt = gi * 4 + mi
        ps = g.ps_gemm.tile([128, S], f32, tag="gemm", name=f"qkps{l}")
        _gemm3(g, ps, wt, mi, hp_t, dhp_t, X)
        if gi < 2:  # Q
            nc.scalar.activation(qk_t[mt][:, X], ps[:, X], AF.Identity,
                                 bias=bqkv_pp[:, mt:mt + 1],
                                 scale=float(DEQ / np.sqrt(DH)))
        else:  # K on DVE
            nc.vector.tensor_scalar(qk_t[mt][:, X], ps[:, X], float(DEQ),
                                    bqkv_pp[:, mt:mt + 1], OP.mult, OP.add)

    def v_chain(gi, n):
        wt = load_wqkv(4 + gi)
        ps = g.ps_gemm.tile([128, S], f32, tag="gemm", name=f"vps{l}")
        for cp in range(4):
            xs = hp_t[cp][:, :, n * 128:(n + 1) * 128]
            dxs = dhp_t[cp][:, :, n * 128:(n + 1) * 128]
            w8 = wt[:, cp, :, 0, :]
            dw8 = wt[:, cp, :, 1, :]
            mm(ps, xs, w8, start=(cp == 0), stop=False, perf_mode=DR)
            mm(ps, xs, dw8, start=False, stop=False, perf_mode=DR)
            mm(ps, dxs, w8, start=False, stop=(cp == 3), perf_mode=DR)
        tmp = g.tmppool.tile([128, S], f16, tag="vtmp", bufs=2,
                             name=f"vtmp{l}_{gi}_{n}")
        nc.vector.scalar_tensor_tensor(
            tmp, ps, float(DEQ), bvb[:, gi * 512:(gi + 1) * 512],
            OP.mult, OP.add)
        v8s = v_t[n // 2][:, n % 2, gi * 8:(gi + 1) * 8, 0:DH]
        tmpr = tmp.rearrange("p (a b) -> p a b", a=8)
        nc.gpsimd.tensor_copy(v8s, tmpr)
        nc.gpsimd.tensor_sub(dv_t[n // 2][:, n % 2, gi * 8:(gi + 1) * 8, 0:DH],
                             tmpr, v8s)

    # --- attention -----------------------------------------------------------
    ctx16 = {}
    cx_t = [g.cxppool.tile([128, 2, S], f8, tag="cx", name=f"cx{l}_{j}")
            for j in range(NEP)]
    dcx_t = [g.cxppool.tile([128, 2, S], f8, tag="dcx", name=f"dcx{l}_{j}")
             for j in range(NEP)]

    def emit_attn_half(hx, filler):
        X = HALVES[hx]
        nkcp = 1 + hx
        at_q = {}

        def emit_scores(h):
            qt = qk_t[h // 2]
            kt = qk_t[8 + h // 2]
            r0 = (h % 2) * DH
            mt_ = g.maskpool.tile([128, 2 * nkcp, SH], f8,
                                  tag=("mA", "mB")[hx], bufs=2,
                                  name=f"mk{l}_{hx}_{h}")
            hw.dma_start(out=mt_, in_=d["mask"][h, hx, :, 0:2 * nkcp, :])
            spsf = g.ps_att.tile([128, 2, 2, SH], f32, tag="att",
                                 name=f"s{l}_{hx}_{h}")
            sps = spsf[:, 0:nkcp]
            for kcp in range(nkcp):
                mrh = mt_[:, 2 * kcp:2 * kcp + 2, :]
                for kci in range(2):
                    kc = kcp * 2 + kci
                    qlo = max(kc * 128, X.start)
                    mm(sps[:, kcp, kci, :], g.idz[:, kci], mrh,
                       start=True, stop=False, perf_mode=DR)
                    mm(sps[:, kcp, kci, qlo - X.start:],
                       kt[r0:r0 + DH, kc * 128:(kc + 1) * 128],
                       qt[r0:r0 + DH, qlo:X.stop], start=False, stop=True)
            at = g.atpool.tile([128, nkcp, 2, SH], f8,
                               tag=("atA", "atB")[hx], bufs=3 - hx,
                               name=f"a{l}_{hx}_{h}")
            nc.scalar.activation(at, sps, AF.Exp)
            if l == 0 and h == 0 and f"dbg_at{'AB'[hx]}" in d:
                hw.dma_start(out=d[f"dbg_at{'AB'[hx]}"], in_=at)
            at_q[h] = at

        def emit_av(h):
            at = at_q.pop(h)
            cps = g.ps_gemm.tile([128, S], f32, tag="gemm", name=f"c{l}_{hx}")
            for kcp in range(nkcp):
                mm(cps[0:DH + 1, X], v_t[kcp][:, :, h, :], at[:, kcp],
                   start=(kcp == 0), stop=False, perf_mode=DR)
                mm(cps[0:DH + 1, X], dv_t[kcp][:, :, h, :], at[:, kcp],
                   start=False, stop=(kcp == nkcp - 1), perf_mode=DR)
            srow = g.srec.tile([1, SH], f32, tag="sw", bufs=2,
                               name=f"sw{l}_{hx}_{h}")
            nc.scalar.activation(srow, cps[DH:DH + 1, X], AF.Copy)
            rec = g.srec.tile([1, SH], f32, tag="sr", bufs=1,
                              name=f"re{l}_{hx}_{h}")
            nc.vector.reciprocal_approx_fast(out=rec, in_=srow)
            recR = g.recpool.tile([DH, SH], f32, tag="recf",
                                  name=f"rr{l}_{hx}_{h}")
            nc.gpsimd.partition_broadcast(recR, rec, channels=DH)
            j, i = h // 4, (h // 2) % 2
            hh = h % 2
            if j not in ctx16:
                ctx16[j] = g.ctxpool.tile([128, 2, SH], f16, tag="ctx",
                                          name=f"cxt{l}_{hx}_{j}")
            nc.vector.tensor_mul(ctx16[j][hh * DH:(hh + 1) * DH, i, :],
                                 cps[0:DH, X], recR)
            if i == 1 and hh == 1:
                ct = ctx16.pop(j)
                nc.gpsimd.tensor_copy(cx_t[j][:, :, X], ct)
                nc.gpsimd.tensor_sub(dcx_t[j][:, :, X], ct,
                                     cx_t[j][:, :, X])

        fi = iter(filler)
        emit_scores(0)
        emit_scores(1)
        for h in range(2, H):
            emit_scores(h)
            emit_av(h - 2)
            for fn in (next(fi, None), next(fi, None)):
                if fn is not None:
                    fn()
        emit_av(H - 2)
        emit_av(H - 1)
        for fn in fi:
            if fn is not None:
                fn()

    # --- out-proj ------------------------------------------------------------
    wo_sb = {}

    def load_wo(gi):
        if gi not in wo_sb:
            wt = g.wpool.tile([128, 4, 2, 2, 512], f8, tag="w",
                              name=f"wo{l}_{gi}")
            hw.dma_start(out=wt, in_=d["wo8"][l, gi])
            wo_sb[gi] = wt
        return wo_sb[gi]

    r1_t = [g.rpool.tile([128, 2, S], f16, tag="r", name=f"r1_{l}_{j}")
            for j in range(NEP)]

    def wo_chain(gi, mi, X):
        wt = load_wo(gi)
        mt = gi * 4 + mi
        ps = g.ps_gemm.tile([128, S], f32, tag="gemm", name=f"wops{l}")
        _gemm3(g, ps, wt, mi, cx_t, dcx_t, X)
        wo_o = g.tmppool.tile([128, SH], f16, tag="f2o", bufs=2,
                              name=f"woo{l}_{mt}")
        nc.scalar.activation(wo_o, ps[:, X], AF.Identity,
                             bias=bo_pp[:, mt:mt + 1], scale=DEQ)
        nc.vector.tensor_add(r1_t[mt // 2][:, mt % 2, X], wo_o,
                             h_t[mt // 2][:, mt % 2, X])

    # --- emit: QKV (A leads, B lags 2 blocks); attention; Wo -----------------
    with nc.named_scope(f"L{l}_qkv"):
        # A-g0, A-g1, B-g0, A-g2, B-g1, A-g3, B-g2, B-g3, then V half A;
        # V half B (token blocks 2,3) is emitted as attn_A filler.
        seq = [("qk", 0, 0), ("qk", 1, 0), ("qk", 0, 1), ("qk", 2, 0),
               ("qk", 1, 1), ("qk", 3, 0), ("qk", 2, 1), ("qk", 3, 1),
               ("v", 0, 0), ("v", 0, 1), ("v", 1, 0), ("v", 1, 1)]
        for kind, gi, b in seq:
            if kind == "qk":
                for mi in range(4):
                    qkv_chain(gi, mi, HALVES[b])
            else:
                v_chain(gi, b)

    with nc.named_scope(f"L{l}_attn"):
        load_wo(0)
        load_wo(1)
        emit_attn_half(0, [lambda gi=gi, n=n: v_chain(gi, n)
                           for gi in range(2) for n in (2, 3)])
        emit_attn_half(1, [None] * 6 +
                       [lambda gi=gi, mi=mi: wo_chain(gi, mi, HALVES[0])
                        for gi in range(2) for mi in range(4)])

    if l == 0 and "dbg_q0" in d:
        hw.dma_start(out=d["dbg_q0"], in_=qk_t[0])
        hw.dma_start(out=d["dbg_k0"], in_=qk_t[8])
        hw.dma_start(out=d["dbg_v0"], in_=v_t[0])
        hw.dma_start(out=d["dbg_cx0"], in_=cx_t[0])
        hw.dma_start(out=d["dbg_r1"], in_=r1_t[0][:, 0, :])
    h1_t = [g.hpool.tile([128, 2, S], f16, tag="h", name=f"h1_{l}_{j}")
            for j in range(NEP)]
    h1p_t = [g.hppool.tile([128, 2, S], f8, tag="hp", name=f"h1p{l}_{j}")
             for j in range(NEP)]
    dh1p_t = [g.dhppool.tile([128, 2, S], f8, tag="dhp", name=f"dh1p{l}_{j}")
              for j in range(NEP)]
    with nc.named_scope(f"L{l}_ln1a"):
        _ln_half(g, 0, r1_t, ln_s[0], ln_b[0], h1_t, h1p_t, dh1p_t,
                 tag=f"l1{l}")
    with nc.named_scope(f"L{l}_wob"):
        for gi in range(2):
            for mi in range(4):
                wo_chain(gi, mi, HALVES[1])
    with nc.named_scope(f"L{l}_ln1b"):
        _ln_half(g, 1, r1_t, ln_s[0], ln_b[0], h1_t, h1p_t, dh1p_t,
                 tag=f"l1{l}")

    # --- FFN -----------------------------------------------------------------
    with nc.named_scope(f"L{l}_ffn"):
        ffp_t = [g.ffpool.tile([128, 2, S], f8, tag="ff", name=f"ff{l}_{j}")
                 for j in range(NF // 2)]
        dffp_t = [g.ffpool.tile([128, 2, S], f8, tag="dff",
                                name=f"dff{l}_{j}") for j in range(NF // 2)]
        w1_sb = {}

        def load_w1(b):
            if b in w1_sb or b > 15:
                return
            wt = g.w1pool.tile([128, 4, 2, 2, 256], f8, tag="w1",
                               name=f"w1_{l}_{b}")
            hw.dma_start(out=wt, in_=d["w1"][l, b])
            w1_sb[b] = wt

        def ffn1_block(b, X):
            wt = w1_sb[b]
            ft = g.tmppool.tile([128, 2, SH], f16, tag="ffh", bufs=2,
                                name=f"ffh{l}_{b}")
            for mi in range(2):
                mt = 2 * b + mi
                ps = g.ps_gemm.tile([128, S], f32, tag="gemm", name=f"f1ps{l}")
                _gemm3(g, ps, wt, mi, h1p_t, dh1p_t, X)
                nc.scalar.activation(ft[:, mi], ps[:, X], AF.Gelu,
                                     bias=b1_pp[:, mt:mt + 1], scale=DEQ)
            nc.vector.tensor_copy(ffp_t[b][:, :, X], ft)
            nc.gpsimd.tensor_sub(dffp_t[b][:, :, X], ft,
                                 ffp_t[b][:, :, X])

        if l == 0 and "dbg_h1" in d:
            hw.dma_start(out=d["dbg_h1"], in_=h1_t[0][:, 0, :])
        load_w1(0)
        load_w1(1)
        for b in range(16):
            load_w1(b + 1)
            ffn1_block(b, HALVES[0])
            if b >= 3:
                ffn1_block(b - 3, HALVES[1])
        for b in range(13, 16):
            ffn1_block(b, HALVES[1])

        # FFN2: 8 blocks of 128 outputs x 16 cp; A/B pairwise per block
        r2_t = [g.rpool.tile([128, 2, S], f16, tag="r", name=f"r2_{l}_{j}")
                for j in range(NEP)]
        h2_t = [g.hpool.tile([128, 2, S], f16, tag="h", name=f"h2_{l}_{j}")
                for j in range(NEP)]
        h2p_t = [g.hppool.tile([128, 2, S], f8, tag="hp", name=f"h2p{l}_{j}")
                 for j in range(NEP)]
        dh2p_t = [g.dhppool.tile([128, 2, S], f8, tag="dhp",
                                 name=f"dh2p{l}_{j}") for j in range(NEP)]
        w2_sb = {}

        def load_w2(b):
            if b in w2_sb or b > 7:
                return
            wt = g.w2pool.tile([128, 16, 2, 2, 128], f8, tag="w2",
                               name=f"w2_{l}_{b}")
            hw.dma_start(out=wt, in_=d["w2"][l, b])
            w2_sb[b] = wt

        def ffn2_block(b, X):
            mt = b
            wt = w2_sb[b]
            ps = g.ps_gemm.tile([128, S], f32, tag="gemm", name=f"f2ps{l}")
            for cp in range(16):
                w8 = wt[:, cp, :, 0, :]
                dw8 = wt[:, cp, :, 1, :]
                mm(ps[:, X], w8, ffp_t[cp][:, :, X], start=(cp == 0),
                   stop=False, perf_mode=DR)
                mm(ps[:, X], dw8, ffp_t[cp][:, :, X], start=False,
                   stop=False, perf_mode=DR)
                mm(ps[:, X], w8, dffp_t[cp][:, :, X], start=False,
                   stop=(cp == 15), perf_mode=DR)
            f2o = g.tmppool.tile([128, SH], f16, tag="f2o", bufs=2,
                                 name=f"f2o{l}_{mt}")
            nc.scalar.activation(f2o, ps[:, X], AF.Identity,
                                 bias=b2_pp[:, mt:mt + 1], scale=DEQ)
            nc.vector.tensor_add(r2_t[mt // 2][:, mt % 2, X], f2o,
                                 h1_t[mt // 2][:, mt % 2, X])

        load_w2(0)
        for b in range(8):
            load_w2(b + 1)
            ffn2_block(b, HALVES[0])
            if b == 7:
                with nc.named_scope(f"L{l}_ln2a"):
                    _ln_half(g, 0, r2_t, ln_s[1], ln_b[1], h2_t, h2p_t,
                             dh2p_t, tag=f"l2{l}")
            ffn2_block(b, HALVES[1])
        with nc.named_scope(f"L{l}_ln2b"):
            _ln_half(g, 1, r2_t, ln_s[1], ln_b[1], h2_t, h2p_t, dh2p_t,
                     tag=f"l2{l}")
        if l == 0 and "dbg_ff0" in d:
            hw.dma_start(out=d["dbg_ff0"], in_=ffp_t[0])
            hw.dma_start(out=d["dbg_r2"], in_=r2_t[0][:, 0, :])

    return h2_t, h2p_t, dh2p_t


def _ln_half(g, hx, r_t, s_pp, b_pp, out_t, outp_t, outdp_t, tag="",
             want_dp=True):
    """LayerNorm of half hx on pair tiles: fp8-DR stats, pair-op vector
    chain, quantized pair outputs (residual pairs optional)."""
    nc = g.nc
    mm = nc.tensor.matmul
    X = HALVES[hx]

    r8 = [g.r8pool.tile([128, 2, S], f8, tag="r8", name=f"r8{tag}_{hx}_{j}")
          for j in range(NEP)]
    sq8 = [g.r8pool.tile([128, 2, S], f8, tag="sq8", name=f"sq{tag}_{hx}_{j}")
           for j in range(NEP)]
    for j in range(NEP):
        nc.gpsimd.tensor_copy(r8[j][:, :, X], r_t[j][:, :, X])
        nc.gpsimd.tensor_mul(sq8[j][:, :, X], r8[j][:, :, X], r8[j][:, :, X])

    sums_r2 = g.ps_gemm.tile([128, S], f32, tag="gemm", name=f"lsr{tag}_{hx}")
    sums_q2 = g.ps_gemm.tile([128, S], f32, tag="gemm", name=f"lsq{tag}_{hx}")
    sums_r = sums_r2[0:1]
    sums_q = sums_q2[0:1]
    for j in range(NEP):
        mm(sums_r2[:, X], g.ones8, r8[j][:, :, X], start=(j == 0),
           stop=(j == NEP - 1), perf_mode=DR)
    for j in range(NEP):
        mm(sums_q2[:, X], g.ones8, sq8[j][:, :, X], start=(j == 0),
           stop=(j == NEP - 1), perf_mode=DR)

    mean_b = g.smallf.tile([1, SH], f16, tag="sb", bufs=3,
                           name=f"mn{tag}_{hx}")
    nc.vector.tensor_scalar(mean_b, sums_r[:, X], 1.0 / E, None, OP.mult)
    meanR = g.bcpool.tile([128, 2, SH], f16, tag="rec", name=f"mR{tag}_{hx}")
    nc.gpsimd.partition_broadcast(meanR[:, 0], mean_b, channels=128)
    nc.gpsimd.tensor_copy(meanR[:, 1], meanR[:, 0])

    s2 = g.smallf.tile([1, SH], f32, tag="sf", bufs=3, name=f"s2{tag}_{hx}")
    nc.scalar.activation(s2, sums_r[:, X], AF.Square)
    varE = g.smallf.tile([1, SH], f32, tag="sf", bufs=3,
                         name=f"vE{tag}_{hx}")
    nc.vector.scalar_tensor_tensor(varE, s2, -1.0 / E, sums_q[:, X],
                                   OP.mult, OP.add)
    std = g.smallf.tile([1, SH], f32, tag="sf", bufs=3, name=f"st{tag}_{hx}")
    nc.scalar.activation(std, varE, AF.Sqrt, bias=g.eps_t[:1, :],
                         scale=1.0 / E)
    rstd = g.smallf.tile([1, SH], f32, tag="sf", bufs=3,
                         name=f"rs{tag}_{hx}")
    nc.vector.reciprocal_approx_fast(out=rstd, in_=std)
    rstd_b = g.smallf.tile([1, SH], f16, tag="sb", bufs=3,
                           name=f"rb{tag}_{hx}")
    nc.vector.tensor_copy(rstd_b, rstd)
    rstdR = g.bcpool.tile([128, 2, SH], f16, tag="rec", name=f"rR{tag}_{hx}")
    nc.gpsimd.partition_broadcast(rstdR[:, 0], rstd_b, channels=128)
    nc.gpsimd.tensor_copy(rstdR[:, 1], rstdR[:, 0])

    for j in range(NEP):
        t2 = g.tmppool.tile([128, 2, SH], f16, tag="t2", bufs=1,
                            name=f"t2{tag}_{hx}_{j}")
        nc.vector.tensor_sub(t2, r_t[j][:, :, X], meanR)
        t1 = g.tmppool.tile([128, 2, SH], f16, tag="t1", bufs=2,
                            name=f"t1{tag}_{hx}_{j}")
        nc.vector.tensor_mul(t1, t2, rstdR)
        for i in range(2):
            c = 2 * j + i
            nc.vector.tensor_scalar(out_t[j][:, i, X], t1[:, i], 
                                    s_pp[:, c:c + 1], b_pp[:, c:c + 1],
                                    OP.mult, OP.add)
        nc.gpsimd.tensor_copy(outp_t[j][:, :, X], out_t[j][:, :, X])
        if want_dp:
            nc.gpsimd.tensor_sub(outdp_t[j][:, :, X], out_t[j][:, :, X],
                                 outp_t[j][:, :, X])


def _final(g, h_t, hp_t, dhp_t):
    nc = g.nc
    d = g.d
    mm = nc.tensor.matmul
    hw = nc.sync

    mrstd = g.smallf.tile([1, S], f16, tag="mrs", bufs=1, name="fmr")
    rstdR = g.bcpool.tile([128, S], f16, tag="rec", bufs=2, name="frR")

    for hx, X in enumerate(HALVES):
        sq8 = [g.r8pool.tile([128, 2, S], f8, tag="sq8",
                             name=f"fsq{hx}_{j}") for j in range(NEP)]
        for j in range(NEP):
            nc.gpsimd.tensor_mul(sq8[j][:, :, X], hp_t[j][:, :, X],
                                 hp_t[j][:, :, X])
        sums_r2 = g.ps_gemm.tile([128, S], f32, tag="gemm", name=f"fsr{hx}")
        sums_q2 = g.ps_gemm.tile([128, S], f32, tag="gemm", name=f"fsq{hx}")
        sums_r = sums_r2[0:1]
        sums_q = sums_q2[0:1]
        for j in range(NEP):
            mm(sums_r2[:, X], g.ones8, hp_t[j][:, :, X], start=(j == 0),
               stop=(j == NEP - 1), perf_mode=DR)
        for j in range(NEP):
            mm(sums_q2[:, X], g.ones8, sq8[j][:, :, X], start=(j == 0),
               stop=(j == NEP - 1), perf_mode=DR)
        s2 = g.smallf.tile([1, SH], f32, tag="sf", bufs=3, name=f"fs2{hx}")
        nc.scalar.activation(s2, sums_r[:, X], AF.Square)
        varE = g.smallf.tile([1, SH], f32, tag="sf", bufs=3, name=f"fvE{hx}")
        nc.vector.scalar_tensor_tensor(varE, s2, -1.0 / E, sums_q[:, X],
                                       OP.mult, OP.add)
        std = g.smallf.tile([1, SH], f32, tag="sf", bufs=3, name=f"fst{hx}")
        nc.scalar.activation(std, varE, AF.Sqrt, bias=g.eps_t[:1, :],
                             scale=1.0 / E)
        rstd = g.smallf.tile([1, SH], f32, tag="sf", bufs=3, name=f"frs{hx}")
        nc.vector.reciprocal_approx_fast(out=rstd, in_=std)
        rstd_b = g.smallf.tile([1, SH], f16, tag="sb", bufs=3,
                               name=f"frb{hx}")
        nc.vector.tensor_copy(rstd_b, rstd)
        nc.gpsimd.partition_broadcast(rstdR[:, X], rstd_b, channels=128)
        nc.vector.scalar_tensor_tensor(mrstd[:, X], sums_r[:, X], 1.0 / E,
                                       rstd, OP.mult, OP.mult)
    hw.dma_start(out=d["mrstd"], in_=mrstd)

    for mt in range(NO):
        gi, mi = divmod(mt, 2)
        if mi == 0:
            wt = g.w2pool.tile([128, 4, 2, 2, 256], f8, tag="gw", bufs=2,
                               name=f"gw{gi}")
            hw.dma_start(out=wt, in_=d["genw8"][gi])
            _final.wt = wt
        ps = g.ps_gemm.tile([128, S], f32, tag="gemm", name=f"hd{mt}")
        _gemm3(g, ps, _final.wt, mi, hp_t, dhp_t, slice(0, S))
        gt = g.outpool.tile([128, S], f16, tag="f16out", name=f"gt{mt}")
        nc.vector.scalar_tensor_tensor(gt, ps, float(DEQ), rstdR,
                                       OP.mult, OP.mult)
        hw.dma_start(out=d["logits"][mt], in_=gt)


def _build():
    if "nc" in _CACHE:
        return _CACHE["nc"]
    from contextlib import ExitStack

    nc = bacc.Bacc("TRN2", debug=False)
    d = _declare(nc)
    with tile.TileContext(nc) as tc:
        with ExitStack() as ctx:
            _emit(nc, tc, d, ctx)
    nc.compile()
    _CACHE["nc"] = nc
    return nc


def kernel_internal(inputs, trace=False, trace_kwargs=None):
    shared = _prep_shared(inputs)
    cores = _prep_percore(inputs)
    nc = _build()
    in_maps = []
    for b in range(B):
        m = dict(shared)
        m.update(cores[b])
        in_maps.append(m)
    res = run_bass_kernel_spmd(
        nc, in_maps, core_ids=list(range(B)), trace=trace,
        **(trace_kwargs or {}),
    )
    nws = _CACHE["nws"]
    gb = _CACHE["gb"]
    outs = []
    for b in range(B):
        lo = np.asarray(res.results[b]["logits"], np.float32)  # [10,128,512]
        mr = np.asarray(res.results[b]["mrstd"], np.float32).reshape(S)
        lo = lo.reshape(NO * 128, S)[:VV * VR].T  # [512, 1200]
        lo = lo + mr[:, None] * nws[None, :] + gb[None, :]
        outs.append(lo)
    out = np.stack(outs).astype(np.float32)  # [B, S, 1200]
    return out, res


def kernel(**inputs):
    out, _ = kernel_internal(inputs)
    return out


# revision 31
# speedup vs baseline: 1.1790x; 1.0713x over previous
"""Trainium2 Bass kernel for nn_BaseGenerator (4-layer dense transformer).

Strategy: pure data-parallel over batch (B=8 -> 8 NeuronCores, no
collectives).  Each core runs the full transformer on one batch element.

v2 scheme (cost model: fp8-e4m3 DoubleRow = 0.5 cycle/col, K=256/instr):
  - ALL GEMMs (QKV, Wo, FFN1, FFN2, head) fp8 DR with 3-term error
    compensation (w8*x8 + dw8*x8 + w8*dx8); weights pre-scaled by 2^8.
  - q/k/at stored fp8; scores fp8 non-DR (1 cyc/col); AV fp8 DR with
    v-residual compensation; softmax via exp + ones-row denominator.
  - attention causally trimmed: mask idz-matmuls / scores / exp / AV only
    cover q >= k-block (masked region written -240 by the idz matmul).
  - LN statistics via fp8-DR ones-matmuls on quantized r8/sq8 pair tiles.
  - token-half software pipelining (A=[0,256) B=[256,512)): GEMM halves are
    emitted with a lag so each half's LN vector chain overlaps the other
    half's matmuls; PE stays busy across LN boundaries.
  - embedding gather and the final-head affine fold (mean*rstd*rowsum(Wp)
    + genW@lnf_b + gen_b) are host-side; device ships f16 rstd-scaled
    logits + an f32 mean*rstd row.
"""

import os
import sys

for _p in ("/opt/trn_rl_repo",):
    if _p not in sys.path:
        sys.path.insert(0, _p)

import ml_dtypes
import numpy as np

import concourse.bass as bass
import concourse.mybir as mybir
import concourse.tile as tile
from concourse import bacc
from concourse.bass_utils import run_bass_kernel_spmd

F16 = np.float16
F8 = ml_dtypes.float8_e4m3

L, E, H, F = 4, 1024, 16, 4096
B, S = 8, 512
SH = S // 2  # half width 256
VV, VR = 40, 30
DIST_V = 200
PAD_ID = 0
DH = E // H  # 64
NE = E // 128  # 8 feature chunks
NEP = NE // 2  # 4 fp8 pair tiles
NF = F // 128  # 32
NO = 10  # logit row tiles (1280 padded)
NEG = -240.0

WSC = 256.0
DEQ = 1.0 / WSC

f32 = mybir.dt.float32
f16 = mybir.dt.float16
f8 = mybir.dt.float8e4
AF = mybir.ActivationFunctionType
OP = mybir.AluOpType
DR = mybir.MatmulPerfMode.DoubleRow

_CACHE = {}

HALVES = (slice(0, SH), slice(SH, S))


# ----------------------------------------------------------------------------
# host-side input prep
# ----------------------------------------------------------------------------

def _f8(x):
    return np.ascontiguousarray(np.asarray(x, np.float32).astype(F8))


def _f16(x):
    return np.ascontiguousarray(np.asarray(x, np.float32).astype(F16))


def _block_dr2(W, gsize):
    """W [O, I] -> fp8 (w8, dw8) blocks [G, 128, I//256, 2(i), 2(t), gsize]
    with [g, p, cp, i, t, o] <- W[g*gsize + o, cp*256 + i*128 + p]*WSC,
    t=0: e4m3 quant, t=1: e4m3 residual."""
    O, I = W.shape
    Ws = np.asarray(W, np.float32) * WSC
    w8 = Ws.astype(F8).astype(np.float32)
    dw8 = (Ws - w8).astype(F8).astype(np.float32)
    out = np.empty((O // gsize, 128, I // 256, 2, 2, gsize), F8)
    for t, wv in enumerate((w8, dw8)):
        Wb = wv.reshape(O // gsize, gsize, I // 256, 2, 128)  # g o cp i p
        out[:, :, :, :, t, :] = Wb.transpose(0, 4, 2, 3, 1).astype(F8)
    return np.ascontiguousarray(out)


def _pp(v):  # [..., N*128] -> [..., 128, N]
    *lead, N = v.shape
    return np.ascontiguousarray(
        v.reshape(*lead, N // 128, 128).swapaxes(-1, -2).astype(np.float32)
    )


def _prep_shared(inp):
    out = {}

    Wqkv = np.asarray(inp["Wqkv"], np.float32).copy()  # [L, 3E, E]
    bqkv = np.asarray(inp["bqkv"], np.float32).copy()  # [L, 3E]
    att_sc = 1.0 / np.sqrt(DH)
    bqkv[:, :E] *= att_sc

    out["wqkv"] = np.stack([_block_dr2(Wqkv[l], 512) for l in range(L)])
    Wo = np.asarray(inp["Wo"], np.float32)
    out["wo8"] = np.stack([_block_dr2(Wo[l], 512) for l in range(L)])
    W1 = np.asarray(inp["W1"], np.float32)
    out["w1"] = np.stack([_block_dr2(W1[l], 256) for l in range(L)])
    W2 = np.asarray(inp["W2"], np.float32)
    out["w2"] = np.stack([_block_dr2(W2[l], 128) for l in range(L)])

    genW = np.asarray(inp["gen_W"], np.float32)  # [1200, E]
    lnf_s_v = np.asarray(inp["lnf_s"], np.float32)
    lnf_b_v = np.asarray(inp["lnf_b"], np.float32)
    Wp = genW * lnf_s_v[None, :]
    Wp_pad = np.zeros((1280, E), np.float32)
    Wp_pad[:1200] = Wp
    out["genw8"] = _block_dr2(Wp_pad, 256)  # [5, 128, 4, 2, 2, 256]
    _CACHE["nws"] = -Wp.sum(1)
    _CACHE["gb"] = genW @ lnf_b_v + np.asarray(inp["gen_b"], np.float32)

    out["bqkv_pp"] = _pp(bqkv[:, : 2 * E])  # [L, 128, 16]
    out["bv_row"] = _f16(bqkv[:, 2 * E:].reshape(L, 1, E))  # [L, 1, E]
    out["bo_pp"] = _pp(np.asarray(inp["bo"], np.float32))
    out["b1_pp"] = _pp(np.asarray(inp["b1"], np.float32))
    out["b2_pp"] = _pp(np.asarray(inp["b2"], np.float32))

    ln_s = np.stack([np.asarray(inp["ln1_s"], np.float32),
                     np.asarray(inp["ln2_s"], np.float32)], 1)
    ln_b = np.stack([np.asarray(inp["ln1_b"], np.float32),
                     np.asarray(inp["ln2_b"], np.float32)], 1)
    out["ln_s_pp"] = _pp(ln_s)  # [L, 2, 128, 8]
    out["ln_b_pp"] = _pp(ln_b)

    idz = np.zeros((128, 2, 2, 128), np.float32)
    for v in range(2):
        idz[:, v, v, :] = np.eye(128)
    out["idz"] = _f8(idz)
    out["ones8"] = _f8(np.ones((128, 2, 128), np.float32))
    return out


def _prep_percore(inp):
    val = np.asarray(inp["val_sequences"]).astype(np.int64)
    ring = np.asarray(inp["ring_sequences"]).astype(np.int64)
    dist = np.asarray(inp["distance_squares"]).astype(np.int64)
    de = np.asarray(inp["dist_emb"], np.float32)  # [200, H]

    # embedding (f16 tables, f32 math, f16 result) == device one-hot matmul
    ve = np.asarray(inp["val_emb"], np.float32).astype(F16).astype(np.float32)
    re = np.asarray(inp["ring_emb"], np.float32).astype(F16).astype(np.float32)
    h = (ve[val] + re[ring]) * np.sqrt(E)  # [B, S, E] f32
    # pair layout [NEP, 128, 2, S]: slot i = feature chunk 2j+i
    hB = h.reshape(B, S, NEP, 2, 128).transpose(0, 2, 4, 3, 1)
    hinit = np.ascontiguousarray(hB.astype(F16))

    m = de[dist].astype(F8).astype(np.float32)  # [B, q, k, H]
    m = m.transpose(0, 3, 2, 1)          # [B, H, k, q]
    kk = np.arange(S)
    causal = kk[:, None] <= kk[None, :]  # keep k <= q
    m = np.where(causal[None, None], m, NEG)
    padk = val == PAD_ID
    m = np.where(padk[:, None, :, None], NEG, m)
    # -> [B, H, 2(qh), 128(p), 4(kc), SH(qi)]: k = kc*128+p, q = qh*256+qi
    m = m.reshape(B, H, 4, 128, 2, SH).transpose(0, 1, 4, 3, 2, 5)
    m = np.ascontiguousarray(m.astype(F8))

    return [{"mask": m[b], "hinit": hinit[b]} for b in range(B)]


# ----------------------------------------------------------------------------
# device program
# ----------------------------------------------------------------------------

def _declare(nc):
    d = {}

    def di(name, shape, dt):
        d[name] = nc.dram_tensor(name, list(shape), dt, kind="ExternalInput").ap()

    di("wqkv", (L, 6, 128, 4, 2, 2, 512), f8)
    di("wo8", (L, 2, 128, 4, 2, 2, 512), f8)
    di("w1", (L, 16, 128, 4, 2, 2, 256), f8)
    di("w2", (L, 8, 128, 16, 2, 2, 128), f8)
    di("genw8", (5, 128, 4, 2, 2, 256), f8)
    di("bqkv_pp", (L, 128, 16), f32)
    di("bv_row", (L, 1, E), f16)
    di("bo_pp", (L, 128, 8), f32)
    di("b1_pp", (L, 128, 32), f32)
    di("b2_pp", (L, 128, 8), f32)
    di("ln_s_pp", (L, 2, 128, 8), f32)
    di("ln_b_pp", (L, 2, 128, 8), f32)
    di("idz", (128, 2, 2, 128), f8)
    di("ones8", (128, 2, 128), f8)
    di("mask", (H, 2, 128, 4, SH), f8)
    di("hinit", (NEP, 128, 2, S), f16)
    d["logits"] = nc.dram_tensor("logits", [NO, 128, S], f16,
                                 kind="ExternalOutput").ap()
    if os.environ.get("BG_DEBUG"):
        def do(name, shape, dt=f16):
            d[name] = nc.dram_tensor(name, list(shape), dt,
                                     kind="ExternalOutput").ap()
        do("dbg_h0", (128, S))
        do("dbg_hp0", (128, 2, S), f8)
        do("dbg_q0", (128, S), f8)
        do("dbg_k0", (128, S), f8)
        do("dbg_v0", (128, 2, H, DH + 1), f8)
        do("dbg_atA", (128, 1, 2, SH), f8)
        do("dbg_atB", (128, 2, 2, SH), f8)
        do("dbg_cx0", (128, 2, S), f8)
        do("dbg_r1", (128, S))
        do("dbg_h1", (128, S))
        do("dbg_ff0", (128, 2, S), f8)
        do("dbg_r2", (128, S))
    d["mrstd"] = nc.dram_tensor("mrstd", [1, S], f16,
                                kind="ExternalOutput").ap()
    return d


class Ctx:
    pass


def _emit(nc, tc, d, ctx):
    hw = nc.sync
    g = Ctx()
    g.nc = nc
    g.d = d

    def pool(name, bufs, space="SBUF"):
        return ctx.enter_context(
            tc.tile_pool(name=name, bufs=bufs, space=space))

    g.wpool = pool("wpool", 3)      # 8KB: wqkv + wo blocks
    g.w1pool = pool("w1pool", 5)    # 4KB: ffn1 blocks (lag-3 span)
    g.w2pool = pool("w2pool", 2)    # 8KB: ffn2 blocks
    g.maskpool = pool("maskpool", 2)
    g.hpool = pool("hpool", 8)    # h/h1/h2 residual pair tiles [128,2,S]
    g.rpool = pool("rpool", 4)     # r1/r2 LN-input pair tiles [128,2,S]
    g.hppool = pool("hppool", 8)
    g.dhppool = pool("dhppool", 8)
    g.r8pool = pool("r8pool", 4)
    g.qkpool = pool("qkpool", 16)   # f8 [128,S]
    g.vpool = pool("vpool", 2)
    g.atpool = pool("atpool", 6)
    g.ctxpool = pool("ctxpool", 2)
    g.cxppool = pool("cxppool", 4)
    g.ffpool = pool("ffpool", 16)
    g.tmppool = pool("tmppool", 4)
    g.smallf = pool("smallf", 8)    # [1, SH]
    g.srec = pool("srec", 2)        # [1, SH] f32 softmax denom
    g.recpool = pool("recpool", 2)  # [DH, SH] f32
    g.bcpool = pool("bcpool", 2)    # [128, S] f16 broadcasts
    g.outpool = pool("outpool", 2)
    g.pppool = pool("pppool", 4)
    g.bvpool = pool("bvpool", 1)

    g.ps_gemm = pool("ps_gemm", 4, "PSUM")
    g.ps_att = pool("ps_att", 2, "PSUM")

    cpool = pool("cpool", 1)
    g.idz = cpool.tile([128, 2, 2, 128], f8)
    hw.dma_start(out=g.idz, in_=d["idz"])
    g.ones8 = cpool.tile([128, 2, 128], f8)
    hw.dma_start(out=g.ones8, in_=d["ones8"])
    g.eps_t = cpool.tile([128, 1], f32)
    nc.vector.memset(g.eps_t, 1e-5)

    # --- embedding: DMA h_init, quantize pairs -------------------------------
    with nc.named_scope("embed"):
        h_t = []
        for j in range(NEP):
            ht = g.hpool.tile([128, 2, S], f16, tag="h", name=f"h0_{j}")
            hw.dma_start(out=ht, in_=d["hinit"][j])
            h_t.append(ht)
        hp_t = [g.hppool.tile([128, 2, S], f8, tag="hp", name=f"ehp{j}")
                for j in range(NEP)]
        dhp_t = [g.dhppool.tile([128, 2, S], f8, tag="dhp", name=f"edhp{j}")
                 for j in range(NEP)]
        for X in HALVES:
            for j in range(NEP):
                nc.scalar.activation(hp_t[j][:, :, X], h_t[j][:, :, X],
                                     AF.Copy)
                nc.gpsimd.tensor_sub(dhp_t[j][:, :, X], h_t[j][:, :, X],
                                     hp_t[j][:, :, X])

    if "dbg_h0" in d:
        hw.dma_start(out=d["dbg_h0"], in_=h_t[0][:, 0, :])
        hw.dma_start(out=d["dbg_hp0"], in_=hp_t[0])
    for l in range(L):
        h_t, hp_t, dhp_t = _layer(g, l, h_t, hp_t, dhp_t)

    with nc.named_scope("final"):
        _final(g, h_t, hp_t, dhp_t)


def _gemm3(g, ps, wt, mi, xp, dxp, X, ncp=4, gs=128):
    """3-term fp8 DR chain into ps[:, X]; wt [128, ncp, 2, 2, G] blocks,
    mi slices gs output columns."""
    mm = g.nc.tensor.matmul
    sl = slice(mi * gs, (mi + 1) * gs)
    for cp in range(ncp):
        w8 = wt[:, cp, :, 0, sl]
        dw8 = wt[:, cp, :, 1, sl]
        mm(ps[:, X], w8, xp[cp][:, :, X], start=(cp == 0), stop=False,
           perf_mode=DR)
        mm(ps[:, X], dw8, xp[cp][:, :, X], start=False, stop=False,
           perf_mode=DR)
        mm(ps[:, X], w8, dxp[cp][:, :, X], start=False, stop=(cp == ncp - 1),
           perf_mode=DR)


def _layer(g, l, h_t, hp_t, dhp_t):
    nc = g.nc
    d = g.d
    mm = nc.tensor.matmul
    hw = nc.sync

    # per-layer small params
    bqkv_pp = g.pppool.tile([128, 16], f32, tag="pp16", bufs=2)
    hw.dma_start(out=bqkv_pp, in_=d["bqkv_pp"][l])
    bo_pp = g.pppool.tile([128, 8], f32, tag="pp8", bufs=8)
    hw.dma_start(out=bo_pp, in_=d["bo_pp"][l])
    b1_pp = g.pppool.tile([128, 32], f32, tag="pp32", bufs=2)
    hw.dma_start(out=b1_pp, in_=d["b1_pp"][l])
    b2_pp = g.pppool.tile([128, 8], f32, tag="pp8", bufs=8)
    hw.dma_start(out=b2_pp, in_=d["b2_pp"][l])
    ln_s = [g.pppool.tile([128, 8], f32, tag="pp8", bufs=8,
                          name=f"lns{l}_{i}") for i in range(2)]
    ln_b = [g.pppool.tile([128, 8], f32, tag="pp8", bufs=8,
                          name=f"lnb{l}_{i}") for i in range(2)]
    for i in range(2):
        hw.dma_start(out=ln_s[i], in_=d["ln_s_pp"][l, i])
        hw.dma_start(out=ln_b[i], in_=d["ln_b_pp"][l, i])
    bvb = g.bvpool.tile([128, E], f16, tag="bvb", name=f"bvb{l}")
    hw.dma_start(out=bvb, in_=d["bv_row"][l].to_broadcast((128, E)))

    wq_sb = {}

    def load_wqkv(gi):
        if gi not in wq_sb:
            wt = g.wpool.tile([128, 4, 2, 2, 512], f8, tag="w",
                              name=f"wqkv{l}_{gi}")
            hw.dma_start(out=wt, in_=d["wqkv"][l, gi])
            wq_sb[gi] = wt
        return wq_sb[gi]

    qk_t = [g.qkpool.tile([128, S], f8, tag="qk", name=f"qk{l}_{t}")
            for t in range(16)]  # q 0..7, k 8..15
    v_t = []
    dv_t = []
    for kcp in range(2):
        vt = g.vpool.tile([128, 2, H, DH + 1], f8, tag="v", name=f"v{l}_{kcp}")
        nc.vector.memset(vt[:, :, :, DH:DH + 1], 1.0)
        v_t.append(vt)
        dvt = g.vpool.tile([128, 2, H, DH + 1], f8, tag="dv",
                           name=f"dv{l}_{kcp}")
        nc.vector.memset(dvt[:, :, :, DH:DH + 1], 0.0)
        dv_t.append(dvt)

    def qkv_chain(gi, mi, X):
        wt = load_wqkv(gi)
        mt = gi * 4 + mi
        ps = g.ps_gemm.tile([128, S], f32, tag="gemm", name=f"qkps{l}")
        _gemm3(g, ps, wt, mi, hp_t, dhp_t, X)
        if gi < 2:  # Q
            nc.scalar.activation(qk_t[mt][:, X], ps[:, X], AF.Identity,
                                 bias=bqkv_pp[:, mt:mt + 1],
                                 scale=float(DEQ / np.sqrt(DH)))
        else:  # K on DVE
            nc.vector.tensor_scalar(qk_t[mt][:, X], ps[:, X], float(DEQ),
                                    bqkv_pp[:, mt:mt + 1], OP.mult, OP.add)

    def v_chain(gi, n):
        wt = load_wqkv(4 + gi)
        ps = g.ps_gemm.tile([128, S], f32, tag="gemm", name=f"vps{l}")
        for cp in range(4):
            xs = hp_t[cp][:, :, n * 128:(n + 1) * 128]
            dxs = dhp_t[cp][:, :, n * 128:(n + 1) * 128]
            w8 = wt[:, cp, :, 0, :]
            dw8 = wt[:, cp, :, 1, :]
            mm(ps, xs, w8, start=(cp == 0), stop=False, perf_mode=DR)
            mm(ps, xs, dw8, start=False, stop=False, perf_mode=DR)
            mm(ps, dxs, w8, start=False, stop=(cp == 3), perf_mode=DR)
        tmp = g.tmppool.tile([128, S], f16, tag="vtmp", bufs=2,
                             name=f"vtmp{l}_{gi}_{n}")
        nc.vector.scalar_tensor_tensor(
            tmp, ps, float(DEQ), bvb[:, gi * 512:(gi + 1) * 512],
            OP.mult, OP.add)
        v8s = v_t[n // 2][:, n % 2, gi * 8:(gi + 1) * 8, 0:DH]
        tmpr = tmp.rearrange("p (a b) -> p a b", a=8)
        nc.vector.tensor_copy(v8s, tmpr)
        nc.vector.tensor_sub(dv_t[n // 2][:, n % 2, gi * 8:(gi + 1) * 8, 0:DH],
                             tmpr, v8s)

    # --- attention -----------------------------------------------------------
    ctx16 = {}
    cx_t = [g.cxppool.tile([128, 2, S], f8, tag="cx", name=f"cx{l}_{j}")
            for j in range(NEP)]
    dcx_t = [g.cxppool.tile([128, 2, S], f8, tag="dcx", name=f"dcx{l}_{j}")
             for j in range(NEP)]

    def emit_attn_half(hx, filler):
        X = HALVES[hx]
        nkcp = 1 + hx
        at_q = {}

        def emit_scores(h):
            qt = qk_t[h // 2]
            kt = qk_t[8 + h // 2]
            r0 = (h % 2) * DH
            mt_ = g.maskpool.tile([128, 2 * nkcp, SH], f8,
                                  tag=("mA", "mB")[hx], bufs=2,
                                  name=f"mk{l}_{hx}_{h}")
            hw.dma_start(out=mt_, in_=d["mask"][h, hx, :, 0:2 * nkcp, :])
            spsf = g.ps_att.tile([128, 2, 2, SH], f32, tag="att",
                                 name=f"s{l}_{hx}_{h}")
            sps = spsf[:, 0:nkcp]
            for kcp in range(nkcp):
                mrh = mt_[:, 2 * kcp:2 * kcp + 2, :]
                for kci in range(2):
                    kc = kcp * 2 + kci
                    qlo = max(kc * 128, X.start)
                    mm(sps[:, kcp, kci, :], g.idz[:, kci], mrh,
                       start=True, stop=False, perf_mode=DR)
                    mm(sps[:, kcp, kci, qlo - X.start:],
                       kt[r0:r0 + DH, kc * 128:(kc + 1) * 128],
                       qt[r0:r0 + DH, qlo:X.stop], start=False, stop=True)
            at = g.atpool.tile([128, nkcp, 2, SH], f8,
                               tag=("atA", "atB")[hx], bufs=3 - hx,
                               name=f"a{l}_{hx}_{h}")
            nc.scalar.activation(at, sps, AF.Exp)
            if l == 0 and h == 0 and f"dbg_at{'AB'[hx]}" in d:
                hw.dma_start(out=d[f"dbg_at{'AB'[hx]}"], in_=at)
            at_q[h] = at

        def emit_av(h):
            at = at_q.pop(h)
            cps = g.ps_gemm.tile([128, S], f32, tag="gemm", name=f"c{l}_{hx}")
            for kcp in range(nkcp):
                mm(cps[0:DH + 1, X], v_t[kcp][:, :, h, :], at[:, kcp],
                   start=(kcp == 0), stop=False, perf_mode=DR)
                mm(cps[0:DH + 1, X], dv_t[kcp][:, :, h, :], at[:, kcp],
                   start=False, stop=(kcp == nkcp - 1), perf_mode=DR)
            srow = g.srec.tile([1, SH], f32, tag="sw", bufs=2,
                               name=f"sw{l}_{hx}_{h}")
            nc.scalar.activation(srow, cps[DH:DH + 1, X], AF.Copy)
            rec = g.srec.tile([1, SH], f32, tag="sr", bufs=1,
                              name=f"re{l}_{hx}_{h}")
            nc.vector.reciprocal_approx_fast(out=rec, in_=srow)
            recR = g.recpool.tile([DH, SH], f32, tag="recf",
                                  name=f"rr{l}_{hx}_{h}")
            nc.gpsimd.partition_broadcast(recR, rec, channels=DH)
            j, i = h // 4, (h // 2) % 2
            hh = h % 2
            if j not in ctx16:
                ctx16[j] = g.ctxpool.tile([128, 2, SH], f16, tag="ctx",
                                          name=f"cxt{l}_{hx}_{j}")
            nc.vector.tensor_mul(ctx16[j][hh * DH:(hh + 1) * DH, i, :],
                                 cps[0:DH, X], recR)
            if i == 1 and hh == 1:
                ct = ctx16.pop(j)
                nc.vector.tensor_copy(cx_t[j][:, :, X], ct)
                nc.vector.tensor_sub(dcx_t[j][:, :, X], ct,
                                     cx_t[j][:, :, X])

        fi = iter(filler)
        emit_scores(0)
        emit_scores(1)
        for h in range(2, H):
            emit_scores(h)
            emit_av(h - 2)
            for fn in (next(fi, None), next(fi, None)):
                if fn is not None:
                    fn()
        emit_av(H - 2)
        emit_av(H - 1)
        for fn in fi:
            if fn is not None:
                fn()

    # --- out-proj ------------------------------------------------------------
    wo_sb = {}

    def load_wo(gi):
        if gi not in wo_sb:
            wt = g.wpool.tile([128, 4, 2, 2, 512], f8, tag="w",
                              name=f"wo{l}_{gi}")
            hw.dma_start(out=wt, in_=d["wo8"][l, gi])
            wo_sb[gi] = wt
        return wo_sb[gi]

    r1_t = [g.rpool.tile([128, 2, S], f16, tag="r", name=f"r1_{l}_{j}")
            for j in range(NEP)]

    def wo_chain(gi, mi, X):
        wt = load_wo(gi)
        mt = gi * 4 + mi
        ps = g.ps_gemm.tile([128, S], f32, tag="gemm", name=f"wops{l}")
        _gemm3(g, ps, wt, mi, cx_t, dcx_t, X)
        wo_o = g.tmppool.tile([128, SH], f16, tag="f2o", bufs=2,
                              name=f"woo{l}_{mt}")
        nc.scalar.activation(wo_o, ps[:, X], AF.Identity,
                             bias=bo_pp[:, mt:mt + 1], scale=DEQ)
        nc.vector.tensor_add(r1_t[mt // 2][:, mt % 2, X], wo_o,
                             h_t[mt // 2][:, mt % 2, X])

    # --- emit: QKV (A leads, B lags 2 blocks); attention; Wo -----------------
    with nc.named_scope(f"L{l}_qkv"):
        # A-g0, A-g1, B-g0, A-g2, B-g1, A-g3, B-g2, B-g3, then V half A;
        # V half B (token blocks 2,3) is emitted as attn_A filler.
        seq = [("qk", 0, 0), ("qk", 1, 0), ("qk", 0, 1), ("qk", 2, 0),
               ("qk", 1, 1), ("qk", 3, 0), ("qk", 2, 1), ("qk", 3, 1),
               ("v", 0, 0), ("v", 0, 1), ("v", 1, 0), ("v", 1, 1)]
        for kind, gi, b in seq:
            if kind == "qk":
                for mi in range(4):
                    qkv_chain(gi, mi, HALVES[b])
            else:
                v_chain(gi, b)

    with nc.named_scope(f"L{l}_attn"):
        load_wo(0)
        load_wo(1)
        emit_attn_half(0, [lambda gi=gi, n=n: v_chain(gi, n)
                           for gi in range(2) for n in (2, 3)])
        emit_attn_half(1, [None] * 6 +
                       [lambda gi=gi, mi=mi: wo_chain(gi, mi, HALVES[0])
                        for gi in range(2) for mi in range(4)])

    if l == 0 and "dbg_q0" in d:
        hw.dma_start(out=d["dbg_q0"], in_=qk_t[0])
        hw.dma_start(out=d["dbg_k0"], in_=qk_t[8])
        hw.dma_start(out=d["dbg_v0"], in_=v_t[0])
        hw.dma_start(out=d["dbg_cx0"], in_=cx_t[0])
        hw.dma_start(out=d["dbg_r1"], in_=r1_t[0][:, 0, :])
    h1_t = [g.hpool.tile([128, 2, S], f16, tag="h", name=f"h1_{l}_{j}")
            for j in range(NEP)]
    h1p_t = [g.hppool.tile([128, 2, S], f8, tag="hp", name=f"h1p{l}_{j}")
             for j in range(NEP)]
    dh1p_t = [g.dhppool.tile([128, 2, S], f8, tag="dhp", name=f"dh1p{l}_{j}")
              for j in range(NEP)]
    with nc.named_scope(f"L{l}_ln1a"):
        _ln_half(g, 0, r1_t, ln_s[0], ln_b[0], h1_t, h1p_t, dh1p_t,
                 tag=f"l1{l}")
    with nc.named_scope(f"L{l}_wob"):
        for gi in range(2):
            for mi in range(4):
                wo_chain(gi, mi, HALVES[1])
    with nc.named_scope(f"L{l}_ln1b"):
        _ln_half(g, 1, r1_t, ln_s[0], ln_b[0], h1_t, h1p_t, dh1p_t,
                 tag=f"l1{l}")

    # --- FFN -----------------------------------------------------------------
    with nc.named_scope(f"L{l}_ffn"):
        ffp_t = [g.ffpool.tile([128, 2, S], f8, tag="ff", name=f"ff{l}_{j}")
                 for j in range(NF // 2)]
        dffp_t = [g.ffpool.tile([128, 2, S], f8, tag="dff",
                                name=f"dff{l}_{j}") for j in range(NF // 2)]
        w1_sb = {}

        def load_w1(b):
            if b in w1_sb or b > 15:
                return
            wt = g.w1pool.tile([128, 4, 2, 2, 256], f8, tag="w1",
                               name=f"w1_{l}_{b}")
            hw.dma_start(out=wt, in_=d["w1"][l, b])
            w1_sb[b] = wt

        def ffn1_block(b, X):
            wt = w1_sb[b]
            ft = g.tmppool.tile([128, 2, SH], f16, tag="ffh", bufs=2,
                                name=f"ffh{l}_{b}")
            for mi in range(2):
                mt = 2 * b + mi
                ps = g.ps_gemm.tile([128, S], f32, tag="gemm", name=f"f1ps{l}")
                _gemm3(g, ps, wt, mi, h1p_t, dh1p_t, X)
                nc.scalar.activation(ft[:, mi], ps[:, X], AF.Gelu,
                                     bias=b1_pp[:, mt:mt + 1], scale=DEQ)
            nc.vector.tensor_copy(ffp_t[b][:, :, X], ft)
            nc.gpsimd.tensor_sub(dffp_t[b][:, :, X], ft,
                                 ffp_t[b][:, :, X])

        if l == 0 and "dbg_h1" in d:
            hw.dma_start(out=d["dbg_h1"], in_=h1_t[0][:, 0, :])
        load_w1(0)
        load_w1(1)
        for b in range(16):
            load_w1(b + 1)
            ffn1_block(b, HALVES[0])
            if b >= 3:
                ffn1_block(b - 3, HALVES[1])
        for b in range(13, 16):
            ffn1_block(b, HALVES[1])

        # FFN2: 8 blocks of 128 outputs x 16 cp; A/B pairwise per block
        r2_t = [g.rpool.tile([128, 2, S], f16, tag="r", name=f"r2_{l}_{j}")
                for j in range(NEP)]
        h2_t = [g.hpool.tile([128, 2, S], f16, tag="h", name=f"h2_{l}_{j}")
                for j in range(NEP)]
        h2p_t = [g.hppool.tile([128, 2, S], f8, tag="hp", name=f"h2p{l}_{j}")
                 for j in range(NEP)]
        dh2p_t = [g.dhppool.tile([128, 2, S], f8, tag="dhp",
                                 name=f"dh2p{l}_{j}") for j in range(NEP)]
        w2_sb = {}

        def load_w2(b):
            if b in w2_sb or b > 7:
                return
            wt = g.w2pool.tile([128, 16, 2, 2, 128], f8, tag="w2",
                               name=f"w2_{l}_{b}")
            hw.dma_start(out=wt, in_=d["w2"][l, b])
            w2_sb[b] = wt

        def ffn2_block(b, X):
            mt = b
            wt = w2_sb[b]
            ps = g.ps_gemm.tile([128, S], f32, tag="gemm", name=f"f2ps{l}")
            for cp in range(16):
                w8 = wt[:, cp, :, 0, :]
                dw8 = wt[:, cp, :, 1, :]
                mm(ps[:, X], w8, ffp_t[cp][:, :, X], start=(cp == 0),
                   stop=False, perf_mode=DR)
                mm(ps[:, X], dw8, ffp_t[cp][:, :, X], start=False,
                   stop=False, perf_mode=DR)
                mm(ps[:, X], w8, dffp_t[cp][:, :, X], start=False,
                   stop=(cp == 15), perf_mode=DR)
            f2o = g.tmppool.tile([128, SH], f16, tag="f2o", bufs=2,
                                 name=f"f2o{l}_{mt}")
            nc.scalar.activation(f2o, ps[:, X], AF.Identity,
                                 bias=b2_pp[:, mt:mt + 1], scale=DEQ)
            nc.vector.tensor_add(r2_t[mt // 2][:, mt % 2, X], f2o,
                                 h1_t[mt // 2][:, mt % 2, X])

        load_w2(0)
        for b in range(8):
            load_w2(b + 1)
            ffn2_block(b, HALVES[0])
            if b == 7:
                with nc.named_scope(f"L{l}_ln2a"):
                    _ln_half(g, 0, r2_t, ln_s[1], ln_b[1], h2_t, h2p_t,
                             dh2p_t, tag=f"l2{l}")
            ffn2_block(b, HALVES[1])
        with nc.named_scope(f"L{l}_ln2b"):
            _ln_half(g, 1, r2_t, ln_s[1], ln_b[1], h2_t, h2p_t, dh2p_t,
                     tag=f"l2{l}")
        if l == 0 and "dbg_ff0" in d:
            hw.dma_start(out=d["dbg_ff0"], in_=ffp_t[0])
            hw.dma_start(out=d["dbg_r2"], in_=r2_t[0][:, 0, :])

    return h2_t, h2p_t, dh2p_t


def _ln_half(g, hx, r_t, s_pp, b_pp, out_t, outp_t, outdp_t, tag="",
             want_dp=True):
    """LayerNorm of half hx on pair tiles: fp8-DR stats, pair-op vector
    chain, quantized pair outputs (residual pairs optional)."""
    nc = g.nc
    mm = nc.tensor.matmul
    X = HALVES[hx]

    r8 = [g.r8pool.tile([128, 2, S], f8, tag="r8", name=f"r8{tag}_{hx}_{j}")
          for j in range(NEP)]
    sq8 = [g.r8pool.tile([128, 2, S], f8, tag="sq8", name=f"sq{tag}_{hx}_{j}")
           for j in range(NEP)]
    for j in range(NEP):
        nc.gpsimd.tensor_copy(r8[j][:, :, X], r_t[j][:, :, X])
        nc.gpsimd.tensor_mul(sq8[j][:, :, X], r8[j][:, :, X], r8[j][:, :, X])

    sums_r2 = g.ps_gemm.tile([128, S], f32, tag="gemm", name=f"lsr{tag}_{hx}")
    sums_q2 = g.ps_gemm.tile([128, S], f32, tag="gemm", name=f"lsq{tag}_{hx}")
    sums_r = sums_r2[0:1]
    sums_q = sums_q2[0:1]
    for j in range(NEP):
        mm(sums_r2[:, X], g.ones8, r8[j][:, :, X], start=(j == 0),
           stop=(j == NEP - 1), perf_mode=DR)
    for j in range(NEP):
        mm(sums_q2[:, X], g.ones8, sq8[j][:, :, X], start=(j == 0),
           stop=(j == NEP - 1), perf_mode=DR)

    mean_b = g.smallf.tile([1, SH], f16, tag="sb", bufs=3,
                           name=f"mn{tag}_{hx}")
    nc.vector.tensor_scalar(mean_b, sums_r[:, X], 1.0 / E, None, OP.mult)
    meanR = g.bcpool.tile([128, 2, SH], f16, tag="rec", name=f"mR{tag}_{hx}")
    nc.gpsimd.partition_broadcast(meanR[:, 0], mean_b, channels=128)
    nc.gpsimd.tensor_copy(meanR[:, 1], meanR[:, 0])

    s2 = g.smallf.tile([1, SH], f32, tag="sf", bufs=3, name=f"s2{tag}_{hx}")
    nc.scalar.activation(s2, sums_r[:, X], AF.Square)
    varE = g.smallf.tile([1, SH], f32, tag="sf", bufs=3,
                         name=f"vE{tag}_{hx}")
    nc.vector.scalar_tensor_tensor(varE, s2, -1.0 / E, sums_q[:, X],
                                   OP.mult, OP.add)
    std = g.smallf.tile([1, SH], f32, tag="sf", bufs=3, name=f"st{tag}_{hx}")
    nc.scalar.activation(std, varE, AF.Sqrt, bias=g.eps_t[:1, :],
                         scale=1.0 / E)
    rstd = g.smallf.tile([1, SH], f32, tag="sf", bufs=3,
                         name=f"rs{tag}_{hx}")
    nc.vector.reciprocal_approx_fast(out=rstd, in_=std)
    rstd_b = g.smallf.tile([1, SH], f16, tag="sb", bufs=3,
                           name=f"rb{tag}_{hx}")
    nc.vector.tensor_copy(rstd_b, rstd)
    rstdR = g.bcpool.tile([128, 2, SH], f16, tag="rec", name=f"rR{tag}_{hx}")
    nc.gpsimd.partition_broadcast(rstdR[:, 0], rstd_b, channels=128)
    nc.gpsimd.tensor_copy(rstdR[:, 1], rstdR[:, 0])

    for j in range(NEP):
        t2 = g.tmppool.tile([128, 2, SH], f16, tag="t2", bufs=1,
                            name=f"t2{tag}_{hx}_{j}")
        nc.vector.tensor_sub(t2, r_t[j][:, :, X], meanR)
        t1 = g.tmppool.tile([128, 2, SH], f16, tag="t1", bufs=2,
                            name=f"t1{tag}_{hx}_{j}")
        nc.vector.tensor_mul(t1, t2, rstdR)
        for i in range(2):
            c = 2 * j + i
            nc.vector.tensor_scalar(out_t[j][:, i, X], t1[:, i], 
                                    s_pp[:, c:c + 1], b_pp[:, c:c + 1],
                                    OP.mult, OP.add)
        nc.scalar.activation(outp_t[j][:, :, X], out_t[j][:, :, X], AF.Copy)
        if want_dp:
            nc.gpsimd.tensor_sub(outdp_t[j][:, :, X], out_t[j][:, :, X],
                                 outp_t[j][:, :, X])


def _final(g, h_t, hp_t, dhp_t):
    nc = g.nc
    d = g.d
    mm = nc.tensor.matmul
    hw = nc.sync

    mrstd = g.smallf.tile([1, S], f16, tag="mrs", bufs=1, name="fmr")
    rstdR = g.bcpool.tile([128, S], f16, tag="rec", bufs=2, name="frR")

    for hx, X in enumerate(HALVES):
        sq8 = [g.r8pool.tile([128, 2, S], f8, tag="sq8",
                             name=f"fsq{hx}_{j}") for j in range(NEP)]
        for j in range(NEP):
            nc.gpsimd.tensor_mul(sq8[j][:, :, X], hp_t[j][:, :, X],
                                 hp_t[j][:, :, X])
        sums_r2 = g.ps_gemm.tile([128, S], f32, tag="gemm", name=f"fsr{hx}")
        sums_q2 = g.ps_gemm.tile([128, S], f32, tag="gemm", name=f"fsq{hx}")
        sums_r = sums_r2[0:1]
        sums_q = sums_q2[0:1]
        for j in range(NEP):
            mm(sums_r2[:, X], g.ones8, hp_t[j][:, :, X], start=(j == 0),
               stop=(j == NEP - 1), perf_mode=DR)
        for j in range(NEP):
            mm(sums_q2[:, X], g.ones8, sq8[j][:, :, X], start=(j == 0),
               stop=(j == NEP - 1), perf_mode=DR)
        s2 = g.smallf.tile([1, SH], f32, tag="sf", bufs=3, name=f"fs2{hx}")
        nc.scalar.activation(s2, sums_r[:, X], AF.Square)
        varE = g.smallf.tile([1, SH], f32, tag="sf", bufs=3, name=f"fvE{hx}")
        nc.vector.scalar_tensor_tensor(varE, s2, -1.0 / E, sums_q[:, X],
                                       OP.mult, OP.add)
        std = g.smallf.tile([1, SH], f32, tag="sf", bufs=3, name=f"fst{hx}")
        nc.scalar.activation(std, varE, AF.Sqrt, bias=g.eps_t[:1, :],
                             scale=1.0 / E)
        rstd = g.smallf.tile([1, SH], f32, tag="sf", bufs=3, name=f"frs{hx}")
        nc.vector.reciprocal_approx_fast(out=rstd, in_=std)
        rstd_b = g.smallf.tile([1, SH], f16, tag="sb", bufs=3,
                               name=f"frb{hx}")
        nc.vector.tensor_copy(rstd_b, rstd)
        nc.gpsimd.partition_broadcast(rstdR[:, X], rstd_b, channels=128)
        nc.vector.scalar_tensor_tensor(mrstd[:, X], sums_r[:, X], 1.0 / E,
                                       rstd, OP.mult, OP.mult)
    hw.dma_start(out=d["mrstd"], in_=mrstd)

    for mt in range(NO):
        gi, mi = divmod(mt, 2)
        if mi == 0:
            wt = g.w2pool.tile([128, 4, 2, 2, 256], f8, tag="gw", bufs=2,
                               name=f"gw{gi}")
            hw.dma_start(out=wt, in_=d["genw8"][gi])
            _final.wt = wt
        ps = g.ps_gemm.tile([128, S], f32, tag="gemm", name=f"hd{mt}")
        _gemm3(g, ps, _final.wt, mi, hp_t, dhp_t, slice(0, S))
        gt = g.outpool.tile([128, S], f16, tag="f16out", name=f"gt{mt}")
        nc.vector.scalar_tensor_tensor(gt, ps, float(DEQ), rstdR,
                                       OP.mult, OP.mult)
        hw.dma_start(out=d["logits"][mt], in_=gt)


def _build():
    if "nc" in _CACHE:
        return _CACHE["nc"]
    from contextlib import ExitStack

    nc = bacc.Bacc("TRN2", debug=False)
    d = _declare(nc)
    with tile.TileContext(nc) as tc:
        with ExitStack() as ctx:
            _emit(nc, tc, d, ctx)
    nc.compile()
    _CACHE["nc"] = nc
    return nc


def kernel_internal(inputs, trace=False, trace_kwargs=None):
    shared = _prep_shared(inputs)
    cores = _prep_percore(inputs)
    nc = _build()
    in_maps = []
    for b in range(B):
        m = dict(shared)
        m.update(cores[b])
        in_maps.append(m)
    res = run_bass_kernel_spmd(
        nc, in_maps, core_ids=list(range(B)), trace=trace,
        **(trace_kwargs or {}),
    )
    nws = _CACHE["nws"]
    gb = _CACHE["gb"]
    outs = []
    for b in range(B):
        lo = np.asarray(res.results[b]["logits"], np.float32)  # [10,128,512]
        mr = np.asarray(res.results[b]["mrstd"], np.float32).reshape(S)
        lo = lo.reshape(NO * 128, S)[:VV * VR].T  # [512, 1200]
        lo = lo + mr[:, None] * nws[None, :] + gb[None, :]
        outs.append(lo)
    out = np.stack(outs).astype(np.float32)  # [B, S, 1200]
    return out, res


def kernel(**inputs):
    out, _ = kernel_internal(inputs)
    return out


# revision 32
# speedup vs baseline: 1.2457x; 1.0566x over previous
"""Trainium2 Bass kernel for nn_BaseGenerator (4-layer dense transformer).

Strategy: pure data-parallel over batch (B=8 -> 8 NeuronCores, no
collectives).  Each core runs the full transformer on one batch element.

v2 scheme (cost model: fp8-e4m3 DoubleRow = 0.5 cycle/col, K=256/instr):
  - ALL GEMMs (QKV, Wo, FFN1, FFN2, head) fp8 DR with 3-term error
    compensation (w8*x8 + dw8*x8 + w8*dx8); weights pre-scaled by 2^8.
  - q/k/at stored fp8; scores fp8 non-DR (1 cyc/col); AV fp8 DR with
    v-residual compensation; softmax via exp + ones-row denominator.
  - attention causally trimmed: mask idz-matmuls / scores / exp / AV only
    cover q >= k-block (masked region written -240 by the idz matmul).
  - LN statistics via fp8-DR ones-matmuls on quantized r8/sq8 pair tiles.
  - token-half software pipelining (A=[0,256) B=[256,512)): GEMM halves are
    emitted with a lag so each half's LN vector chain overlaps the other
    half's matmuls; PE stays busy across LN boundaries.
  - embedding gather and the final-head affine fold (mean*rstd*rowsum(Wp)
    + genW@lnf_b + gen_b) are host-side; device ships f16 rstd-scaled
    logits + an f32 mean*rstd row.
"""

import os
import sys

for _p in ("/opt/trn_rl_repo",):
    if _p not in sys.path:
        sys.path.insert(0, _p)

import ml_dtypes
import numpy as np

import concourse.bass as bass
import concourse.mybir as mybir
import concourse.tile as tile
from concourse import bacc
from concourse.bass_utils import run_bass_kernel_spmd

F16 = np.float16
F8 = ml_dtypes.float8_e4m3

L, E, H, F = 4, 1024, 16, 4096
B, S = 8, 512
SH = S // 2  # half width 256
VV, VR = 40, 30
DIST_V = 200
PAD_ID = 0
DH = E // H  # 64
NE = E // 128  # 8 feature chunks
NEP = NE // 2  # 4 fp8 pair tiles
NF = F // 128  # 32
NO = 10  # logit row tiles (1280 padded)
NEG = -240.0

WSC = 256.0
DEQ = 1.0 / WSC

f32 = mybir.dt.float32
f16 = mybir.dt.float16
f8 = mybir.dt.float8e4
AF = mybir.ActivationFunctionType
OP = mybir.AluOpType
DR = mybir.MatmulPerfMode.DoubleRow

_CACHE = {}

HALVES = (slice(0, SH), slice(SH, S))


# ----------------------------------------------------------------------------
# host-side input prep
# ----------------------------------------------------------------------------

def _f8(x):
    return np.ascontiguousarray(np.asarray(x, np.float32).astype(F8))


def _f16(x):
    return np.ascontiguousarray(np.asarray(x, np.float32).astype(F16))


def _block_dr2(W, gsize):
    """W [O, I] -> fp8 (w8, dw8) blocks [G, 128, I//256, 2(i), 2(t), gsize]
    with [g, p, cp, i, t, o] <- W[g*gsize + o, cp*256 + i*128 + p]*WSC,
    t=0: e4m3 quant, t=1: e4m3 residual."""
    O, I = W.shape
    Ws = np.asarray(W, np.float32) * WSC
    w8 = Ws.astype(F8).astype(np.float32)
    dw8 = (Ws - w8).astype(F8).astype(np.float32)
    out = np.empty((O // gsize, 128, I // 256, 2, 2, gsize), F8)
    for t, wv in enumerate((w8, dw8)):
        Wb = wv.reshape(O // gsize, gsize, I // 256, 2, 128)  # g o cp i p
        out[:, :, :, :, t, :] = Wb.transpose(0, 4, 2, 3, 1).astype(F8)
    return np.ascontiguousarray(out)


def _pp(v):  # [..., N*128] -> [..., 128, N]
    *lead, N = v.shape
    return np.ascontiguousarray(
        v.reshape(*lead, N // 128, 128).swapaxes(-1, -2).astype(np.float32)
    )


def _prep_shared(inp):
    out = {}

    Wqkv = np.asarray(inp["Wqkv"], np.float32).copy()  # [L, 3E, E]
    bqkv = np.asarray(inp["bqkv"], np.float32).copy()  # [L, 3E]
    att_sc = 1.0 / np.sqrt(DH)
    bqkv[:, :E] *= att_sc

    out["wqkv"] = np.stack([_block_dr2(Wqkv[l], 512) for l in range(L)])
    Wo = np.asarray(inp["Wo"], np.float32)
    out["wo8"] = np.stack([_block_dr2(Wo[l], 512) for l in range(L)])
    W1 = np.asarray(inp["W1"], np.float32)
    out["w1"] = np.stack([_block_dr2(W1[l], 256) for l in range(L)])
    W2 = np.asarray(inp["W2"], np.float32)
    out["w2"] = np.stack([_block_dr2(W2[l], 128) for l in range(L)])

    genW = np.asarray(inp["gen_W"], np.float32)  # [1200, E]
    lnf_s_v = np.asarray(inp["lnf_s"], np.float32)
    lnf_b_v = np.asarray(inp["lnf_b"], np.float32)
    Wp = genW * lnf_s_v[None, :]
    Wp_pad = np.zeros((1280, E), np.float32)
    Wp_pad[:1200] = Wp
    out["genw8"] = _block_dr2(Wp_pad, 256)  # [5, 128, 4, 2, 2, 256]
    _CACHE["nws"] = -Wp.sum(1)
    _CACHE["gb"] = genW @ lnf_b_v + np.asarray(inp["gen_b"], np.float32)

    out["bqkv_pp"] = _pp(bqkv[:, : 2 * E])  # [L, 128, 16]
    out["bv_row"] = _f16(bqkv[:, 2 * E:].reshape(L, 1, E))  # [L, 1, E]
    out["bo_pp"] = _pp(np.asarray(inp["bo"], np.float32))
    out["b1_pp"] = _pp(np.asarray(inp["b1"], np.float32))
    out["b2_pp"] = _pp(np.asarray(inp["b2"], np.float32))

    ln_s = np.stack([np.asarray(inp["ln1_s"], np.float32),
                     np.asarray(inp["ln2_s"], np.float32)], 1)
    ln_b = np.stack([np.asarray(inp["ln1_b"], np.float32),
                     np.asarray(inp["ln2_b"], np.float32)], 1)
    out["ln_s_pp"] = _pp(ln_s)  # [L, 2, 128, 8]
    out["ln_b_pp"] = _pp(ln_b)

    idz = np.zeros((128, 2, 2, 128), np.float32)
    for v in range(2):
        idz[:, v, v, :] = np.eye(128)
    out["idz"] = _f8(idz)
    out["ones8"] = _f8(np.ones((128, 2, 128), np.float32))
    return out


def _prep_percore(inp):
    val = np.asarray(inp["val_sequences"]).astype(np.int64)
    ring = np.asarray(inp["ring_sequences"]).astype(np.int64)
    dist = np.asarray(inp["distance_squares"]).astype(np.int64)
    de = np.asarray(inp["dist_emb"], np.float32)  # [200, H]

    # embedding (f16 tables, f32 math, f16 result) == device one-hot matmul
    ve = np.asarray(inp["val_emb"], np.float32).astype(F16).astype(np.float32)
    re = np.asarray(inp["ring_emb"], np.float32).astype(F16).astype(np.float32)
    h = (ve[val] + re[ring]) * np.sqrt(E)  # [B, S, E] f32
    # pair layout [NEP, 128, 2, S]: slot i = feature chunk 2j+i
    hB = h.reshape(B, S, NEP, 2, 128).transpose(0, 2, 4, 3, 1)
    hinit = np.ascontiguousarray(hB.astype(F16))

    m = de[dist].astype(F8).astype(np.float32)  # [B, q, k, H]
    m = m.transpose(0, 3, 2, 1)          # [B, H, k, q]
    kk = np.arange(S)
    causal = kk[:, None] <= kk[None, :]  # keep k <= q
    m = np.where(causal[None, None], m, NEG)
    padk = val == PAD_ID
    m = np.where(padk[:, None, :, None], NEG, m)
    # -> [B, H, 2(qh), 128(p), 4(kc), SH(qi)]: k = kc*128+p, q = qh*256+qi
    m = m.reshape(B, H, 4, 128, 2, SH).transpose(0, 1, 4, 3, 2, 5)
    m = np.ascontiguousarray(m.astype(F8))

    return [{"mask": m[b], "hinit": hinit[b]} for b in range(B)]


# ----------------------------------------------------------------------------
# device program
# ----------------------------------------------------------------------------

def _declare(nc):
    d = {}

    def di(name, shape, dt):
        d[name] = nc.dram_tensor(name, list(shape), dt, kind="ExternalInput").ap()

    di("wqkv", (L, 6, 128, 4, 2, 2, 512), f8)
    di("wo8", (L, 2, 128, 4, 2, 2, 512), f8)
    di("w1", (L, 16, 128, 4, 2, 2, 256), f8)
    di("w2", (L, 8, 128, 16, 2, 2, 128), f8)
    di("genw8", (5, 128, 4, 2, 2, 256), f8)
    di("bqkv_pp", (L, 128, 16), f32)
    di("bv_row", (L, 1, E), f16)
    di("bo_pp", (L, 128, 8), f32)
    di("b1_pp", (L, 128, 32), f32)
    di("b2_pp", (L, 128, 8), f32)
    di("ln_s_pp", (L, 2, 128, 8), f32)
    di("ln_b_pp", (L, 2, 128, 8), f32)
    di("idz", (128, 2, 2, 128), f8)
    di("ones8", (128, 2, 128), f8)
    di("mask", (H, 2, 128, 4, SH), f8)
    di("hinit", (NEP, 128, 2, S), f16)
    d["logits"] = nc.dram_tensor("logits", [NO, 128, S], f16,
                                 kind="ExternalOutput").ap()
    if os.environ.get("BG_DEBUG"):
        def do(name, shape, dt=f16):
            d[name] = nc.dram_tensor(name, list(shape), dt,
                                     kind="ExternalOutput").ap()
        do("dbg_h0", (128, S))
        do("dbg_hp0", (128, 2, S), f8)
        do("dbg_q0", (128, S), f8)
        do("dbg_k0", (128, S), f8)
        do("dbg_v0", (128, 2, H, DH + 1), f8)
        do("dbg_atA", (128, 1, 2, SH), f8)
        do("dbg_atB", (128, 2, 2, SH), f8)
        do("dbg_cx0", (128, 2, S), f8)
        do("dbg_r1", (128, S))
        do("dbg_h1", (128, S))
        do("dbg_ff0", (128, 2, S), f8)
        do("dbg_r2", (128, S))
    d["mrstd"] = nc.dram_tensor("mrstd", [1, S], f16,
                                kind="ExternalOutput").ap()
    return d


class Ctx:
    pass


def _emit(nc, tc, d, ctx):
    hw = nc.sync
    g = Ctx()
    g.nc = nc
    g.d = d

    def pool(name, bufs, space="SBUF"):
        return ctx.enter_context(
            tc.tile_pool(name=name, bufs=bufs, space=space))

    g.wpool = pool("wpool", 3)      # 8KB: wqkv + wo blocks
    g.w1pool = pool("w1pool", 5)    # 4KB: ffn1 blocks (lag-3 span)
    g.w2pool = pool("w2pool", 2)    # 8KB: ffn2 blocks
    g.maskpool = pool("maskpool", 2)
    g.hpool = pool("hpool", 8)    # h/h1/h2 residual pair tiles [128,2,S]
    g.rpool = pool("rpool", 4)     # r1/r2 LN-input pair tiles [128,2,S]
    g.hppool = pool("hppool", 8)
    g.dhppool = pool("dhppool", 8)
    g.r8pool = pool("r8pool", 4)
    g.qkpool = pool("qkpool", 16)   # f8 [128,S]
    g.vpool = pool("vpool", 2)
    g.atpool = pool("atpool", 6)
    g.ctxpool = pool("ctxpool", 2)
    g.cxppool = pool("cxppool", 4)
    g.ffpool = pool("ffpool", 16)
    g.tmppool = pool("tmppool", 4)
    g.smallf = pool("smallf", 8)    # [1, SH]
    g.srec = pool("srec", 2)        # [1, SH] f32 softmax denom
    g.recpool = pool("recpool", 2)  # [DH, SH] f32
    g.bcpool = pool("bcpool", 2)    # [128, S] f16 broadcasts
    g.outpool = pool("outpool", 2)
    g.pppool = pool("pppool", 4)
    g.bvpool = pool("bvpool", 1)

    g.ps_gemm = pool("ps_gemm", 4, "PSUM")
    g.ps_att = pool("ps_att", 2, "PSUM")

    cpool = pool("cpool", 1)
    g.idz = cpool.tile([128, 2, 2, 128], f8)
    hw.dma_start(out=g.idz, in_=d["idz"])
    g.ones8 = cpool.tile([128, 2, 128], f8)
    hw.dma_start(out=g.ones8, in_=d["ones8"])
    g.eps_t = cpool.tile([128, 1], f32)
    nc.vector.memset(g.eps_t, 1e-5)

    # --- embedding: DMA h_init per half, quantize pairs ----------------------
    with nc.named_scope("embed"):
        h_t = [g.hpool.tile([128, 2, S], f16, tag="h", name=f"h0_{j}")
               for j in range(NEP)]
        hp_t = [g.hppool.tile([128, 2, S], f8, tag="hp", name=f"ehp{j}")
                for j in range(NEP)]
        dhp_t = [g.dhppool.tile([128, 2, S], f8, tag="dhp", name=f"edhp{j}")
                 for j in range(NEP)]
        for X in HALVES:
            for j in range(NEP):
                hw.dma_start(out=h_t[j][:, :, X], in_=d["hinit"][j][:, :, X])
            for j in range(NEP):
                nc.scalar.activation(hp_t[j][:, :, X], h_t[j][:, :, X],
                                     AF.Copy)
                nc.gpsimd.tensor_sub(dhp_t[j][:, :, X], h_t[j][:, :, X],
                                     hp_t[j][:, :, X])

    if "dbg_h0" in d:
        hw.dma_start(out=d["dbg_h0"], in_=h_t[0][:, 0, :])
        hw.dma_start(out=d["dbg_hp0"], in_=hp_t[0])
    for l in range(L):
        h_t, hp_t, dhp_t = _layer(g, l, h_t, hp_t, dhp_t)

    with nc.named_scope("final"):
        _final(g, h_t, hp_t, dhp_t)


def _gemm3(g, ps, wt, mi, xp, dxp, X, ncp=4, gs=128):
    """3-term fp8 DR chain into ps[:, X]; wt [128, ncp, 2, 2, G] blocks,
    mi slices gs output columns."""
    mm = g.nc.tensor.matmul
    sl = slice(mi * gs, (mi + 1) * gs)
    for cp in range(ncp):
        w8 = wt[:, cp, :, 0, sl]
        dw8 = wt[:, cp, :, 1, sl]
        mm(ps[:, X], w8, xp[cp][:, :, X], start=(cp == 0), stop=False,
           perf_mode=DR)
        mm(ps[:, X], dw8, xp[cp][:, :, X], start=False, stop=False,
           perf_mode=DR)
        mm(ps[:, X], w8, dxp[cp][:, :, X], start=False, stop=(cp == ncp - 1),
           perf_mode=DR)


def _layer(g, l, h_t, hp_t, dhp_t):
    nc = g.nc
    d = g.d
    mm = nc.tensor.matmul
    hw = nc.sync

    # per-layer small params
    bqkv_pp = g.pppool.tile([128, 16], f32, tag="pp16", bufs=2)
    hw.dma_start(out=bqkv_pp, in_=d["bqkv_pp"][l])
    bo_pp = g.pppool.tile([128, 8], f32, tag="pp8", bufs=8)
    hw.dma_start(out=bo_pp, in_=d["bo_pp"][l])
    b1_pp = g.pppool.tile([128, 32], f32, tag="pp32", bufs=2)
    hw.dma_start(out=b1_pp, in_=d["b1_pp"][l])
    b2_pp = g.pppool.tile([128, 8], f32, tag="pp8", bufs=8)
    hw.dma_start(out=b2_pp, in_=d["b2_pp"][l])
    ln_s = [g.pppool.tile([128, 8], f32, tag="pp8", bufs=8,
                          name=f"lns{l}_{i}") for i in range(2)]
    ln_b = [g.pppool.tile([128, 8], f32, tag="pp8", bufs=8,
                          name=f"lnb{l}_{i}") for i in range(2)]
    for i in range(2):
        hw.dma_start(out=ln_s[i], in_=d["ln_s_pp"][l, i])
        hw.dma_start(out=ln_b[i], in_=d["ln_b_pp"][l, i])
    bvb = g.bvpool.tile([128, E], f16, tag="bvb", name=f"bvb{l}")
    hw.dma_start(out=bvb, in_=d["bv_row"][l].to_broadcast((128, E)))

    wq_sb = {}

    def load_wqkv(gi):
        if gi not in wq_sb:
            wt = g.wpool.tile([128, 4, 2, 2, 512], f8, tag="w",
                              name=f"wqkv{l}_{gi}")
            hw.dma_start(out=wt, in_=d["wqkv"][l, gi])
            wq_sb[gi] = wt
        return wq_sb[gi]

    qk_t = [g.qkpool.tile([128, S], f8, tag="qk", name=f"qk{l}_{t}")
            for t in range(16)]  # q 0..7, k 8..15
    v_t = []
    dv_t = []
    for kcp in range(2):
        vt = g.vpool.tile([128, 2, H, DH + 1], f8, tag="v", name=f"v{l}_{kcp}")
        nc.vector.memset(vt[:, :, :, DH:DH + 1], 1.0)
        v_t.append(vt)
        dvt = g.vpool.tile([128, 2, H, DH + 1], f8, tag="dv",
                           name=f"dv{l}_{kcp}")
        nc.vector.memset(dvt[:, :, :, DH:DH + 1], 0.0)
        dv_t.append(dvt)

    def qkv_chain(gi, mi, X):
        wt = load_wqkv(gi)
        mt = gi * 4 + mi
        ps = g.ps_gemm.tile([128, S], f32, tag="gemm", name=f"qkps{l}")
        _gemm3(g, ps, wt, mi, hp_t, dhp_t, X)
        if gi < 2:  # Q
            nc.scalar.activation(qk_t[mt][:, X], ps[:, X], AF.Identity,
                                 bias=bqkv_pp[:, mt:mt + 1],
                                 scale=float(DEQ / np.sqrt(DH)))
        else:  # K on DVE
            nc.vector.tensor_scalar(qk_t[mt][:, X], ps[:, X], float(DEQ),
                                    bqkv_pp[:, mt:mt + 1], OP.mult, OP.add)

    def v_chain(gi, n):
        wt = load_wqkv(4 + gi)
        ps = g.ps_gemm.tile([128, S], f32, tag="gemm", name=f"vps{l}")
        for cp in range(4):
            xs = hp_t[cp][:, :, n * 128:(n + 1) * 128]
            dxs = dhp_t[cp][:, :, n * 128:(n + 1) * 128]
            w8 = wt[:, cp, :, 0, :]
            dw8 = wt[:, cp, :, 1, :]
            mm(ps, xs, w8, start=(cp == 0), stop=False, perf_mode=DR)
            mm(ps, xs, dw8, start=False, stop=False, perf_mode=DR)
            mm(ps, dxs, w8, start=False, stop=(cp == 3), perf_mode=DR)
        tmp = g.tmppool.tile([128, S], f16, tag="vtmp", bufs=2,
                             name=f"vtmp{l}_{gi}_{n}")
        nc.vector.scalar_tensor_tensor(
            tmp, ps, float(DEQ), bvb[:, gi * 512:(gi + 1) * 512],
            OP.mult, OP.add)
        v8s = v_t[n // 2][:, n % 2, gi * 8:(gi + 1) * 8, 0:DH]
        tmpr = tmp.rearrange("p (a b) -> p a b", a=8)
        nc.vector.tensor_copy(v8s, tmpr)
        nc.vector.tensor_sub(dv_t[n // 2][:, n % 2, gi * 8:(gi + 1) * 8, 0:DH],
                             tmpr, v8s)

    # --- attention -----------------------------------------------------------
    ctx16 = {}
    cx_t = [g.cxppool.tile([128, 2, S], f8, tag="cx", name=f"cx{l}_{j}")
            for j in range(NEP)]
    dcx_t = [g.cxppool.tile([128, 2, S], f8, tag="dcx", name=f"dcx{l}_{j}")
             for j in range(NEP)]

    def emit_attn_half(hx, filler):
        X = HALVES[hx]
        nkcp = 1 + hx
        at_q = {}

        def emit_scores(h):
            qt = qk_t[h // 2]
            kt = qk_t[8 + h // 2]
            r0 = (h % 2) * DH
            mt_ = g.maskpool.tile([128, 2 * nkcp, SH], f8,
                                  tag=("mA", "mB")[hx], bufs=2,
                                  name=f"mk{l}_{hx}_{h}")
            hw.dma_start(out=mt_, in_=d["mask"][h, hx, :, 0:2 * nkcp, :])
            spsf = g.ps_att.tile([128, 2, 2, SH], f32, tag="att",
                                 name=f"s{l}_{hx}_{h}")
            sps = spsf[:, 0:nkcp]
            for kcp in range(nkcp):
                mrh = mt_[:, 2 * kcp:2 * kcp + 2, :]
                for kci in range(2):
                    kc = kcp * 2 + kci
                    qlo = max(kc * 128, X.start)
                    mm(sps[:, kcp, kci, :], g.idz[:, kci], mrh,
                       start=True, stop=False, perf_mode=DR)
                    mm(sps[:, kcp, kci, qlo - X.start:],
                       kt[r0:r0 + DH, kc * 128:(kc + 1) * 128],
                       qt[r0:r0 + DH, qlo:X.stop], start=False, stop=True)
            at = g.atpool.tile([128, nkcp, 2, SH], f8,
                               tag=("atA", "atB")[hx], bufs=3 - hx,
                               name=f"a{l}_{hx}_{h}")
            nc.scalar.activation(at, sps, AF.Exp)
            if l == 0 and h == 0 and f"dbg_at{'AB'[hx]}" in d:
                hw.dma_start(out=d[f"dbg_at{'AB'[hx]}"], in_=at)
            at_q[h] = at

        def emit_av(h):
            at = at_q.pop(h)
            cps = g.ps_gemm.tile([128, S], f32, tag="gemm", name=f"c{l}_{hx}")
            for kcp in range(nkcp):
                mm(cps[0:DH + 1, X], v_t[kcp][:, :, h, :], at[:, kcp],
                   start=(kcp == 0), stop=False, perf_mode=DR)
                mm(cps[0:DH + 1, X], dv_t[kcp][:, :, h, :], at[:, kcp],
                   start=False, stop=(kcp == nkcp - 1), perf_mode=DR)
            srow = g.srec.tile([1, SH], f32, tag="sw", bufs=2,
                               name=f"sw{l}_{hx}_{h}")
            nc.scalar.activation(srow, cps[DH:DH + 1, X], AF.Copy)
            rec = g.srec.tile([1, SH], f32, tag="sr", bufs=1,
                              name=f"re{l}_{hx}_{h}")
            nc.vector.reciprocal_approx_fast(out=rec, in_=srow)
            recR = g.recpool.tile([DH, SH], f32, tag="recf",
                                  name=f"rr{l}_{hx}_{h}")
            nc.gpsimd.partition_broadcast(recR, rec, channels=DH)
            j, i = h // 4, (h // 2) % 2
            hh = h % 2
            if j not in ctx16:
                ctx16[j] = g.ctxpool.tile([128, 2, SH], f16, tag="ctx",
                                          name=f"cxt{l}_{hx}_{j}")
            nc.vector.tensor_mul(ctx16[j][hh * DH:(hh + 1) * DH, i, :],
                                 cps[0:DH, X], recR)
            if i == 1 and hh == 1:
                ct = ctx16.pop(j)
                nc.vector.tensor_copy(cx_t[j][:, :, X], ct)
                nc.vector.tensor_sub(dcx_t[j][:, :, X], ct,
                                     cx_t[j][:, :, X])

        fi = iter(filler)
        emit_scores(0)
        emit_scores(1)
        for h in range(2, H):
            emit_scores(h)
            emit_av(h - 2)
            for fn in (next(fi, None), next(fi, None)):
                if fn is not None:
                    fn()
        emit_av(H - 2)
        emit_av(H - 1)
        for fn in fi:
            if fn is not None:
                fn()

    # --- out-proj ------------------------------------------------------------
    wo_sb = {}

    def load_wo(gi):
        if gi not in wo_sb:
            wt = g.wpool.tile([128, 4, 2, 2, 512], f8, tag="w",
                              name=f"wo{l}_{gi}")
            hw.dma_start(out=wt, in_=d["wo8"][l, gi])
            wo_sb[gi] = wt
        return wo_sb[gi]

    r1_t = [g.rpool.tile([128, 2, S], f16, tag="r", name=f"r1_{l}_{j}")
            for j in range(NEP)]

    def wo_chain(gi, mi, X):
        wt = load_wo(gi)
        mt = gi * 4 + mi
        ps = g.ps_gemm.tile([128, S], f32, tag="gemm", name=f"wops{l}")
        _gemm3(g, ps, wt, mi, cx_t, dcx_t, X)
        wo_o = g.tmppool.tile([128, SH], f16, tag="f2o", bufs=2,
                              name=f"woo{l}_{mt}")
        nc.scalar.activation(wo_o, ps[:, X], AF.Identity,
                             bias=bo_pp[:, mt:mt + 1], scale=DEQ)
        nc.vector.tensor_add(r1_t[mt // 2][:, mt % 2, X], wo_o,
                             h_t[mt // 2][:, mt % 2, X])

    # --- emit: QKV (A leads, B lags 2 blocks); attention; Wo -----------------
    with nc.named_scope(f"L{l}_qkv"):
        # A-g0, A-g1, B-g0, A-g2, B-g1, A-g3, B-g2, B-g3, then V half A;
        # V half B (token blocks 2,3) is emitted as attn_A filler.
        seq = [("qk", 0, 0), ("qk", 1, 0), ("qk", 0, 1), ("qk", 2, 0),
               ("qk", 1, 1), ("qk", 3, 0), ("qk", 2, 1), ("qk", 3, 1),
               ("v", 0, 0), ("v", 0, 1), ("v", 1, 0), ("v", 1, 1)]
        for kind, gi, b in seq:
            if kind == "qk":
                for mi in range(4):
                    qkv_chain(gi, mi, HALVES[b])
            else:
                v_chain(gi, b)

    with nc.named_scope(f"L{l}_attn"):
        load_wo(0)
        load_wo(1)
        emit_attn_half(0, [lambda gi=gi, n=n: v_chain(gi, n)
                           for gi in range(2) for n in (2, 3)])
        emit_attn_half(1, [None] * 6 +
                       [lambda gi=gi, mi=mi: wo_chain(gi, mi, HALVES[0])
                        for gi in range(2) for mi in range(4)])

    if l == 0 and "dbg_q0" in d:
        hw.dma_start(out=d["dbg_q0"], in_=qk_t[0])
        hw.dma_start(out=d["dbg_k0"], in_=qk_t[8])
        hw.dma_start(out=d["dbg_v0"], in_=v_t[0])
        hw.dma_start(out=d["dbg_cx0"], in_=cx_t[0])
        hw.dma_start(out=d["dbg_r1"], in_=r1_t[0][:, 0, :])
    h1_t = [g.hpool.tile([128, 2, S], f16, tag="h", name=f"h1_{l}_{j}")
            for j in range(NEP)]
    h1p_t = [g.hppool.tile([128, 2, S], f8, tag="hp", name=f"h1p{l}_{j}")
             for j in range(NEP)]
    dh1p_t = [g.dhppool.tile([128, 2, S], f8, tag="dhp", name=f"dh1p{l}_{j}")
              for j in range(NEP)]
    with nc.named_scope(f"L{l}_ln1a"):
        _ln_half(g, 0, r1_t, ln_s[0], ln_b[0], h1_t, h1p_t, dh1p_t,
                 tag=f"l1{l}")
    with nc.named_scope(f"L{l}_wob"):
        for gi in range(2):
            for mi in range(4):
                wo_chain(gi, mi, HALVES[1])
    with nc.named_scope(f"L{l}_ln1b"):
        _ln_half(g, 1, r1_t, ln_s[0], ln_b[0], h1_t, h1p_t, dh1p_t,
                 tag=f"l1{l}")

    # --- FFN -----------------------------------------------------------------
    with nc.named_scope(f"L{l}_ffn"):
        ffp_t = [g.ffpool.tile([128, 2, S], f8, tag="ff", name=f"ff{l}_{j}")
                 for j in range(NF // 2)]
        dffp_t = [g.ffpool.tile([128, 2, S], f8, tag="dff",
                                name=f"dff{l}_{j}") for j in range(NF // 2)]
        w1_sb = {}

        def load_w1(b):
            if b in w1_sb or b > 15:
                return
            wt = g.w1pool.tile([128, 4, 2, 2, 256], f8, tag="w1",
                               name=f"w1_{l}_{b}")
            hw.dma_start(out=wt, in_=d["w1"][l, b])
            w1_sb[b] = wt

        def ffn1_block(b, X):
            wt = w1_sb[b]
            ft = g.tmppool.tile([128, 2, SH], f16, tag="ffh", bufs=2,
                                name=f"ffh{l}_{b}")
            for mi in range(2):
                mt = 2 * b + mi
                ps = g.ps_gemm.tile([128, S], f32, tag="gemm", name=f"f1ps{l}")
                _gemm3(g, ps, wt, mi, h1p_t, dh1p_t, X)
                nc.scalar.activation(ft[:, mi], ps[:, X], AF.Gelu,
                                     bias=b1_pp[:, mt:mt + 1], scale=DEQ)
            nc.vector.tensor_copy(ffp_t[b][:, :, X], ft)
            nc.vector.tensor_sub(dffp_t[b][:, :, X], ft,
                                 ffp_t[b][:, :, X])

        if l == 0 and "dbg_h1" in d:
            hw.dma_start(out=d["dbg_h1"], in_=h1_t[0][:, 0, :])
        load_w1(0)
        load_w1(1)
        for b in range(16):
            load_w1(b + 1)
            ffn1_block(b, HALVES[0])
            if b >= 3:
                ffn1_block(b - 3, HALVES[1])
        for b in range(13, 16):
            ffn1_block(b, HALVES[1])

        # FFN2: 8 blocks of 128 outputs x 16 cp; A/B pairwise per block
        r2_t = [g.rpool.tile([128, 2, S], f16, tag="r", name=f"r2_{l}_{j}")
                for j in range(NEP)]
        h2_t = [g.hpool.tile([128, 2, S], f16, tag="h", name=f"h2_{l}_{j}")
                for j in range(NEP)]
        h2p_t = [g.hppool.tile([128, 2, S], f8, tag="hp", name=f"h2p{l}_{j}")
                 for j in range(NEP)]
        dh2p_t = [g.dhppool.tile([128, 2, S], f8, tag="dhp",
                                 name=f"dh2p{l}_{j}") for j in range(NEP)]
        w2_sb = {}

        def load_w2(b):
            if b in w2_sb or b > 7:
                return
            wt = g.w2pool.tile([128, 16, 2, 2, 128], f8, tag="w2",
                               name=f"w2_{l}_{b}")
            hw.dma_start(out=wt, in_=d["w2"][l, b])
            w2_sb[b] = wt

        def ffn2_block(b, X):
            mt = b
            wt = w2_sb[b]
            ps = g.ps_gemm.tile([128, S], f32, tag="gemm", name=f"f2ps{l}")
            for cp in range(16):
                w8 = wt[:, cp, :, 0, :]
                dw8 = wt[:, cp, :, 1, :]
                mm(ps[:, X], w8, ffp_t[cp][:, :, X], start=(cp == 0),
                   stop=False, perf_mode=DR)
                mm(ps[:, X], dw8, ffp_t[cp][:, :, X], start=False,
                   stop=False, perf_mode=DR)
                mm(ps[:, X], w8, dffp_t[cp][:, :, X], start=False,
                   stop=(cp == 15), perf_mode=DR)
            f2o = g.tmppool.tile([128, SH], f16, tag="f2o", bufs=2,
                                 name=f"f2o{l}_{mt}")
            nc.scalar.activation(f2o, ps[:, X], AF.Identity,
                                 bias=b2_pp[:, mt:mt + 1], scale=DEQ)
            nc.vector.tensor_add(r2_t[mt // 2][:, mt % 2, X], f2o,
                                 h1_t[mt // 2][:, mt % 2, X])

        load_w2(0)
        for b in range(8):
            load_w2(b + 1)
            ffn2_block(b, HALVES[0])
            if b == 7:
                with nc.named_scope(f"L{l}_ln2a"):
                    _ln_half(g, 0, r2_t, ln_s[1], ln_b[1], h2_t, h2p_t,
                             dh2p_t, tag=f"l2{l}")
            ffn2_block(b, HALVES[1])
        with nc.named_scope(f"L{l}_ln2b"):
            _ln_half(g, 1, r2_t, ln_s[1], ln_b[1], h2_t, h2p_t, dh2p_t,
                     tag=f"l2{l}")
        if l == 0 and "dbg_ff0" in d:
            hw.dma_start(out=d["dbg_ff0"], in_=ffp_t[0])
            hw.dma_start(out=d["dbg_r2"], in_=r2_t[0][:, 0, :])

    return h2_t, h2p_t, dh2p_t


def _ln_half(g, hx, r_t, s_pp, b_pp, out_t, outp_t, outdp_t, tag="",
             want_dp=True):
    """LayerNorm of half hx on pair tiles: fp8-DR stats, pair-op vector
    chain, quantized pair outputs (residual pairs optional)."""
    nc = g.nc
    mm = nc.tensor.matmul
    X = HALVES[hx]

    r8 = [g.r8pool.tile([128, 2, S], f8, tag="r8", name=f"r8{tag}_{hx}_{j}")
          for j in range(NEP)]
    sq8 = [g.r8pool.tile([128, 2, S], f8, tag="sq8", name=f"sq{tag}_{hx}_{j}")
           for j in range(NEP)]
    for j in range(NEP):
        nc.gpsimd.tensor_copy(r8[j][:, :, X], r_t[j][:, :, X])
        nc.vector.tensor_mul(sq8[j][:, :, X], r8[j][:, :, X], r8[j][:, :, X])

    sums_r2 = g.ps_gemm.tile([128, S], f32, tag="gemm", name=f"lsr{tag}_{hx}")
    sums_q2 = g.ps_gemm.tile([128, S], f32, tag="gemm", name=f"lsq{tag}_{hx}")
    sums_r = sums_r2[0:1]
    sums_q = sums_q2[0:1]
    for j in range(NEP):
        mm(sums_r2[:, X], g.ones8, r8[j][:, :, X], start=(j == 0),
           stop=(j == NEP - 1), perf_mode=DR)
    for j in range(NEP):
        mm(sums_q2[:, X], g.ones8, sq8[j][:, :, X], start=(j == 0),
           stop=(j == NEP - 1), perf_mode=DR)

    mean_b = g.smallf.tile([1, SH], f16, tag="sb", bufs=3,
                           name=f"mn{tag}_{hx}")
    nc.vector.tensor_scalar(mean_b, sums_r[:, X], 1.0 / E, None, OP.mult)
    meanR = g.bcpool.tile([128, 2, SH], f16, tag="rec", name=f"mR{tag}_{hx}")
    nc.gpsimd.partition_broadcast(meanR[:, 0], mean_b, channels=128)
    nc.gpsimd.tensor_copy(meanR[:, 1], meanR[:, 0])

    s2 = g.smallf.tile([1, SH], f32, tag="sf", bufs=3, name=f"s2{tag}_{hx}")
    nc.scalar.activation(s2, sums_r[:, X], AF.Square)
    varE = g.smallf.tile([1, SH], f32, tag="sf", bufs=3,
                         name=f"vE{tag}_{hx}")
    nc.vector.scalar_tensor_tensor(varE, s2, -1.0 / E, sums_q[:, X],
                                   OP.mult, OP.add)
    std = g.smallf.tile([1, SH], f32, tag="sf", bufs=3, name=f"st{tag}_{hx}")
    nc.scalar.activation(std, varE, AF.Sqrt, bias=g.eps_t[:1, :],
                         scale=1.0 / E)
    rstd = g.smallf.tile([1, SH], f32, tag="sf", bufs=3,
                         name=f"rs{tag}_{hx}")
    nc.vector.reciprocal_approx_fast(out=rstd, in_=std)
    rstd_b = g.smallf.tile([1, SH], f16, tag="sb", bufs=3,
                           name=f"rb{tag}_{hx}")
    nc.vector.tensor_copy(rstd_b, rstd)
    rstdR = g.bcpool.tile([128, 2, SH], f16, tag="rec", name=f"rR{tag}_{hx}")
    nc.gpsimd.partition_broadcast(rstdR[:, 0], rstd_b, channels=128)
    nc.gpsimd.tensor_copy(rstdR[:, 1], rstdR[:, 0])

    for j in range(NEP):
        t2 = g.tmppool.tile([128, 2, SH], f16, tag="t2", bufs=1,
                            name=f"t2{tag}_{hx}_{j}")
        nc.vector.tensor_sub(t2, r_t[j][:, :, X], meanR)
        t1 = g.tmppool.tile([128, 2, SH], f16, tag="t1", bufs=2,
                            name=f"t1{tag}_{hx}_{j}")
        nc.vector.tensor_mul(t1, t2, rstdR)
        for i in range(2):
            c = 2 * j + i
            nc.vector.tensor_scalar(out_t[j][:, i, X], t1[:, i], 
                                    s_pp[:, c:c + 1], b_pp[:, c:c + 1],
                                    OP.mult, OP.add)
        nc.scalar.activation(outp_t[j][:, :, X], out_t[j][:, :, X], AF.Copy)
        if want_dp:
            nc.gpsimd.tensor_sub(outdp_t[j][:, :, X], out_t[j][:, :, X],
                                 outp_t[j][:, :, X])


def _final(g, h_t, hp_t, dhp_t):
    nc = g.nc
    d = g.d
    mm = nc.tensor.matmul
    hw = nc.sync

    mrstd = g.smallf.tile([1, S], f16, tag="mrs", bufs=1, name="fmr")
    rstdR = g.bcpool.tile([128, S], f16, tag="rec", bufs=2, name="frR")

    for hx, X in enumerate(HALVES):
        sq8 = [g.r8pool.tile([128, 2, S], f8, tag="sq8",
                             name=f"fsq{hx}_{j}") for j in range(NEP)]
        for j in range(NEP):
            nc.vector.tensor_mul(sq8[j][:, :, X], hp_t[j][:, :, X],
                                 hp_t[j][:, :, X])
        sums_r2 = g.ps_gemm.tile([128, S], f32, tag="gemm", name=f"fsr{hx}")
        sums_q2 = g.ps_gemm.tile([128, S], f32, tag="gemm", name=f"fsq{hx}")
        sums_r = sums_r2[0:1]
        sums_q = sums_q2[0:1]
        for j in range(NEP):
            mm(sums_r2[:, X], g.ones8, hp_t[j][:, :, X], start=(j == 0),
               stop=(j == NEP - 1), perf_mode=DR)
        for j in range(NEP):
            mm(sums_q2[:, X], g.ones8, sq8[j][:, :, X], start=(j == 0),
               stop=(j == NEP - 1), perf_mode=DR)
        s2 = g.smallf.tile([1, SH], f32, tag="sf", bufs=3, name=f"fs2{hx}")
        nc.scalar.activation(s2, sums_r[:, X], AF.Square)
        varE = g.smallf.tile([1, SH], f32, tag="sf", bufs=3, name=f"fvE{hx}")
        nc.vector.scalar_tensor_tensor(varE, s2, -1.0 / E, sums_q[:, X],
                                       OP.mult, OP.add)
        std = g.smallf.tile([1, SH], f32, tag="sf", bufs=3, name=f"fst{hx}")
        nc.scalar.activation(std, varE, AF.Sqrt, bias=g.eps_t[:1, :],
                             scale=1.0 / E)
        rstd = g.smallf.tile([1, SH], f32, tag="sf", bufs=3, name=f"frs{hx}")
        nc.vector.reciprocal_approx_fast(out=rstd, in_=std)
        rstd_b = g.smallf.tile([1, SH], f16, tag="sb", bufs=3,
                               name=f"frb{hx}")
        nc.vector.tensor_copy(rstd_b, rstd)
        nc.gpsimd.partition_broadcast(rstdR[:, X], rstd_b, channels=128)
        nc.vector.scalar_tensor_tensor(mrstd[:, X], sums_r[:, X], 1.0 / E,
                                       rstd, OP.mult, OP.mult)
    hw.dma_start(out=d["mrstd"], in_=mrstd)

    for mt in range(NO):
        gi, mi = divmod(mt, 2)
        if mi == 0:
            wt = g.w2pool.tile([128, 4, 2, 2, 256], f8, tag="gw", bufs=2,
                               name=f"gw{gi}")
            hw.dma_start(out=wt, in_=d["genw8"][gi])
            _final.wt = wt
        ps = g.ps_gemm.tile([128, S], f32, tag="gemm", name=f"hd{mt}")
        _gemm3(g, ps, _final.wt, mi, hp_t, dhp_t, slice(0, S))
        gt = g.outpool.tile([128, S], f16, tag="f16out", name=f"gt{mt}")
        nc.vector.scalar_tensor_tensor(gt, ps, float(DEQ), rstdR,
                                       OP.mult, OP.mult)
        hw.dma_start(out=d["logits"][mt], in_=gt)


def _build():
    if "nc" in _CACHE:
        return _CACHE["nc"]
    from contextlib import ExitStack

    nc = bacc.Bacc("TRN2", debug=False)
    d = _declare(nc)
    with tile.TileContext(nc) as tc:
        with ExitStack() as ctx:
            _emit(nc, tc, d, ctx)
    nc.compile()
    _CACHE["nc"] = nc
    return nc


def kernel_internal(inputs, trace=False, trace_kwargs=None):
    shared = _prep_shared(inputs)
    cores = _prep_percore(inputs)
    nc = _build()
    in_maps = []
    for b in range(B):
        m = dict(shared)
        m.update(cores[b])
        in_maps.append(m)
    res = run_bass_kernel_spmd(
        nc, in_maps, core_ids=list(range(B)), trace=trace,
        **(trace_kwargs or {}),
    )
    nws = _CACHE["nws"]
    gb = _CACHE["gb"]
    outs = []
    for b in range(B):
        lo = np.asarray(res.results[b]["logits"], np.float32)  # [10,128,512]
        mr = np.asarray(res.results[b]["mrstd"], np.float32).reshape(S)
        lo = lo.reshape(NO * 128, S)[:VV * VR].T  # [512, 1200]
        lo = lo + mr[:, None] * nws[None, :] + gb[None, :]
        outs.append(lo)
    out = np.stack(outs).astype(np.float32)  # [B, S, 1200]
    return out, res


def kernel(**inputs):
    out, _ = kernel_internal(inputs)
    return out


# revision 34
# speedup vs baseline: 1.3015x; 1.0447x over previous
"""Trainium2 Bass kernel for nn_BaseGenerator (4-layer dense transformer).

Strategy: pure data-parallel over batch (B=8 -> 8 NeuronCores, no
collectives).  Each core runs the full transformer on one batch element.

v2 scheme (cost model: fp8-e4m3 DoubleRow = 0.5 cycle/col, K=256/instr):
  - ALL GEMMs (QKV, Wo, FFN1, FFN2, head) fp8 DR with 3-term error
    compensation (w8*x8 + dw8*x8 + w8*dx8); weights pre-scaled by 2^8.
  - q/k/at stored fp8; scores fp8 non-DR (1 cyc/col); AV fp8 DR with
    v-residual compensation; softmax via exp + ones-row denominator.
  - attention causally trimmed: mask idz-matmuls / scores / exp / AV only
    cover q >= k-block (masked region written -240 by the idz matmul).
  - LN statistics via fp8-DR ones-matmuls on quantized r8/sq8 pair tiles.
  - token-half software pipelining (A=[0,256) B=[256,512)): GEMM halves are
    emitted with a lag so each half's LN vector chain overlaps the other
    half's matmuls; PE stays busy across LN boundaries.
  - embedding gather and the final-head affine fold (mean*rstd*rowsum(Wp)
    + genW@lnf_b + gen_b) are host-side; device ships f16 rstd-scaled
    logits + an f32 mean*rstd row.
"""

import os
import sys

for _p in ("/opt/trn_rl_repo",):
    if _p not in sys.path:
        sys.path.insert(0, _p)

import ml_dtypes
import numpy as np

import concourse.bass as bass
import concourse.mybir as mybir
import concourse.tile as tile
from concourse import bacc
from concourse.bass_utils import run_bass_kernel_spmd

F16 = np.float16
F8 = ml_dtypes.float8_e4m3

L, E, H, F = 4, 1024, 16, 4096
B, S = 8, 512
SH = S // 2  # half width 256
VV, VR = 40, 30
DIST_V = 200
PAD_ID = 0
DH = E // H  # 64
NE = E // 128  # 8 feature chunks
NEP = NE // 2  # 4 fp8 pair tiles
NF = F // 128  # 32
NO = 10  # logit row tiles (1280 padded)
NEG = -240.0

WSC = 256.0
DEQ = 1.0 / WSC

f32 = mybir.dt.float32
f16 = mybir.dt.float16
f8 = mybir.dt.float8e4
AF = mybir.ActivationFunctionType
OP = mybir.AluOpType
DR = mybir.MatmulPerfMode.DoubleRow

_CACHE = {}

HALVES = (slice(0, SH), slice(SH, S))


# ----------------------------------------------------------------------------
# host-side input prep
# ----------------------------------------------------------------------------

def _f8(x):
    return np.ascontiguousarray(np.asarray(x, np.float32).astype(F8))


def _f16(x):
    return np.ascontiguousarray(np.asarray(x, np.float32).astype(F16))


def _block_dr2(W, gsize):
    """W [O, I] -> fp8 (w8, dw8) blocks [G, 128, I//256, 2(i), 2(t), gsize]
    with [g, p, cp, i, t, o] <- W[g*gsize + o, cp*256 + i*128 + p]*WSC,
    t=0: e4m3 quant, t=1: e4m3 residual."""
    O, I = W.shape
    Ws = np.asarray(W, np.float32) * WSC
    w8 = Ws.astype(F8).astype(np.float32)
    dw8 = (Ws - w8).astype(F8).astype(np.float32)
    out = np.empty((O // gsize, 128, I // 256, 2, 2, gsize), F8)
    for t, wv in enumerate((w8, dw8)):
        Wb = wv.reshape(O // gsize, gsize, I // 256, 2, 128)  # g o cp i p
        out[:, :, :, :, t, :] = Wb.transpose(0, 4, 2, 3, 1).astype(F8)
    return np.ascontiguousarray(out)


def _pp(v):  # [..., N*128] -> [..., 128, N]
    *lead, N = v.shape
    return np.ascontiguousarray(
        v.reshape(*lead, N // 128, 128).swapaxes(-1, -2).astype(np.float32)
    )


def _prep_shared(inp):
    out = {}

    Wqkv = np.asarray(inp["Wqkv"], np.float32).copy()  # [L, 3E, E]
    bqkv = np.asarray(inp["bqkv"], np.float32).copy()  # [L, 3E]
    att_sc = 1.0 / np.sqrt(DH)
    bqkv[:, :E] *= att_sc

    out["wqkv"] = np.stack([_block_dr2(Wqkv[l], 512) for l in range(L)])
    Wo = np.asarray(inp["Wo"], np.float32)
    out["wo8"] = np.stack([_block_dr2(Wo[l], 512) for l in range(L)])
    W1 = np.asarray(inp["W1"], np.float32)
    out["w1"] = np.stack([_block_dr2(W1[l], 256) for l in range(L)])
    W2 = np.asarray(inp["W2"], np.float32)
    out["w2"] = np.stack([_block_dr2(W2[l], 128) for l in range(L)])

    genW = np.asarray(inp["gen_W"], np.float32)  # [1200, E]
    lnf_s_v = np.asarray(inp["lnf_s"], np.float32)
    lnf_b_v = np.asarray(inp["lnf_b"], np.float32)
    Wp = genW * lnf_s_v[None, :]
    Wp_pad = np.zeros((1280, E), np.float32)
    Wp_pad[:1200] = Wp
    out["genw8"] = _block_dr2(Wp_pad, 256)  # [5, 128, 4, 2, 2, 256]
    _CACHE["nws"] = -Wp.sum(1)
    _CACHE["gb"] = genW @ lnf_b_v + np.asarray(inp["gen_b"], np.float32)

    out["bqkv_pp"] = _pp(bqkv[:, : 2 * E])  # [L, 128, 16]
    out["bv_row"] = _f16(bqkv[:, 2 * E:].reshape(L, 1, E))  # [L, 1, E]
    out["bo_pp"] = _pp(np.asarray(inp["bo"], np.float32))
    out["b1_pp"] = _pp(np.asarray(inp["b1"], np.float32))
    out["b2_pp"] = _pp(np.asarray(inp["b2"], np.float32))

    ln_s = np.stack([np.asarray(inp["ln1_s"], np.float32),
                     np.asarray(inp["ln2_s"], np.float32)], 1)
    ln_b = np.stack([np.asarray(inp["ln1_b"], np.float32),
                     np.asarray(inp["ln2_b"], np.float32)], 1)
    out["ln_s_pp"] = _pp(ln_s)  # [L, 2, 128, 8]
    out["ln_b_pp"] = _pp(ln_b)

    idz = np.zeros((128, 2, 2, 128), np.float32)
    for v in range(2):
        idz[:, v, v, :] = np.eye(128)
    out["idz"] = _f8(idz)
    out["ones8"] = _f8(np.ones((128, 2, 128), np.float32))
    return out


def _prep_percore(inp):
    val = np.asarray(inp["val_sequences"]).astype(np.int64)
    ring = np.asarray(inp["ring_sequences"]).astype(np.int64)
    dist = np.asarray(inp["distance_squares"]).astype(np.int64)
    de = np.asarray(inp["dist_emb"], np.float32)  # [200, H]

    # embedding (f16 tables, f32 math, f16 result) == device one-hot matmul
    ve = np.asarray(inp["val_emb"], np.float32).astype(F16).astype(np.float32)
    re = np.asarray(inp["ring_emb"], np.float32).astype(F16).astype(np.float32)
    h = (ve[val] + re[ring]) * np.sqrt(E)  # [B, S, E] f32
    # pair layout [NEP, 128, 2, S]: slot i = feature chunk 2j+i
    hB = h.reshape(B, S, NEP, 2, 128).transpose(0, 2, 4, 3, 1)
    hinit = np.ascontiguousarray(hB.astype(F16))

    m = de[dist].astype(F8).astype(np.float32)  # [B, q, k, H]
    m = m.transpose(0, 3, 2, 1)          # [B, H, k, q]
    kk = np.arange(S)
    causal = kk[:, None] <= kk[None, :]  # keep k <= q
    m = np.where(causal[None, None], m, NEG)
    padk = val == PAD_ID
    m = np.where(padk[:, None, :, None], NEG, m)
    # -> [B, H, 2(qh), 128(p), 4(kc), SH(qi)]: k = kc*128+p, q = qh*256+qi
    m = m.reshape(B, H, 4, 128, 2, SH).transpose(0, 1, 4, 3, 2, 5)
    m = np.ascontiguousarray(m.astype(F8))

    return [{"mask": m[b], "hinit": hinit[b]} for b in range(B)]


# ----------------------------------------------------------------------------
# device program
# ----------------------------------------------------------------------------

def _declare(nc):
    d = {}

    def di(name, shape, dt):
        d[name] = nc.dram_tensor(name, list(shape), dt, kind="ExternalInput").ap()

    di("wqkv", (L, 6, 128, 4, 2, 2, 512), f8)
    di("wo8", (L, 2, 128, 4, 2, 2, 512), f8)
    di("w1", (L, 16, 128, 4, 2, 2, 256), f8)
    di("w2", (L, 8, 128, 16, 2, 2, 128), f8)
    di("genw8", (5, 128, 4, 2, 2, 256), f8)
    di("bqkv_pp", (L, 128, 16), f32)
    di("bv_row", (L, 1, E), f16)
    di("bo_pp", (L, 128, 8), f32)
    di("b1_pp", (L, 128, 32), f32)
    di("b2_pp", (L, 128, 8), f32)
    di("ln_s_pp", (L, 2, 128, 8), f32)
    di("ln_b_pp", (L, 2, 128, 8), f32)
    di("idz", (128, 2, 2, 128), f8)
    di("ones8", (128, 2, 128), f8)
    di("mask", (H, 2, 128, 4, SH), f8)
    di("hinit", (NEP, 128, 2, S), f16)
    d["logits"] = nc.dram_tensor("logits", [NO, 128, S], f16,
                                 kind="ExternalOutput").ap()
    if os.environ.get("BG_DEBUG"):
        def do(name, shape, dt=f16):
            d[name] = nc.dram_tensor(name, list(shape), dt,
                                     kind="ExternalOutput").ap()
        do("dbg_h0", (128, S))
        do("dbg_hp0", (128, 2, S), f8)
        do("dbg_q0", (128, S), f8)
        do("dbg_k0", (128, S), f8)
        do("dbg_v0", (128, 2, H, DH + 1), f8)
        do("dbg_atA", (128, 1, 2, SH), f8)
        do("dbg_atB", (128, 2, 2, SH), f8)
        do("dbg_cx0", (128, 2, S), f8)
        do("dbg_r1", (128, S))
        do("dbg_h1", (128, S))
        do("dbg_ff0", (128, 2, S), f8)
        do("dbg_r2", (128, S))
    d["mrstd"] = nc.dram_tensor("mrstd", [1, S], f16,
                                kind="ExternalOutput").ap()
    return d


class Ctx:
    pass


def _emit(nc, tc, d, ctx):
    hw = nc.sync
    g = Ctx()
    g.nc = nc
    g.d = d

    def pool(name, bufs, space="SBUF"):
        return ctx.enter_context(
            tc.tile_pool(name=name, bufs=bufs, space=space))

    g.wpool = pool("wpool", 3)      # 8KB: wqkv + wo blocks
    g.w1pool = pool("w1pool", 5)    # 4KB: ffn1 blocks (lag-3 span)
    g.w2pool = pool("w2pool", 2)    # 8KB: ffn2 blocks
    g.maskpool = pool("maskpool", 2)
    g.hpool = pool("hpool", 8)    # h/h1/h2 residual pair tiles [128,2,S]
    g.rpool = pool("rpool", 4)     # r1/r2 LN-input pair tiles [128,2,S]
    g.hppool = pool("hppool", 8)
    g.dhppool = pool("dhppool", 8)
    g.r8pool = pool("r8pool", 4)
    g.qkpool = pool("qkpool", 16)   # f8 [128,S]
    g.vpool = pool("vpool", 2)
    g.atpool = pool("atpool", 6)
    g.ctxpool = pool("ctxpool", 2)
    g.cxppool = pool("cxppool", 4)
    g.ffpool = pool("ffpool", 16)
    g.tmppool = pool("tmppool", 4)
    g.smallf = pool("smallf", 8)    # [1, SH]
    g.srec = pool("srec", 2)        # [1, SH] f32 softmax denom
    g.recpool = pool("recpool", 2)  # [DH, SH] f32
    g.bcpool = pool("bcpool", 2)    # [128, S] f16 broadcasts
    g.outpool = pool("outpool", 2)
    g.pppool = pool("pppool", 4)
    g.bvpool = pool("bvpool", 1)

    g.ps_gemm = pool("ps_gemm", 4, "PSUM")
    g.ps_att = pool("ps_att", 2, "PSUM")

    cpool = pool("cpool", 1)
    g.idz = cpool.tile([128, 2, 2, 128], f8)
    hw.dma_start(out=g.idz, in_=d["idz"])
    g.ones8 = cpool.tile([128, 2, 128], f8)
    hw.dma_start(out=g.ones8, in_=d["ones8"])
    g.eps_t = cpool.tile([128, 1], f32)
    nc.vector.memset(g.eps_t, 1e-5)

    # --- embedding: DMA h_init, quantize pairs (no residual: QKV is 2-term)
    g.wq_prefetch = {}
    for gi in range(2):
        wt = g.wpool.tile([128, 4, 2, 2, 512], f8, tag="w", name=f"wqkv0_{gi}")
        hw.dma_start(out=wt, in_=d["wqkv"][0, gi])
        g.wq_prefetch[gi] = wt
    with nc.named_scope("embed"):
        h_t = []
        for j in range(NEP):
            ht = g.hpool.tile([128, 2, S], f16, tag="h", name=f"h0_{j}")
            hw.dma_start(out=ht, in_=d["hinit"][j])
            h_t.append(ht)
        hp_t = [g.hppool.tile([128, 2, S], f8, tag="hp", name=f"ehp{j}")
                for j in range(NEP)]
        dhp_t = None
        for X in HALVES:
            for j in range(NEP):
                nc.scalar.activation(hp_t[j][:, :, X], h_t[j][:, :, X],
                                     AF.Copy)

    if "dbg_h0" in d:
        hw.dma_start(out=d["dbg_h0"], in_=h_t[0][:, 0, :])
        hw.dma_start(out=d["dbg_hp0"], in_=hp_t[0])
    for l in range(L):
        h_t, hp_t, dhp_t = _layer(g, l, h_t, hp_t, dhp_t)

    with nc.named_scope("final"):
        _final(g, h_t, hp_t, dhp_t)


def _gemm2(g, ps, wt, mi, xp, X, ncp=4, gs=128):
    """2-term fp8 DR chain (weight-compensated only) into ps[:, X]."""
    mm = g.nc.tensor.matmul
    sl = slice(mi * gs, (mi + 1) * gs)
    for cp in range(ncp):
        w8 = wt[:, cp, :, 0, sl]
        dw8 = wt[:, cp, :, 1, sl]
        mm(ps[:, X], w8, xp[cp][:, :, X], start=(cp == 0), stop=False,
           perf_mode=DR)
        mm(ps[:, X], dw8, xp[cp][:, :, X], start=False, stop=(cp == ncp - 1),
           perf_mode=DR)


def _gemm3(g, ps, wt, mi, xp, dxp, X, ncp=4, gs=128):
    """3-term fp8 DR chain into ps[:, X]; wt [128, ncp, 2, 2, G] blocks,
    mi slices gs output columns."""
    mm = g.nc.tensor.matmul
    sl = slice(mi * gs, (mi + 1) * gs)
    for cp in range(ncp):
        w8 = wt[:, cp, :, 0, sl]
        dw8 = wt[:, cp, :, 1, sl]
        mm(ps[:, X], w8, xp[cp][:, :, X], start=(cp == 0), stop=False,
           perf_mode=DR)
        mm(ps[:, X], dw8, xp[cp][:, :, X], start=False, stop=False,
           perf_mode=DR)
        mm(ps[:, X], w8, dxp[cp][:, :, X], start=False, stop=(cp == ncp - 1),
           perf_mode=DR)


def _layer(g, l, h_t, hp_t, dhp_t):
    nc = g.nc
    d = g.d
    mm = nc.tensor.matmul
    hw = nc.sync

    # per-layer small params
    bqkv_pp = g.pppool.tile([128, 16], f32, tag="pp16", bufs=2)
    hw.dma_start(out=bqkv_pp, in_=d["bqkv_pp"][l])
    bo_pp = g.pppool.tile([128, 8], f32, tag="pp8", bufs=8)
    hw.dma_start(out=bo_pp, in_=d["bo_pp"][l])
    b1_pp = g.pppool.tile([128, 32], f32, tag="pp32", bufs=2)
    hw.dma_start(out=b1_pp, in_=d["b1_pp"][l])
    b2_pp = g.pppool.tile([128, 8], f32, tag="pp8", bufs=8)
    hw.dma_start(out=b2_pp, in_=d["b2_pp"][l])
    ln_s = [g.pppool.tile([128, 8], f32, tag="pp8", bufs=8,
                          name=f"lns{l}_{i}") for i in range(2)]
    ln_b = [g.pppool.tile([128, 8], f32, tag="pp8", bufs=8,
                          name=f"lnb{l}_{i}") for i in range(2)]
    for i in range(2):
        hw.dma_start(out=ln_s[i], in_=d["ln_s_pp"][l, i])
        hw.dma_start(out=ln_b[i], in_=d["ln_b_pp"][l, i])
    bvb = g.bvpool.tile([128, E], f16, tag="bvb", name=f"bvb{l}")
    hw.dma_start(out=bvb, in_=d["bv_row"][l].to_broadcast((128, E)))

    wq_sb = {} if l != 0 else dict(g.wq_prefetch)

    def load_wqkv(gi):
        if gi not in wq_sb:
            wt = g.wpool.tile([128, 4, 2, 2, 512], f8, tag="w",
                              name=f"wqkv{l}_{gi}")
            hw.dma_start(out=wt, in_=d["wqkv"][l, gi])
            wq_sb[gi] = wt
        return wq_sb[gi]

    qk_t = [g.qkpool.tile([128, S], f8, tag="qk", name=f"qk{l}_{t}")
            for t in range(16)]  # q 0..7, k 8..15
    v_t = []
    dv_t = []
    for kcp in range(2):
        vt = g.vpool.tile([128, 2, H, DH + 1], f8, tag="v", name=f"v{l}_{kcp}")
        nc.vector.memset(vt[:, :, :, DH:DH + 1], 1.0)
        v_t.append(vt)
        dvt = g.vpool.tile([128, 2, H, DH + 1], f8, tag="dv",
                           name=f"dv{l}_{kcp}")
        nc.vector.memset(dvt[:, :, :, DH:DH + 1], 0.0)
        dv_t.append(dvt)

    def qkv_chain(gi, mi, X):
        wt = load_wqkv(gi)
        mt = gi * 4 + mi
        ps = g.ps_gemm.tile([128, S], f32, tag="gemm", name=f"qkps{l}")
        _gemm2(g, ps, wt, mi, hp_t, X)
        if gi < 2:  # Q
            nc.scalar.activation(qk_t[mt][:, X], ps[:, X], AF.Identity,
                                 bias=bqkv_pp[:, mt:mt + 1],
                                 scale=float(DEQ / np.sqrt(DH)))
        else:  # K on DVE
            nc.vector.tensor_scalar(qk_t[mt][:, X], ps[:, X], float(DEQ),
                                    bqkv_pp[:, mt:mt + 1], OP.mult, OP.add)

    def v_chain(gi, n):
        wt = load_wqkv(4 + gi)
        ps = g.ps_gemm.tile([128, S], f32, tag="gemm", name=f"vps{l}")
        for cp in range(4):
            xs = hp_t[cp][:, :, n * 128:(n + 1) * 128]
            w8 = wt[:, cp, :, 0, :]
            dw8 = wt[:, cp, :, 1, :]
            mm(ps, xs, w8, start=(cp == 0), stop=False, perf_mode=DR)
            mm(ps, xs, dw8, start=False, stop=(cp == 3), perf_mode=DR)
        tmp = g.tmppool.tile([128, S], f16, tag="vtmp", bufs=2,
                             name=f"vtmp{l}_{gi}_{n}")
        nc.vector.scalar_tensor_tensor(
            tmp, ps, float(DEQ), bvb[:, gi * 512:(gi + 1) * 512],
            OP.mult, OP.add)
        v8s = v_t[n // 2][:, n % 2, gi * 8:(gi + 1) * 8, 0:DH]
        tmpr = tmp.rearrange("p (a b) -> p a b", a=8)
        nc.vector.tensor_copy(v8s, tmpr)
        nc.vector.tensor_sub(dv_t[n // 2][:, n % 2, gi * 8:(gi + 1) * 8, 0:DH],
                             tmpr, v8s)

    # --- attention -----------------------------------------------------------
    ctx16 = {}
    cx_t = [g.cxppool.tile([128, 2, S], f8, tag="cx", name=f"cx{l}_{j}")
            for j in range(NEP)]
    dcx_t = [g.cxppool.tile([128, 2, S], f8, tag="dcx", name=f"dcx{l}_{j}")
             for j in range(NEP)]

    def emit_attn_half(hx, filler):
        X = HALVES[hx]
        nkcp = 1 + hx
        at_q = {}

        def emit_scores(h):
            qt = qk_t[h // 2]
            kt = qk_t[8 + h // 2]
            r0 = (h % 2) * DH
            mt_ = g.maskpool.tile([128, 2 * nkcp, SH], f8,
                                  tag=("mA", "mB")[hx], bufs=2,
                                  name=f"mk{l}_{hx}_{h}")
            hw.dma_start(out=mt_, in_=d["mask"][h, hx, :, 0:2 * nkcp, :])
            spsf = g.ps_att.tile([128, 2, 2, SH], f32, tag="att",
                                 name=f"s{l}_{hx}_{h}")
            sps = spsf[:, 0:nkcp]
            for kcp in range(nkcp):
                mrh = mt_[:, 2 * kcp:2 * kcp + 2, :]
                for kci in range(2):
                    kc = kcp * 2 + kci
                    qlo = max(kc * 128, X.start)
                    mm(sps[:, kcp, kci, :], g.idz[:, kci], mrh,
                       start=True, stop=False, perf_mode=DR)
                    mm(sps[:, kcp, kci, qlo - X.start:],
                       kt[r0:r0 + DH, kc * 128:(kc + 1) * 128],
                       qt[r0:r0 + DH, qlo:X.stop], start=False, stop=True)
            at = g.atpool.tile([128, nkcp, 2, SH], f8,
                               tag=("atA", "atB")[hx], bufs=3 - hx,
                               name=f"a{l}_{hx}_{h}")
            nc.scalar.activation(at, sps, AF.Exp)
            if l == 0 and h == 0 and f"dbg_at{'AB'[hx]}" in d:
                hw.dma_start(out=d[f"dbg_at{'AB'[hx]}"], in_=at)
            at_q[h] = at

        def emit_av(h):
            at = at_q.pop(h)
            cps = g.ps_gemm.tile([128, S], f32, tag="gemm", name=f"c{l}_{hx}")
            for kcp in range(nkcp):
                mm(cps[0:DH + 1, X], v_t[kcp][:, :, h, :], at[:, kcp],
                   start=(kcp == 0), stop=False, perf_mode=DR)
                mm(cps[0:DH + 1, X], dv_t[kcp][:, :, h, :], at[:, kcp],
                   start=False, stop=(kcp == nkcp - 1), perf_mode=DR)
            srow = g.srec.tile([1, SH], f32, tag="sw", bufs=2,
                               name=f"sw{l}_{hx}_{h}")
            nc.scalar.activation(srow, cps[DH:DH + 1, X], AF.Copy)
            rec = g.srec.tile([1, SH], f32, tag="sr", bufs=1,
                              name=f"re{l}_{hx}_{h}")
            nc.vector.reciprocal_approx_fast(out=rec, in_=srow)
            recR = g.recpool.tile([DH, SH], f32, tag="recf",
                                  name=f"rr{l}_{hx}_{h}")
            nc.gpsimd.partition_broadcast(recR, rec, channels=DH)
            j, i = h // 4, (h // 2) % 2
            hh = h % 2
            if j not in ctx16:
                ctx16[j] = g.ctxpool.tile([128, 2, SH], f16, tag="ctx",
                                          name=f"cxt{l}_{hx}_{j}")
            nc.vector.tensor_mul(ctx16[j][hh * DH:(hh + 1) * DH, i, :],
                                 cps[0:DH, X], recR)
            if i == 1 and hh == 1:
                ct = ctx16.pop(j)
                nc.vector.tensor_copy(cx_t[j][:, :, X], ct)
                nc.vector.tensor_sub(dcx_t[j][:, :, X], ct,
                                     cx_t[j][:, :, X])

        fi = iter(filler)
        emit_scores(0)
        emit_scores(1)
        for h in range(2, H):
            emit_scores(h)
            emit_av(h - 2)
            for fn in (next(fi, None), next(fi, None)):
                if fn is not None:
                    fn()
        emit_av(H - 2)
        emit_av(H - 1)
        for fn in fi:
            if fn is not None:
                fn()

    # --- out-proj ------------------------------------------------------------
    wo_sb = {}

    def load_wo(gi):
        if gi not in wo_sb:
            wt = g.wpool.tile([128, 4, 2, 2, 512], f8, tag="w",
                              name=f"wo{l}_{gi}")
            hw.dma_start(out=wt, in_=d["wo8"][l, gi])
            wo_sb[gi] = wt
        return wo_sb[gi]

    r1_t = [g.rpool.tile([128, 2, S], f16, tag="r", name=f"r1_{l}_{j}")
            for j in range(NEP)]

    def wo_chain(gi, mi, X):
        wt = load_wo(gi)
        mt = gi * 4 + mi
        ps = g.ps_gemm.tile([128, S], f32, tag="gemm", name=f"wops{l}")
        _gemm3(g, ps, wt, mi, cx_t, dcx_t, X)
        wo_o = g.tmppool.tile([128, SH], f16, tag="f2o", bufs=2,
                              name=f"woo{l}_{mt}")
        nc.scalar.activation(wo_o, ps[:, X], AF.Identity,
                             bias=bo_pp[:, mt:mt + 1], scale=DEQ)
        nc.vector.tensor_add(r1_t[mt // 2][:, mt % 2, X], wo_o,
                             h_t[mt // 2][:, mt % 2, X])

    # --- emit: QKV (A leads, B lags 2 blocks); attention; Wo -----------------
    with nc.named_scope(f"L{l}_qkv"):
        # A-g0, A-g1, B-g0, A-g2, B-g1, A-g3, B-g2, B-g3, then V half A;
        # V half B (token blocks 2,3) is emitted as attn_A filler.
        seq = [("qk", 0, 0), ("qk", 1, 0), ("qk", 0, 1), ("qk", 2, 0),
               ("qk", 1, 1), ("qk", 3, 0), ("qk", 2, 1), ("qk", 3, 1),
               ("v", 0, 0), ("v", 0, 1), ("v", 1, 0), ("v", 1, 1)]
        for kind, gi, b in seq:
            if kind == "qk":
                for mi in range(4):
                    qkv_chain(gi, mi, HALVES[b])
            else:
                v_chain(gi, b)

    with nc.named_scope(f"L{l}_attn"):
        load_wo(0)
        load_wo(1)
        emit_attn_half(0, [lambda gi=gi, n=n: v_chain(gi, n)
                           for gi in range(2) for n in (2, 3)])
        emit_attn_half(1, [None] * 6 +
                       [lambda gi=gi, mi=mi: wo_chain(gi, mi, HALVES[0])
                        for gi in range(2) for mi in range(4)])

    if l == 0 and "dbg_q0" in d:
        hw.dma_start(out=d["dbg_q0"], in_=qk_t[0])
        hw.dma_start(out=d["dbg_k0"], in_=qk_t[8])
        hw.dma_start(out=d["dbg_v0"], in_=v_t[0])
        hw.dma_start(out=d["dbg_cx0"], in_=cx_t[0])
        hw.dma_start(out=d["dbg_r1"], in_=r1_t[0][:, 0, :])
    h1_t = [g.hpool.tile([128, 2, S], f16, tag="h", name=f"h1_{l}_{j}")
            for j in range(NEP)]
    h1p_t = [g.hppool.tile([128, 2, S], f8, tag="hp", name=f"h1p{l}_{j}")
             for j in range(NEP)]
    dh1p_t = [g.dhppool.tile([128, 2, S], f8, tag="dhp", name=f"dh1p{l}_{j}")
              for j in range(NEP)]
    with nc.named_scope(f"L{l}_ln1a"):
        _ln_half(g, 0, r1_t, ln_s[0], ln_b[0], h1_t, h1p_t, dh1p_t,
                 tag=f"l1{l}")
    with nc.named_scope(f"L{l}_wob"):
        for gi in range(2):
            for mi in range(4):
                wo_chain(gi, mi, HALVES[1])
    with nc.named_scope(f"L{l}_ln1b"):
        _ln_half(g, 1, r1_t, ln_s[0], ln_b[0], h1_t, h1p_t, dh1p_t,
                 tag=f"l1{l}")

    # --- FFN -----------------------------------------------------------------
    with nc.named_scope(f"L{l}_ffn"):
        ffp_t = [g.ffpool.tile([128, 2, S], f8, tag="ff", name=f"ff{l}_{j}")
                 for j in range(NF // 2)]
        dffp_t = [g.ffpool.tile([128, 2, S], f8, tag="dff",
                                name=f"dff{l}_{j}") for j in range(NF // 2)]
        w1_sb = {}

        def load_w1(b):
            if b in w1_sb or b > 15:
                return
            wt = g.w1pool.tile([128, 4, 2, 2, 256], f8, tag="w1",
                               name=f"w1_{l}_{b}")
            hw.dma_start(out=wt, in_=d["w1"][l, b])
            w1_sb[b] = wt

        def ffn1_block(b, X):
            wt = w1_sb[b]
            ft = g.tmppool.tile([128, 2, SH], f16, tag="ffh", bufs=2,
                                name=f"ffh{l}_{b}")
            for mi in range(2):
                mt = 2 * b + mi
                ps = g.ps_gemm.tile([128, S], f32, tag="gemm", name=f"f1ps{l}")
                _gemm3(g, ps, wt, mi, h1p_t, dh1p_t, X)
                nc.scalar.activation(ft[:, mi], ps[:, X], AF.Gelu,
                                     bias=b1_pp[:, mt:mt + 1], scale=DEQ)
            nc.vector.tensor_copy(ffp_t[b][:, :, X], ft)
            nc.gpsimd.tensor_sub(dffp_t[b][:, :, X], ft,
                                 ffp_t[b][:, :, X])

        if l == 0 and "dbg_h1" in d:
            hw.dma_start(out=d["dbg_h1"], in_=h1_t[0][:, 0, :])
        load_w1(0)
        load_w1(1)
        for b in range(16):
            load_w1(b + 1)
            ffn1_block(b, HALVES[0])
            if b >= 3:
                ffn1_block(b - 3, HALVES[1])
        for b in range(13, 16):
            ffn1_block(b, HALVES[1])

        # FFN2: 8 blocks of 128 outputs x 16 cp; A/B pairwise per block
        r2_t = [g.rpool.tile([128, 2, S], f16, tag="r", name=f"r2_{l}_{j}")
                for j in range(NEP)]
        h2_t = [g.hpool.tile([128, 2, S], f16, tag="h", name=f"h2_{l}_{j}")
                for j in range(NEP)]
        h2p_t = [g.hppool.tile([128, 2, S], f8, tag="hp", name=f"h2p{l}_{j}")
                 for j in range(NEP)]
        dh2p_t = [g.dhppool.tile([128, 2, S], f8, tag="dhp",
                                 name=f"dh2p{l}_{j}") for j in range(NEP)]             if l == L - 1 else None
        w2_sb = {}

        def load_w2(b):
            if b in w2_sb or b > 7:
                return
            wt = g.w2pool.tile([128, 16, 2, 2, 128], f8, tag="w2",
                               name=f"w2_{l}_{b}")
            hw.dma_start(out=wt, in_=d["w2"][l, b])
            w2_sb[b] = wt

        def ffn2_block(b, X):
            mt = b
            wt = w2_sb[b]
            ps = g.ps_gemm.tile([128, S], f32, tag="gemm", name=f"f2ps{l}")
            for cp in range(16):
                w8 = wt[:, cp, :, 0, :]
                dw8 = wt[:, cp, :, 1, :]
                mm(ps[:, X], w8, ffp_t[cp][:, :, X], start=(cp == 0),
                   stop=False, perf_mode=DR)
                mm(ps[:, X], dw8, ffp_t[cp][:, :, X], start=False,
                   stop=False, perf_mode=DR)
                mm(ps[:, X], w8, dffp_t[cp][:, :, X], start=False,
                   stop=(cp == 15), perf_mode=DR)
            f2o = g.tmppool.tile([128, SH], f16, tag="f2o", bufs=2,
                                 name=f"f2o{l}_{mt}")
            nc.scalar.activation(f2o, ps[:, X], AF.Identity,
                                 bias=b2_pp[:, mt:mt + 1], scale=DEQ)
            nc.vector.tensor_add(r2_t[mt // 2][:, mt % 2, X], f2o,
                                 h1_t[mt // 2][:, mt % 2, X])

        load_w2(0)
        for b in range(8):
            load_w2(b + 1)
            ffn2_block(b, HALVES[0])
            if b == 7:
                with nc.named_scope(f"L{l}_ln2a"):
                    _ln_half(g, 0, r2_t, ln_s[1], ln_b[1], h2_t, h2p_t,
                             dh2p_t, tag=f"l2{l}", want_dp=(l == L - 1))
            ffn2_block(b, HALVES[1])
        with nc.named_scope(f"L{l}_ln2b"):
            _ln_half(g, 1, r2_t, ln_s[1], ln_b[1], h2_t, h2p_t, dh2p_t,
                     tag=f"l2{l}", want_dp=(l == L - 1))
        if l == 0 and "dbg_ff0" in d:
            hw.dma_start(out=d["dbg_ff0"], in_=ffp_t[0])
            hw.dma_start(out=d["dbg_r2"], in_=r2_t[0][:, 0, :])

    return h2_t, h2p_t, dh2p_t


def _ln_half(g, hx, r_t, s_pp, b_pp, out_t, outp_t, outdp_t, tag="",
             want_dp=True):
    """LayerNorm of half hx on pair tiles: fp8-DR stats, pair-op vector
    chain, quantized pair outputs (residual pairs optional)."""
    nc = g.nc
    mm = nc.tensor.matmul
    X = HALVES[hx]

    r8 = [g.r8pool.tile([128, 2, S], f8, tag="r8", name=f"r8{tag}_{hx}_{j}")
          for j in range(NEP)]
    sq8 = [g.r8pool.tile([128, 2, S], f8, tag="sq8", name=f"sq{tag}_{hx}_{j}")
           for j in range(NEP)]
    for j in range(NEP):
        nc.gpsimd.tensor_copy(r8[j][:, :, X], r_t[j][:, :, X])
        nc.vector.tensor_mul(sq8[j][:, :, X], r_t[j][:, :, X],
                             r_t[j][:, :, X])

    sums_r2 = g.ps_gemm.tile([128, S], f32, tag="gemm", name=f"lsr{tag}_{hx}")
    sums_q2 = g.ps_gemm.tile([128, S], f32, tag="gemm", name=f"lsq{tag}_{hx}")
    sums_r = sums_r2[0:1]
    sums_q = sums_q2[0:1]
    for j in range(NEP):
        mm(sums_r2[:, X], g.ones8, r8[j][:, :, X], start=(j == 0),
           stop=(j == NEP - 1), perf_mode=DR)
    for j in range(NEP):
        mm(sums_q2[:, X], g.ones8, sq8[j][:, :, X], start=(j == 0),
           stop=(j == NEP - 1), perf_mode=DR)

    mean_b = g.smallf.tile([1, SH], f16, tag="sb", bufs=3,
                           name=f"mn{tag}_{hx}")
    nc.vector.tensor_scalar(mean_b, sums_r[:, X], 1.0 / E, None, OP.mult)
    meanR = g.bcpool.tile([128, 2, SH], f16, tag="rec", name=f"mR{tag}_{hx}")
    nc.gpsimd.partition_broadcast(meanR[:, 0], mean_b, channels=128)
    nc.gpsimd.tensor_copy(meanR[:, 1], meanR[:, 0])

    s2 = g.smallf.tile([1, SH], f32, tag="sf", bufs=3, name=f"s2{tag}_{hx}")
    nc.scalar.activation(s2, sums_r[:, X], AF.Square)
    varE = g.smallf.tile([1, SH], f32, tag="sf", bufs=3,
                         name=f"vE{tag}_{hx}")
    nc.vector.scalar_tensor_tensor(varE, s2, -1.0 / E, sums_q[:, X],
                                   OP.mult, OP.add)
    std = g.smallf.tile([1, SH], f32, tag="sf", bufs=3, name=f"st{tag}_{hx}")
    nc.scalar.activation(std, varE, AF.Sqrt, bias=g.eps_t[:1, :],
                         scale=1.0 / E)
    rstd = g.smallf.tile([1, SH], f32, tag="sf", bufs=3,
                         name=f"rs{tag}_{hx}")
    nc.vector.reciprocal_approx_fast(out=rstd, in_=std)
    rstd_b = g.smallf.tile([1, SH], f16, tag="sb", bufs=3,
                           name=f"rb{tag}_{hx}")
    nc.vector.tensor_copy(rstd_b, rstd)
    rstdR = g.bcpool.tile([128, 2, SH], f16, tag="rec", name=f"rR{tag}_{hx}")
    nc.gpsimd.partition_broadcast(rstdR[:, 0], rstd_b, channels=128)
    nc.gpsimd.tensor_copy(rstdR[:, 1], rstdR[:, 0])

    for j in range(NEP):
        t2 = g.tmppool.tile([128, 2, SH], f16, tag="t2", bufs=1,
                            name=f"t2{tag}_{hx}_{j}")
        nc.vector.tensor_sub(t2, r_t[j][:, :, X], meanR)
        t1 = g.tmppool.tile([128, 2, SH], f16, tag="t1", bufs=2,
                            name=f"t1{tag}_{hx}_{j}")
        nc.vector.tensor_mul(t1, t2, rstdR)
        for i in range(2):
            c = 2 * j + i
            nc.vector.tensor_scalar(out_t[j][:, i, X], t1[:, i], 
                                    s_pp[:, c:c + 1], b_pp[:, c:c + 1],
                                    OP.mult, OP.add)
        nc.scalar.activation(outp_t[j][:, :, X], out_t[j][:, :, X], AF.Copy)
        if want_dp:
            nc.gpsimd.tensor_sub(outdp_t[j][:, :, X], out_t[j][:, :, X],
                                 outp_t[j][:, :, X])


def _final(g, h_t, hp_t, dhp_t):
    nc = g.nc
    d = g.d
    mm = nc.tensor.matmul
    hw = nc.sync

    mrstd = g.smallf.tile([1, S], f16, tag="mrs", bufs=1, name="fmr")
    rstdR = g.bcpool.tile([128, S], f16, tag="rec", bufs=2, name="frR")

    for hx, X in enumerate(HALVES):
        sq8 = [g.r8pool.tile([128, 2, S], f8, tag="sq8",
                             name=f"fsq{hx}_{j}") for j in range(NEP)]
        for j in range(NEP):
            nc.vector.tensor_mul(sq8[j][:, :, X], hp_t[j][:, :, X],
                                 hp_t[j][:, :, X])
        sums_r2 = g.ps_gemm.tile([128, S], f32, tag="gemm", name=f"fsr{hx}")
        sums_q2 = g.ps_gemm.tile([128, S], f32, tag="gemm", name=f"fsq{hx}")
        sums_r = sums_r2[0:1]
        sums_q = sums_q2[0:1]
        for j in range(NEP):
            mm(sums_r2[:, X], g.ones8, hp_t[j][:, :, X], start=(j == 0),
               stop=(j == NEP - 1), perf_mode=DR)
        for j in range(NEP):
            mm(sums_q2[:, X], g.ones8, sq8[j][:, :, X], start=(j == 0),
               stop=(j == NEP - 1), perf_mode=DR)
        s2 = g.smallf.tile([1, SH], f32, tag="sf", bufs=3, name=f"fs2{hx}")
        nc.scalar.activation(s2, sums_r[:, X], AF.Square)
        varE = g.smallf.tile([1, SH], f32, tag="sf", bufs=3, name=f"fvE{hx}")
        nc.vector.scalar_tensor_tensor(varE, s2, -1.0 / E, sums_q[:, X],
                                       OP.mult, OP.add)
        std = g.smallf.tile([1, SH], f32, tag="sf", bufs=3, name=f"fst{hx}")
        nc.scalar.activation(std, varE, AF.Sqrt, bias=g.eps_t[:1, :],
                             scale=1.0 / E)
        rstd = g.smallf.tile([1, SH], f32, tag="sf", bufs=3, name=f"frs{hx}")
        nc.vector.reciprocal_approx_fast(out=rstd, in_=std)
        rstd_b = g.smallf.tile([1, SH], f16, tag="sb", bufs=3,
                               name=f"frb{hx}")
        nc.vector.tensor_copy(rstd_b, rstd)
        nc.gpsimd.partition_broadcast(rstdR[:, X], rstd_b, channels=128)
        nc.vector.scalar_tensor_tensor(mrstd[:, X], sums_r[:, X], 1.0 / E,
                                       rstd, OP.mult, OP.mult)
    hw.dma_start(out=d["mrstd"], in_=mrstd)

    for mt in range(NO):
        gi, mi = divmod(mt, 2)
        if mi == 0:
            wt = g.w2pool.tile([128, 4, 2, 2, 256], f8, tag="gw", bufs=2,
                               name=f"gw{gi}")
            hw.dma_start(out=wt, in_=d["genw8"][gi])
            _final.wt = wt
        ps = g.ps_gemm.tile([128, S], f32, tag="gemm", name=f"hd{mt}")
        _gemm3(g, ps, _final.wt, mi, hp_t, dhp_t, slice(0, S))
        gt = g.outpool.tile([128, S], f16, tag="f16out", name=f"gt{mt}")
        nc.vector.scalar_tensor_tensor(gt, ps, float(DEQ), rstdR,
                                       OP.mult, OP.mult)
        hw.dma_start(out=d["logits"][mt], in_=gt)


def _build():
    if "nc" in _CACHE:
        return _CACHE["nc"]
    from contextlib import ExitStack

    nc = bacc.Bacc("TRN2", debug=False)
    d = _declare(nc)
    with tile.TileContext(nc) as tc:
        with ExitStack() as ctx:
            _emit(nc, tc, d, ctx)
    nc.compile()
    _CACHE["nc"] = nc
    return nc


def kernel_internal(inputs, trace=False, trace_kwargs=None):
    shared = _prep_shared(inputs)
    cores = _prep_percore(inputs)
    nc = _build()
    in_maps = []
    for b in range(B):
        m = dict(shared)
        m.update(cores[b])
        in_maps.append(m)
    res = run_bass_kernel_spmd(
        nc, in_maps, core_ids=list(range(B)), trace=trace,
        **(trace_kwargs or {}),
    )
    nws = _CACHE["nws"]
    gb = _CACHE["gb"]
    outs = []
    for b in range(B):
        lo = np.asarray(res.results[b]["logits"], np.float32)  # [10,128,512]
        mr = np.asarray(res.results[b]["mrstd"], np.float32).reshape(S)
        lo = lo.reshape(NO * 128, S)[:VV * VR].T  # [512, 1200]
        lo = lo + mr[:, None] * nws[None, :] + gb[None, :]
        outs.append(lo)
    out = np.stack(outs).astype(np.float32)  # [B, S, 1200]
    return out, res


def kernel(**inputs):
    out, _ = kernel_internal(inputs)
    return out
